# revision 34
# baseline (speedup 1.0000x reference)
"""Cadzow update (batched rank-K truncation + Toeplitz averaging) on 8 trn2 cores.

Data-parallel over the batch of 128 matrices (16 per core). Per matrix:
  A = w1@Sp + w2@Tp + w4*Tp + w3*T
  rank-K via oversampled subspace iteration + host Rayleigh-Ritz:
    K1 (device): G = A^T A; chain G2=(G^2*2^-21), G4, G8, G16 (fp32r matmuls);
      3 rungs V <- orth(G16 V) with a quintic Newton-Schulz Gram conditioner
      (4 matrices packed per 128x128 block-diag tile); ships per matrix
      Gh = V^T G16 V, Bh = V^T V (16x16), Vt = V^T and Wpt = (A V)^T.
    host: robust whitened generalized eig of (Gh, Bh); top-K selector
      Ms = Z10 Z10^T (16x16).
    K2 (device): Tpnew = Wpt^T Ms Vt (both orientations from the small
      factors); Spnew = Sp - Tpnew + avgdiag(2 Tpnew - Sp) where the
      diagonal averaging runs via a shear-DMA layout (diag sums by
      ones-matmul) and the Toeplitz broadcast is read back from a
      mod-511 periodic DRAM buffer with all-positive strides.

All big matmuls run as fp32r (~4x PE throughput at >=256-wide outputs);
the 16x16 Grams / Newton-Schulz stay fp32. Outputs are written in natural
layout (no 4-byte-granular transposed DMA anywhere).
"""
import os
import numpy as np
from contextlib import ExitStack

# The axon ntff profile hook (antenv.axon_hooks) is absent in this image;
# a set BASS_TRACE would crash run_bass_kernel_spmd, so clear it.
os.environ.pop("BASS_TRACE", None)

import concourse.bass as bass
import concourse.bacc as bacc
import concourse.mybir as mybir
from concourse import tile
from concourse.bass_utils import run_bass_kernel_spmd

F32 = mybir.dt.float32
F32R = mybir.dt.float32r
BF16 = mybir.dt.bfloat16
N_CORES = 8
B_FULL = 128
BPC = B_FULL // N_CORES          # 16 matrices per core
R = 256
LA = 16                          # subspace dim
H = 128                          # partitions
GRP = 4                          # matrices packed per 128x128 Gram tile
N_RUNGS = 3
NS_STEPS = 3
MUO = (3.4445, -4.7750, 2.0315)  # quintic NS coefficients
G2_SCALE = 2.0 ** -21

SHEAR_N = 512 * 257              # shear scratch elems per matrix
QBUF_N = 511 * 129               # periodic Toeplitz buffer elems per matrix


def _halfslc(hh, w=R):
    return slice(w * hh, w * hh + w)


def _load_256(nc, dst, src_b):
    """DRAM (256, X) -> SBUF [128, 2X] (row halves side by side)."""
    X = src_b.shape[-1]
    nc.sync.dma_start(out=dst[:, 0:X], in_=src_b[0:H, :])
    nc.sync.dma_start(out=dst[:, X:2 * X], in_=src_b[H:2 * H, :])


def _mm256_wide(nc, psum_pool, out_t, lhs_t, rhs_t, scale=None):
    """out = L^T @ Rhs for 256x256 [128,512]-tiled operands (4 matmuls)."""
    for mh in range(2):
        ps = psum_pool.tile([H, 2 * R], F32, tag="wide")
        for kh in range(2):
            nc.tensor.matmul(
                ps[:, 0:R],
                lhs_t[:, R * kh + H * mh: R * kh + H * mh + H],
                rhs_t[:, R * kh: R * kh + R],
                start=(kh == 0), stop=(kh == 1),
            )
        if scale is None:
            nc.vector.tensor_copy(out_t[:, R * mh: R * mh + R], ps[:, 0:R])
        else:
            nc.scalar.mul(out_t[:, R * mh: R * mh + R], ps[:, 0:R], scale)


def _transpose_256(nc, ptr_pool, out_t, in_t, ident):
    """out = in^T for a 256x256 [128,512] tile (4 PE transposes)."""
    for i in range(2):
        for j in range(2):
            ps = ptr_pool.tile([H, H], F32, tag="tr")
            nc.tensor.transpose(
                ps[:, :],
                in_t[:, R * j + H * i: R * j + H * i + H].bitcast(F32),
                ident[:, :],
            )
            nc.vector.tensor_copy(out_t[:, R * i + H * j: R * i + H * j + H], ps[:, :])


def build_k1(bpc=BPC, n_rungs=N_RUNGS, ns_steps=NS_STEPS):
    nc = bacc.Bacc("TRN2", target_bir_lowering=False)
    sp_d = nc.dram_tensor("sp", [bpc, R, R], F32R, kind="ExternalInput")
    tp_d = nc.dram_tensor("tp", [bpc, R, R], F32R, kind="ExternalInput")
    t_d = nc.dram_tensor("t", [bpc, R, R], F32, kind="ExternalInput")
    w1t_d = nc.dram_tensor("w1t", [R, R], F32R, kind="ExternalInput")
    w2t_d = nc.dram_tensor("w2t", [R, R], F32R, kind="ExternalInput")
    w3_d = nc.dram_tensor("w3", [R, R], F32, kind="ExternalInput")
    w4_d = nc.dram_tensor("w4", [R, R], F32, kind="ExternalInput")
    ident_d = nc.dram_tensor("ident", [H, H], F32, kind="ExternalInput")
    eyema_d = nc.dram_tensor("eyema", [H, H], F32, kind="ExternalInput")  # MUO[0]*I
    blocktr_d = nc.dram_tensor("blocktr", [H, H], F32, kind="ExternalInput")
    seed_d = nc.dram_tensor("seed", [H, 2 * LA], F32R, kind="ExternalInput")
    ghbh_out = nc.dram_tensor("ghbh_out", [bpc, LA, 2 * LA], F32,
                              kind="ExternalOutput")
    vtwpt_out = nc.dram_tensor("vtwpt_out", [bpc, LA, 2 * R], F32R,
                               kind="ExternalOutput")

    n_pack = (bpc + GRP - 1) // GRP
    with tile.TileContext(nc) as tc, ExitStack() as ctx:
        cpool = ctx.enter_context(tc.tile_pool(name="consts", bufs=1))
        inpool = ctx.enter_context(tc.tile_pool(name="inp", bufs=2))
        tpool = ctx.enter_context(tc.tile_pool(name="trans", bufs=2))
        keep = ctx.enter_context(tc.tile_pool(name="keep", bufs=1))
        spool = ctx.enter_context(tc.tile_pool(name="small", bufs=2))
        sone = ctx.enter_context(tc.tile_pool(name="sone", bufs=1))
        pmm = ctx.enter_context(tc.tile_pool(name="pmm", bufs=2, space="PSUM"))
        ptr = ctx.enter_context(tc.tile_pool(name="ptr", bufs=2, space="PSUM"))
        psm = ctx.enter_context(tc.tile_pool(name="psm", bufs=2, space="PSUM"))
        psb = ctx.enter_context(tc.tile_pool(name="psb", bufs=2, space="PSUM"))

        w1t = cpool.tile([H, 2 * R], F32R); _load_256(nc, w1t, w1t_d)
        w2t = cpool.tile([H, 2 * R], F32R); _load_256(nc, w2t, w2t_d)
        w3 = cpool.tile([H, 2 * R], F32); _load_256(nc, w3, w3_d)
        w4 = cpool.tile([H, 2 * R], F32); _load_256(nc, w4, w4_d)
        ident = cpool.tile([H, H], F32)
        nc.sync.dma_start(out=ident[:, :], in_=ident_d[:, :])
        eyema = cpool.tile([H, H], F32)
        nc.sync.dma_start(out=eyema[:, :], in_=eyema_d[:, :])
        blocktr = cpool.tile([H, H], F32)
        nc.sync.dma_start(out=blocktr[:, :], in_=blocktr_d[:, :])
        seed = cpool.tile([H, 2 * LA], F32R)
        nc.sync.dma_start(out=seed[:, :], in_=seed_d[:, :])
        onescol = cpool.tile([H, 1], F32)
        nc.any.memset(onescol[:, :], 1.0)

        ats, s0s, s1s, vs = [], [], [], []
        # ---- phase L: loads, A, A^T, G (per matrix; pipelines across b) ----
        sp2 = tp2 = t2 = None
        for b in range(bpc):
            if b % 2 == 0:
                # one DMA per tensor loads a PAIR of matrices [128, 1024]
                sp2 = inpool.tile([H, 4 * R], F32R, tag="sp")
                tp2 = inpool.tile([H, 4 * R], F32R, tag="tp")
                t2 = inpool.tile([H, 4 * R], F32, tag="t")
                for dst, src, eng in ((sp2, sp_d, nc.sync), (tp2, tp_d, nc.gpsimd),
                                      (t2, t_d, nc.gpsimd)):
                    eng.dma_start(
                        out=dst[:, :].rearrange("p (b hh c) -> p b hh c",
                                                b=2, hh=2, c=R),
                        in_=src[b:b + 2].rearrange("b (hh p) c -> p b hh c", p=H),
                    )
            m = b % 2
            sp_t = sp2[:, 2 * R * m: 2 * R * m + 2 * R]
            tp_t = tp2[:, 2 * R * m: 2 * R * m + 2 * R]
            t_t = t2[:, 2 * R * m: 2 * R * m + 2 * R]

            x1 = tpool.tile([H, 2 * R], F32, tag="x1")
            nc.vector.tensor_mul(x1[:, :], w4[:, :], tp_t[:, :].bitcast(F32))
            x2 = tpool.tile([H, 2 * R], F32, tag="x2")
            nc.vector.tensor_mul(x2[:, :], w3[:, :], t_t[:, :])
            nc.vector.tensor_add(x1[:, :], x1[:, :], x2[:, :])
            a_t = tpool.tile([H, 2 * R], F32R, tag="a")
            for mh in range(2):
                ps = pmm.tile([H, 2 * R], F32, tag="wide")
                for kh in range(2):
                    nc.tensor.matmul(
                        ps[:, 0:R],
                        w1t[:, R * kh + H * mh: R * kh + H * mh + H],
                        sp_t[:, R * kh: R * kh + R],
                        start=(kh == 0), stop=False,
                    )
                for kh in range(2):
                    nc.tensor.matmul(
                        ps[:, 0:R],
                        w2t[:, R * kh + H * mh: R * kh + H * mh + H],
                        tp_t[:, R * kh: R * kh + R],
                        start=False, stop=(kh == 1),
                    )
                nc.vector.tensor_add(
                    a_t[:, R * mh: R * mh + R], ps[:, 0:R],
                    x1[:, R * mh: R * mh + R],
                )
            at_t = keep.tile([H, 2 * R], F32R, tag=f"at{b}")
            _transpose_256(nc, ptr, at_t, a_t, ident)
            s0_t = keep.tile([H, 2 * R], F32R, tag=f"s0_{b}")
            _mm256_wide(nc, pmm, s0_t, a_t, a_t)          # G
            s1_t = keep.tile([H, 2 * R], F32R, tag=f"s1_{b}")
            v_t = keep.tile([H, 2 * LA], F32R, tag=f"v{b}")
            nc.vector.tensor_copy(v_t[:, :], seed[:, :].bitcast(F32))
            ats.append(at_t); s0s.append(s0_t); s1s.append(s1_t); vs.append(v_t)

        # ---- phase C: chain G2..G16, step-major so the PE never stalls ----
        for b in range(bpc):                               # G2 = (G^2)*2^-21
            _mm256_wide(nc, pmm, s1s[b], s0s[b], s0s[b], scale=G2_SCALE)
        for b in range(bpc):                               # G4
            _mm256_wide(nc, pmm, s0s[b], s1s[b], s1s[b])
        for b in range(bpc):                               # G8
            _mm256_wide(nc, pmm, s1s[b], s0s[b], s0s[b])
        for b in range(bpc):                               # G16 -> hs = s0s
            _mm256_wide(nc, pmm, s0s[b], s1s[b], s1s[b])
        hs = s0s

        # ---- phase R: rungs, the 4 packs' NS chains interleaved ----
        a_c, b_c, c_c = MUO
        for r in range(n_rungs):
            mbds, cts, yts = [], [], []
            for p in range(n_pack):
                mbd = sone.tile([H, H], F32, tag=f"mbd{p}")
                nc.any.memset(mbd[:, :], 0.0)
                mbds.append(mbd)
            for b in range(bpc):
                p, sl = b // GRP, (b % GRP) * 32
                yt_ps = psm.tile([LA, R], F32, tag="sm")
                for kh in range(2):
                    nc.tensor.matmul(
                        yt_ps[:, :],
                        vs[b][:, LA * kh: LA * kh + LA],
                        hs[b][:, R * kh: R * kh + R],
                        start=(kh == 0), stop=(kh == 1),
                    )
                yt_t = sone.tile([LA, R], F32, tag=f"ytt{b}")
                nc.vector.tensor_copy(yt_t[:, :], yt_ps[:, :])
                y_t = spool.tile([H, 2 * LA], F32, tag="yy")
                for hh in range(2):
                    tr_ps = ptr.tile([H, LA], F32, tag="tr")
                    nc.tensor.transpose(
                        tr_ps[:, :],
                        yt_t[:, H * hh: H * hh + H],
                        ident[:LA, :LA],
                    )
                    nc.vector.tensor_copy(y_t[:, LA * hh: LA * hh + LA], tr_ps[:, :])
                # gram into the pack's block-diag tile
                m_ps = psb.tile([H, H], F32, tag="smb")
                for kh in range(2):
                    nc.tensor.matmul(
                        m_ps[sl:sl + LA, sl:sl + LA],
                        y_t[:, LA * kh: LA * kh + LA],
                        y_t[:, LA * kh: LA * kh + LA],
                        start=(kh == 0), stop=(kh == 1),
                        tile_position=(0, sl),
                    )
                nc.vector.tensor_copy(
                    mbds[p][sl:sl + LA, sl:sl + LA], m_ps[sl:sl + LA, sl:sl + LA]
                )
                yts.append(yt_t)

            # trace normalization, p-interleaved
            mns, rrvs = [], []
            for p in range(n_pack):
                masked = spool.tile([H, H], F32, tag="masked")
                nc.vector.tensor_mul(masked[:, :], mbds[p][:, :], ident[:, :])
                dr_ps = psb.tile([1, H], F32, tag="smb")
                nc.tensor.matmul(dr_ps[:, :], onescol[:, :], masked[:, :],
                                 start=True, stop=True)
                drow = spool.tile([1, H], F32, tag="drow")
                nc.vector.tensor_copy(drow[:, :], dr_ps[:, :])
                dg_ps = psb.tile([H, 1], F32, tag="smb")
                nc.tensor.transpose(dg_ps[:, :], drow[:, :], ident[:1, :1])
                diag = spool.tile([H, 1], F32, tag="diag")
                nc.vector.tensor_copy(diag[:, :], dg_ps[:, :])
                tr_ps = psb.tile([H, 1], F32, tag="smb")
                nc.tensor.matmul(tr_ps[:, :], blocktr[:, :], diag[:, :],
                                 start=True, stop=True)
                tre = spool.tile([H, 1], F32, tag="tre")
                nc.vector.tensor_scalar_add(tre[:, :], tr_ps[:, :], 1e-30)
                itv = spool.tile([H, 1], F32, tag="itv")
                nc.vector.reciprocal(itv[:, :], tre[:, :])
                sq = spool.tile([H, 1], F32, tag="sq")
                nc.scalar.activation(
                    sq[:, :], tre[:, :], mybir.ActivationFunctionType.Sqrt,
                )
                rrv = sone.tile([H, 1], F32, tag=f"rrv{p}")
                nc.vector.reciprocal(rrv[:, :], sq[:, :])
                mn = sone.tile([H, H], F32, tag=f"mn{p}")
                nc.vector.tensor_scalar_mul(mn[:, :], mbds[p][:, :], itv[:, :])
                mns.append(mn); rrvs.append(rrv)

            # quintic NS, steps interleaved across the 4 packs
            mcurs = list(mns)
            cts = [sone.tile([H, H], F32, tag=f"ct{p}", name=f"ct{p}")
                   for p in range(n_pack)]
            for st in range(ns_steps):
                m2_pss, csts = [], []
                for p in range(n_pack):
                    m2_ps = psb.tile([H, H], F32, tag="smb")
                    nc.tensor.matmul(m2_ps[:, :], mcurs[p][:, :], mcurs[p][:, :],
                                     start=True, stop=True)
                    m2_pss.append(m2_ps)
                for p in range(n_pack):
                    cst = sone.tile([H, H], F32, tag=f"cst{p}")
                    nc.vector.tensor_scalar_mul(cst[:, :], mcurs[p][:, :], b_c)
                    nc.vector.tensor_add(cst[:, :], cst[:, :], eyema[:, :])
                    m2s = spool.tile([H, H], F32, tag="m2s")
                    nc.scalar.mul(m2s[:, :], m2_pss[p][:, :], c_c)
                    nc.vector.tensor_add(cst[:, :], cst[:, :], m2s[:, :])
                    csts.append(cst)
                if st < ns_steps - 1:
                    cms = []
                    for p in range(n_pack):
                        cm_ps = psb.tile([H, H], F32, tag="smb")
                        nc.tensor.matmul(cm_ps[:, :], csts[p][:, :], mcurs[p][:, :],
                                         start=True, stop=True)
                        cm = spool.tile([H, H], F32, tag=f"cm{p}")
                        nc.vector.tensor_copy(cm[:, :], cm_ps[:, :])
                        cms.append(cm)
                    for p in range(n_pack):
                        mn2_ps = psb.tile([H, H], F32, tag="smb")
                        nc.tensor.matmul(mn2_ps[:, :], cms[p][:, :], csts[p][:, :],
                                         start=True, stop=True)
                        mnew = sone.tile([H, H], F32, tag=f"mnew{p}_{st}")
                        nc.vector.tensor_copy(mnew[:, :], mn2_ps[:, :])
                        mcurs[p] = mnew
                for p in range(n_pack):
                    if st == 0:
                        nc.vector.tensor_copy(cts[p][:, :], csts[p][:, :])
                    else:
                        ct_ps = psb.tile([H, H], F32, tag="smb")
                        nc.tensor.matmul(ct_ps[:, :], cts[p][:, :], csts[p][:, :],
                                         start=True, stop=True)
                        nc.vector.tensor_copy(cts[p][:, :], ct_ps[:, :])
            for p in range(n_pack):
                nc.vector.tensor_scalar_mul(cts[p][:, :], cts[p][:, :], rrvs[p][:, :])

            # extract each pack's diag blocks to partition base 0 via an
            # identity matmul (operands share base sl; out lands at base 0)
            ct0s = []
            for p in range(n_pack):
                for kk in range(GRP):
                    sl = kk * 32
                    c0_ps = psb.tile([LA, LA], F32, tag="smb")
                    nc.tensor.matmul(
                        c0_ps[:, :],
                        ident[sl:sl + LA, sl:sl + LA],
                        cts[p][sl:sl + LA, sl:sl + LA],
                        start=True, stop=True,
                        tile_position=(sl, 0),
                    )
                    ct0 = sone.tile([LA, LA], F32, tag=f"ct0_{p}_{kk}",
                                    name=f"ct0_{p}_{kk}")
                    nc.vector.tensor_copy(ct0[:, :], c0_ps[:, :])
                    ct0s.append(ct0)
            # apply: V_b = Y_b @ Ct0_b (all operands at base 0)
            for b in range(bpc):
                for hh in range(2):
                    vp = ptr.tile([H, LA], F32, tag="tr")
                    nc.tensor.matmul(
                        vp[:, :],
                        yts[b][:, H * hh: H * hh + H],
                        ct0s[b][:, :],
                        start=True, stop=True,
                    )
                    nc.vector.tensor_copy(
                        vs[b][:, LA * hh: LA * hh + LA], vp[:, :]
                    )

        # ---- phase O: outputs Gh, Bh, Vt, Wpt (pipelines across b) ----
        for b in range(bpc):
            zt_ps = psm.tile([LA, R], F32, tag="sm")
            for kh in range(2):
                nc.tensor.matmul(
                    zt_ps[:, :],
                    vs[b][:, LA * kh: LA * kh + LA],
                    hs[b][:, R * kh: R * kh + R],
                    start=(kh == 0), stop=(kh == 1),
                )
            zt_t = spool.tile([LA, R], F32, tag="ztt")
            nc.vector.tensor_copy(zt_t[:, :], zt_ps[:, :])
            z_t = spool.tile([H, 2 * LA], F32, tag="zz")
            for hh in range(2):
                tr_ps = ptr.tile([H, LA], F32, tag="tr")
                nc.tensor.transpose(
                    tr_ps[:, :], zt_t[:, H * hh: H * hh + H],
                    ident[:LA, :LA],
                )
                nc.vector.tensor_copy(z_t[:, LA * hh: LA * hh + LA], tr_ps[:, :])
            ghbh_t = spool.tile([LA, 2 * LA], F32, tag="ghbh")
            gh_ps = psb.tile([LA, LA], F32, tag="smb")
            for kh in range(2):
                nc.tensor.matmul(
                    gh_ps[:, :],
                    z_t[:, LA * kh: LA * kh + LA],
                    vs[b][:, LA * kh: LA * kh + LA].bitcast(F32),
                    start=(kh == 0), stop=(kh == 1),
                )
            nc.vector.tensor_copy(ghbh_t[:, 0:LA], gh_ps[:, :])
            bh_ps = psb.tile([LA, LA], F32, tag="smb")
            for kh in range(2):
                nc.tensor.matmul(
                    bh_ps[:, :],
                    vs[b][:, LA * kh: LA * kh + LA].bitcast(F32),
                    vs[b][:, LA * kh: LA * kh + LA].bitcast(F32),
                    start=(kh == 0), stop=(kh == 1),
                )
            nc.vector.tensor_copy(ghbh_t[:, LA:2 * LA], bh_ps[:, :])
            nc.sync.dma_start(out=ghbh_out[b], in_=ghbh_t[:, :])

            vw_t = spool.tile([LA, 2 * R], F32R, tag="vw")
            for hh in range(2):
                tr_ps = psm.tile([LA, H], F32, tag="sm")
                nc.tensor.transpose(
                    tr_ps[:, :],
                    vs[b][:, LA * hh: LA * hh + LA].bitcast(F32),
                    ident[:, :],
                )
                nc.vector.tensor_copy(vw_t[:, H * hh: H * hh + H], tr_ps[:, :])
            wpt_ps = psm.tile([LA, R], F32, tag="sm")
            for kh in range(2):
                nc.tensor.matmul(
                    wpt_ps[:, :],
                    vs[b][:, LA * kh: LA * kh + LA],
                    ats[b][:, R * kh: R * kh + R],
                    start=(kh == 0), stop=(kh == 1),
                )
            nc.vector.tensor_copy(vw_t[:, R:2 * R], wpt_ps[:, :])
            nc.sync.dma_start(out=vtwpt_out[b], in_=vw_t[:, :])
    nc.compile()
    return nc


def build_k2(bpc=BPC):
    nc = bacc.Bacc("TRN2", target_bir_lowering=False)
    sp_d = nc.dram_tensor("sp", [bpc, R, R], F32, kind="ExternalInput")
    # packed per-matrix smalls: [vt | wpt | ms] = [16, 256+256+16]
    vwm_d = nc.dram_tensor("vwm", [bpc, LA, 2 * R + LA], F32R,
                           kind="ExternalInput")
    ident_d = nc.dram_tensor("ident", [H, H], F32, kind="ExternalInput")
    invc_d = nc.dram_tensor("invc", [1, 511], F32, kind="ExternalInput")
    tpn_out = nc.dram_tensor("tpn_out", [bpc, R, R], F32, kind="ExternalOutput")
    spn_out = nc.dram_tensor("spn_out", [bpc, R, R], F32, kind="ExternalOutput")
    scr1 = nc.dram_tensor("scr1", [bpc, SHEAR_N], BF16)
    scrq = nc.dram_tensor("scrq", [bpc, QBUF_N], F32)

    with tile.TileContext(nc) as tc, ExitStack() as ctx:
        cpool = ctx.enter_context(tc.tile_pool(name="consts", bufs=1))
        inpool = ctx.enter_context(tc.tile_pool(name="inp", bufs=2))
        tpool = ctx.enter_context(tc.tile_pool(name="trans", bufs=2))
        zpool = ctx.enter_context(tc.tile_pool(name="zp", bufs=1))
        spool = ctx.enter_context(tc.tile_pool(name="small", bufs=3))
        pmm = ctx.enter_context(tc.tile_pool(name="pmm", bufs=2, space="PSUM"))
        ptr = ctx.enter_context(tc.tile_pool(name="ptr", bufs=2, space="PSUM"))
        psm = ctx.enter_context(tc.tile_pool(name="psm", bufs=2, space="PSUM"))

        ident = cpool.tile([H, H], F32)
        nc.sync.dma_start(out=ident[:, :], in_=ident_d[:, :])
        invc = cpool.tile([1, 511], F32)
        nc.sync.dma_start(out=invc[:, :], in_=invc_d[:, :])
        ones = cpool.tile([H, 1], BF16)
        nc.any.memset(ones[:, :], 1.0)
        onesr = cpool.tile([1, H], BF16)
        nc.any.memset(onesr[:, :], 1.0)

        # two rotating zero-padded staging tiles (pads stay zero across reuse)
        m2zs = []
        for zz in range(2):
            m2z = zpool.tile([H, 1024], BF16, name=f"m2z{zz}")
            nc.any.memset(m2z[:, :], 0.0)
            m2zs.append(m2z)
        # one DMA zeroes the shear-gap head [0,255) of every matrix slot
        nc.sync.dma_start(
            out=scr1[:, 0:255], in_=m2zs[0][0:bpc, 256:511],
        )

        sp2 = None
        for b in range(bpc):
            if b % 2 == 0:
                sp2 = inpool.tile([H, 4 * R], F32, tag="sp")
                nc.sync.dma_start(
                    out=sp2[:, :].rearrange("p (b hh c) -> p b hh c",
                                            b=2, hh=2, c=R),
                    in_=sp_d[b:b + 2].rearrange("b (hh p) c -> p b hh c", p=H),
                )
            m = b % 2
            sp_t = sp2[:, 2 * R * m: 2 * R * m + 2 * R]
            vwm_t = inpool.tile([LA, 2 * R + LA], F32R, tag="vwm")
            nc.gpsimd.dma_start(out=vwm_t[:, :], in_=vwm_d[b])
            vt_t = vwm_t[:, 0:R]
            wpt_t = vwm_t[:, R:2 * R]
            ms_t = vwm_t[:, 2 * R:2 * R + LA]

            # Sp^T via PE transposes
            spt_t = tpool.tile([H, 2 * R], F32, tag="spt")
            _transpose_256f(nc, ptr, spt_t, sp_t, ident)

            # P1 = Ms @ Wpt ; P2 = Ms @ Vt   (Ms symmetric)
            p1_ps = psm.tile([LA, R], F32, tag="sm")
            nc.tensor.matmul(p1_ps[:, :], ms_t, wpt_t, start=True, stop=True)
            p1_t = spool.tile([LA, R], F32R, tag="p1")
            nc.vector.tensor_copy(p1_t[:, :], p1_ps[:, :])
            p2_ps = psm.tile([LA, R], F32, tag="sm")
            nc.tensor.matmul(p2_ps[:, :], ms_t, vt_t, start=True, stop=True)
            p2_t = spool.tile([LA, R], F32R, tag="p2")
            nc.vector.tensor_copy(p2_t[:, :], p2_ps[:, :])

            # TpnewT = V P1 ; Tpnew = W' P2  (fp32r, 256-wide)
            tpnT = tpool.tile([H, 2 * R], F32, tag="tpnT")
            tpn = tpool.tile([H, 2 * R], F32, tag="tpn")
            for hh in range(2):
                ps = pmm.tile([H, 2 * R], F32, tag="wide")
                nc.tensor.matmul(
                    ps[:, 0:R],
                    vt_t[:, H * hh: H * hh + H],
                    p1_t[:, :],
                    start=True, stop=True,
                )
                nc.vector.tensor_copy(tpnT[:, R * hh: R * hh + R], ps[:, 0:R])
                ps2 = pmm.tile([H, 2 * R], F32, tag="wide")
                nc.tensor.matmul(
                    ps2[:, 0:R],
                    wpt_t[:, H * hh: H * hh + H],
                    p2_t[:, :],
                    start=True, stop=True,
                )
                nc.vector.tensor_copy(tpn[:, R * hh: R * hh + R], ps2[:, 0:R])
            # Tpnew out, one DMA (natural layout)
            nc.sync.dma_start(
                out=tpn_out[b].rearrange("(hh p) c -> p hh c", p=H),
                in_=tpn[:, :].rearrange("p (hh c) -> p hh c", hh=2),
            )

            # M2T = 2*TpnewT - SpT into the rotating bf16 staging tile
            m2z = m2zs[b % 2]
            for hh in range(2):
                nc.vector.tensor_scalar_mul(
                    m2z[:, 512 * hh: 512 * hh + R],
                    tpnT[:, R * hh: R * hh + R], 2.0,
                )
                nc.vector.tensor_tensor(
                    out=m2z[:, 512 * hh: 512 * hh + R],
                    in0=m2z[:, 512 * hh: 512 * hh + R],
                    in1=spt_t[:, R * hh: R * hh + R],
                    op=mybir.AluOpType.subtract,
                )
            # shear-write both halves in one DMA (row i at 511*i + 255)
            nc.gpsimd.dma_start(
                out=scr1[b][255: 255 + 511 * 2 * H].rearrange(
                    "(hh p f) -> p hh f", p=H, hh=2),
                in_=m2z[:, :].rearrange("p (hh x) -> p hh x", hh=2)[:, :, 0:511],
            )
            # sheared read (stride 512) both halves in one DMA
            shm = tpool.tile([H, 1022], BF16, tag="shm")
            nc.gpsimd.dma_start(
                out=shm[:, :].rearrange("p (hh f) -> p hh f", hh=2),
                in_=scr1[b][0: 512 * 2 * H].rearrange(
                    "(hh p f) -> p hh f", p=H, hh=2)[:, :, 0:511],
            )
            sums_ps = psm.tile([1, 511], F32, tag="sm3")
            for hh in range(2):
                nc.tensor.matmul(sums_ps[:, :], ones[:, :],
                                 shm[:, 511 * hh: 511 * hh + 511],
                                 start=(hh == 0), stop=(hh == 1))
            avg = spool.tile([1, 511], BF16, tag="avg")
            nc.vector.tensor_mul(avg[:, :], sums_ps[:1, :], invc[:, :])
            avgb_ps = pmm.tile([H, 2 * R], F32, tag="wide")
            nc.tensor.matmul(avgb_ps[:, 0:511], onesr[:, :], avg[:, :],
                             start=True, stop=True)
            avgb = spool.tile([H, 511], F32, tag="avgb")
            nc.vector.tensor_copy(avgb[:, :], avgb_ps[:, 0:511])
            # periodic Q buffer: 128 rows + 1 wrap row of avg at stride 511
            nc.sync.dma_start(
                out=scrq[b][0: 511 * H].rearrange("(p f) -> p f", p=H),
                in_=avgb[:, :],
            )
            nc.sync.dma_start(
                out=scrq[b][511 * H: 511 * (H + 1)].rearrange(
                    "(p f) -> p f", p=1),
                in_=avgb[0:1, :],
            )
            # toepT[p, f] = avg[f - p + 255] via mod-511 reads, one DMA:
            # cols [0,256) = rows 128..255 (base 127), cols [256,512) = rows
            # 0..127 (base 255); bases differ by +128 so a single 2D AP works
            ttT = tpool.tile([H, 2 * R], F32, tag="ttT")
            for hh, base in ((1, 255), (0, 127)):
                nc.sync.dma_start(
                    out=ttT[:, R * hh: R * hh + R],
                    in_=scrq[b][base: base + 510 * H].rearrange(
                        "(p f) -> p f", p=H)[:, 0:R],
                )
            # toep natural = transpose(toepT)  (note halves are swapped in
            # ttT: cols [0,256) hold output rows 128..255)
            ttN = tpool.tile([H, 2 * R], F32, tag="ttN")
            for i in range(2):
                for j in range(2):
                    ps = ptr.tile([H, H], F32, tag="tr")
                    nc.tensor.transpose(
                        ps[:, :],
                        ttT[:, R * (1 - j) + H * i: R * (1 - j) + H * i + H],
                        ident[:, :],
                    )
                    nc.vector.tensor_copy(
                        ttN[:, R * i + H * j: R * i + H * j + H], ps[:, :])
            # Spnew = Sp - Tpnew + toep_nat ; one DMA out
            spn = tpool.tile([H, 2 * R], F32, tag="spn")
            nc.vector.tensor_tensor(
                out=spn[:, :], in0=sp_t, in1=tpn[:, :],
                op=mybir.AluOpType.subtract,
            )
            nc.vector.tensor_add(spn[:, :], spn[:, :], ttN[:, :])
            nc.sync.dma_start(
                out=spn_out[b].rearrange("(hh p) c -> p hh c", p=H),
                in_=spn[:, :].rearrange("p (hh c) -> p hh c", hh=2),
            )
    nc.compile()
    return nc


def _transpose_256f(nc, ptr_pool, out_t, in_t, ident):
    """out = in^T for a 256x256 [128,512] fp32 tile (4 PE transposes)."""
    for i in range(2):
        for j in range(2):
            ps = ptr_pool.tile([H, H], F32, tag="tr")
            nc.tensor.transpose(
                ps[:, :], in_t[:, R * j + H * i: R * j + H * i + H], ident[:, :]
            )
            nc.vector.tensor_copy(out_t[:, R * i + H * j: R * i + H * j + H], ps[:, :])


def _host_consts():
    ident = np.eye(H, dtype=np.float32)
    eyema = (MUO[0] * np.eye(H)).astype(np.float32)
    blocktr = np.zeros((H, H), np.float32)
    for g in range(4):
        blocktr[g * 32: g * 32 + LA, g * 32: g * 32 + 32] = 1.0
    i = np.arange(R, dtype=np.float32)[:, None]
    j = np.arange(LA, dtype=np.float32)[None, :]
    v0 = np.cos(0.37 * (i + 1) * (j + 1) + 0.11 * i).astype(np.float32)
    seed = np.concatenate([v0[0:H, :], v0[H:R, :]], axis=1)  # [128, 32]
    counts = (R - np.abs(np.arange(511) - 255)).astype(np.float32)
    invc = (1.0 / counts)[None, :].astype(np.float32)
    return ident, eyema, blocktr, seed, invc


def _host_bridge(gh, bh, Kv):
    """Robust whitened generalized eig; returns Ms = Z10 Z10^T per matrix."""
    n = gh.shape[0]
    ms = np.zeros((n, LA, LA), np.float32)
    for b in range(n):
        Gs = 0.5 * (gh[b] + gh[b].T).astype(np.float64)
        Bs = 0.5 * (bh[b] + bh[b].T).astype(np.float64)
        lb, Ub = np.linalg.eigh(Bs)
        lmax = max(float(lb.max()), 0.0)
        keep = lb > lmax * 1e-7 if lmax > 0 else lb > -1.0
        if not np.any(keep):
            continue
        Wh = Ub[:, keep] / np.sqrt(np.maximum(lb[keep], 1e-300))[None, :]
        Gw = Wh.T @ Gs @ Wh
        d, Qw = np.linalg.eigh(Gw)
        Z = Wh @ Qw[:, ::-1][:, :Kv]
        ms[b] = (Z @ Z.T).astype(np.float32)
    return ms


def _host_fallback(T, Tp, Sp, w1, w2, w3, w4, Kv):
    """Numpy implementation (used only if the device path fails)."""
    f32 = np.float32
    A = (np.einsum('rk,bkc->brc', w1, Sp) + np.einsum('rk,bkc->brc', w2, Tp)
         + w4[None] * Tp + w3[None] * T).astype(f32)
    G = np.einsum('brc,brd->bcd', A, A).astype(f32)
    d, q = np.linalg.eigh(G.astype(np.float64))
    Vs = q[:, :, ::-1][:, :, :Kv]
    AV = np.einsum('brc,bcl->brl', A.astype(np.float64), Vs)
    Tpnew = np.einsum('brl,bcl->brc', AV, Vs).astype(f32)
    m, n = R, R
    D = m + n - 1
    ii = np.arange(m)[:, None]; jj = np.arange(n)[None, :]
    dd = jj - ii + (m - 1)
    M2 = (2.0 * Tpnew - Sp).astype(f32)
    Z = np.zeros((M2.shape[0], m, D), f32)
    Z[:, ii, dd] = M2
    sums = Z.sum(axis=1)
    counts = (m - np.abs(np.arange(D) - (m - 1))).astype(f32)
    avg = sums / counts
    Spnew = (Sp - Tpnew + avg[:, dd]).astype(f32)
    return (T, Tpnew, Spnew)


_K1 = None
_K2 = None


def _get_kernels():
    global _K1, _K2
    if _K1 is None:
        _K1 = build_k1()
    if _K2 is None:
        _K2 = build_k2()
    return _K1, _K2


def _run_k2(Sp, vt_all, wpt_all, ms_all, nc2=None):
    ident, eyema, blocktr, seed, invc = _host_consts()
    if nc2 is None:
        nc2 = build_k2()
    vwm = np.concatenate([vt_all, wpt_all, ms_all], axis=2)  # [B, 16, 528]
    vwm = np.ascontiguousarray(vwm, dtype=np.float32)
    in_maps = []
    for c in range(N_CORES):
        sl = slice(c * BPC, (c + 1) * BPC)
        in_maps.append({
            "sp": Sp[sl], "vwm": vwm[sl], "ident": ident, "invc": invc,
        })
    r2 = run_bass_kernel_spmd(nc2, in_maps, list(range(N_CORES)))
    LAST_EXEC_NS[1] = r2.exec_time_ns
    res2 = r2.results
    Tpnew = np.concatenate([res2[c]["tpn_out"] for c in range(N_CORES)], axis=0)
    Spnew = np.concatenate([res2[c]["spn_out"] for c in range(N_CORES)], axis=0)
    return Tpnew, Spnew


def _kernel_device(T, Tp, Sp, w1, w2, w3, w4, Kv):
    ident, eyema, blocktr, seed, invc = _host_consts()
    w1t = np.ascontiguousarray(w1.T)
    w2t = np.ascontiguousarray(w2.T)
    nc1, nc2 = _get_kernels()
    in_maps1 = []
    for c in range(N_CORES):
        sl = slice(c * BPC, (c + 1) * BPC)
        in_maps1.append({
            "sp": Sp[sl], "tp": Tp[sl], "t": T[sl],
            "w1t": w1t, "w2t": w2t, "w3": w3, "w4": w4,
            "ident": ident, "eyema": eyema, "blocktr": blocktr, "seed": seed,
        })
    r1 = run_bass_kernel_spmd(nc1, in_maps1, list(range(N_CORES)))
    LAST_EXEC_NS[0] = r1.exec_time_ns
    res1 = r1.results
    ghbh = np.concatenate([res1[c]["ghbh_out"] for c in range(N_CORES)], axis=0)
    vtwpt = np.concatenate([res1[c]["vtwpt_out"] for c in range(N_CORES)], axis=0)
    gh, bh = ghbh[:, :, 0:LA], ghbh[:, :, LA:2 * LA]
    vt_all, wpt_all = vtwpt[:, :, 0:R], vtwpt[:, :, R:2 * R]
    ms_all = _host_bridge(gh, bh, Kv)
    Tpnew, Spnew = _run_k2(Sp, vt_all, wpt_all, ms_all, nc2=nc2)
    return (T, Tpnew, Spnew)


def _kernel_hybrid(T, Tp, Sp, w1, w2, w3, w4, Kv):
    """Host eigensolve for the subspace + device K2 for apply/averaging."""
    f32 = np.float32
    A = (np.einsum('rk,bkc->brc', w1, Sp) + np.einsum('rk,bkc->brc', w2, Tp)
         + w4[None] * Tp + w3[None] * T).astype(f32)
    G = np.einsum('brc,brd->bcd', A, A)
    d, q = np.linalg.eigh(G.astype(np.float64))
    Vs = q[:, :, ::-1][:, :, :Kv]                       # [B, 256, K]
    vt_all = np.zeros((B_FULL, LA, R), f32)
    vt_all[:, :Kv, :] = Vs.transpose(0, 2, 1).astype(f32)
    AV = np.einsum('brc,bcl->brl', A.astype(np.float64), Vs)
    wpt_all = np.zeros((B_FULL, LA, R), f32)
    wpt_all[:, :Kv, :] = AV.transpose(0, 2, 1).astype(f32)
    ms_all = np.zeros((B_FULL, LA, LA), f32)
    ms_all[:, :Kv, :Kv] = np.eye(Kv, dtype=f32)[None]
    Tpnew, Spnew = _run_k2(Sp, vt_all, wpt_all, ms_all)
    return (T, Tpnew, Spnew)


def kernel(T, Tp, Sp, w1, w2, w3, w4, K):
    T = np.ascontiguousarray(np.asarray(T, dtype=np.float32))
    Tp = np.ascontiguousarray(np.asarray(Tp, dtype=np.float32))
    Sp = np.ascontiguousarray(np.asarray(Sp, dtype=np.float32))
    w1 = np.asarray(w1, dtype=np.float32); w2 = np.asarray(w2, dtype=np.float32)
    w3 = np.asarray(w3, dtype=np.float32); w4 = np.asarray(w4, dtype=np.float32)
    Kv = int(np.asarray(K))
    try:
        return _kernel_device(T, Tp, Sp, w1, w2, w3, w4, Kv)
    except Exception:
        import traceback
        traceback.print_exc()
        print("K1 device path failed; host eigensolve + device K2")
    try:
        return _kernel_hybrid(T, Tp, Sp, w1, w2, w3, w4, Kv)
    except Exception:
        import traceback
        traceback.print_exc()
        print("hybrid path failed; full host fallback")
        return _host_fallback(T, Tp, Sp, w1, w2, w3, w4, Kv)


LAST_EXEC_NS = [None, None]


# revision 36
# speedup vs baseline: 1.3493x; 1.3493x over previous
"""Cadzow update (batched rank-K truncation + Toeplitz averaging) on 8 trn2 cores.

Data-parallel over the batch of 128 matrices (16 per core). Per matrix:
  A = w1@Sp + w2@Tp + w4*Tp + w3*T
  rank-K via oversampled subspace iteration + host Rayleigh-Ritz:
    K1 (device): G = A^T A; chain G2=(G^2*2^-21), G4, G8, G16 (fp32r matmuls);
      3 rungs V <- orth(G16 V) with a quintic Newton-Schulz Gram conditioner
      (4 matrices packed per 128x128 block-diag tile); ships per matrix
      Gh = V^T G16 V, Bh = V^T V (16x16), Vt = V^T and Wpt = (A V)^T.
    host: robust whitened generalized eig of (Gh, Bh); top-K selector
      Ms = Z10 Z10^T (16x16).
    K2 (device): Tpnew = Wpt^T Ms Vt (both orientations from the small
      factors); Spnew = Sp - Tpnew + avgdiag(2 Tpnew - Sp) where the
      diagonal averaging runs via a shear-DMA layout (diag sums by
      ones-matmul) and the Toeplitz broadcast is read back from a
      mod-511 periodic DRAM buffer with all-positive strides.

All big matmuls run as fp32r (~4x PE throughput at >=256-wide outputs);
the 16x16 Grams / Newton-Schulz stay fp32. Outputs are written in natural
layout (no 4-byte-granular transposed DMA anywhere).
"""
import os
import numpy as np
from contextlib import ExitStack

# The axon ntff profile hook (antenv.axon_hooks) is absent in this image;
# a set BASS_TRACE would crash run_bass_kernel_spmd, so clear it.
os.environ.pop("BASS_TRACE", None)

import concourse.bass as bass
import concourse.bacc as bacc
import concourse.mybir as mybir
from concourse import tile
from concourse.bass_utils import run_bass_kernel_spmd

F32 = mybir.dt.float32
F32R = mybir.dt.float32r
BF16 = mybir.dt.bfloat16
N_CORES = 8
B_FULL = 128
BPC = B_FULL // N_CORES          # 16 matrices per core
R = 256
LA = 16                          # subspace dim
H = 128                          # partitions
GRP = 4                          # matrices packed per 128x128 Gram tile
N_RUNGS = 3
NS_STEPS = 3
MUO = (3.4445, -4.7750, 2.0315)  # quintic NS coefficients
G2_SCALE = 2.0 ** -21

SHEAR_N = 512 * 257              # shear scratch elems per matrix
QBUF_N = 511 * 129               # periodic Toeplitz buffer elems per matrix


def _halfslc(hh, w=R):
    return slice(w * hh, w * hh + w)


def _load_256(nc, dst, src_b):
    """DRAM (256, X) -> SBUF [128, 2X] (row halves side by side)."""
    X = src_b.shape[-1]
    nc.sync.dma_start(out=dst[:, 0:X], in_=src_b[0:H, :])
    nc.sync.dma_start(out=dst[:, X:2 * X], in_=src_b[H:2 * H, :])


def _mm256_wide(nc, psum_pool, out_t, lhs_t, rhs_t, scale=None):
    """out = L^T @ Rhs for 256x256 [128,512]-tiled operands (4 matmuls)."""
    for mh in range(2):
        ps = psum_pool.tile([H, 2 * R], F32, tag="wide")
        for kh in range(2):
            nc.tensor.matmul(
                ps[:, 0:R],
                lhs_t[:, R * kh + H * mh: R * kh + H * mh + H],
                rhs_t[:, R * kh: R * kh + R],
                start=(kh == 0), stop=(kh == 1),
            )
        if scale is None:
            nc.vector.tensor_copy(out_t[:, R * mh: R * mh + R], ps[:, 0:R])
        else:
            nc.scalar.mul(out_t[:, R * mh: R * mh + R], ps[:, 0:R], scale)


def _transpose_256(nc, ptr_pool, out_t, in_t, ident):
    """out = in^T for a 256x256 [128,512] tile (4 PE transposes)."""
    for i in range(2):
        for j in range(2):
            ps = ptr_pool.tile([H, H], F32, tag="tr")
            nc.tensor.transpose(
                ps[:, :],
                in_t[:, R * j + H * i: R * j + H * i + H].bitcast(F32),
                ident[:, :],
            )
            nc.vector.tensor_copy(out_t[:, R * i + H * j: R * i + H * j + H], ps[:, :])


def build_k1(bpc=BPC, n_rungs=N_RUNGS, ns_steps=NS_STEPS):
    nc = bacc.Bacc("TRN2", target_bir_lowering=False)
    sp_d = nc.dram_tensor("sp", [bpc, R, R], F32R, kind="ExternalInput")
    tp_d = nc.dram_tensor("tp", [bpc, R, R], F32R, kind="ExternalInput")
    t_d = nc.dram_tensor("t", [bpc, R, R], F32, kind="ExternalInput")
    w1t_d = nc.dram_tensor("w1t", [R, R], F32R, kind="ExternalInput")
    w2t_d = nc.dram_tensor("w2t", [R, R], F32R, kind="ExternalInput")
    w3_d = nc.dram_tensor("w3", [R, R], F32, kind="ExternalInput")
    w4_d = nc.dram_tensor("w4", [R, R], F32, kind="ExternalInput")
    ident_d = nc.dram_tensor("ident", [H, H], F32, kind="ExternalInput")
    eyema_d = nc.dram_tensor("eyema", [H, H], F32, kind="ExternalInput")  # MUO[0]*I
    blocktr_d = nc.dram_tensor("blocktr", [H, H], F32, kind="ExternalInput")
    seed_d = nc.dram_tensor("seed", [H, 2 * LA], F32R, kind="ExternalInput")
    ghbh_out = nc.dram_tensor("ghbh_out", [bpc, LA, 2 * LA], F32,
                              kind="ExternalOutput")
    vtwpt_out = nc.dram_tensor("vtwpt_out", [bpc, LA, 2 * R], F32R,
                               kind="ExternalOutput")

    n_pack = (bpc + GRP - 1) // GRP
    with tile.TileContext(nc) as tc, ExitStack() as ctx:
        cpool = ctx.enter_context(tc.tile_pool(name="consts", bufs=1))
        inpool = ctx.enter_context(tc.tile_pool(name="inp", bufs=2))
        tpool = ctx.enter_context(tc.tile_pool(name="trans", bufs=2))
        keep = ctx.enter_context(tc.tile_pool(name="keep", bufs=1))
        spool = ctx.enter_context(tc.tile_pool(name="small", bufs=2))
        sone = ctx.enter_context(tc.tile_pool(name="sone", bufs=1))
        pmm = ctx.enter_context(tc.tile_pool(name="pmm", bufs=2, space="PSUM"))
        ptr = ctx.enter_context(tc.tile_pool(name="ptr", bufs=2, space="PSUM"))
        psm = ctx.enter_context(tc.tile_pool(name="psm", bufs=2, space="PSUM"))
        psb = ctx.enter_context(tc.tile_pool(name="psb", bufs=2, space="PSUM"))

        w1t = cpool.tile([H, 2 * R], F32R); _load_256(nc, w1t, w1t_d)
        w2t = cpool.tile([H, 2 * R], F32R); _load_256(nc, w2t, w2t_d)
        w3 = cpool.tile([H, 2 * R], F32); _load_256(nc, w3, w3_d)
        w4 = cpool.tile([H, 2 * R], F32); _load_256(nc, w4, w4_d)
        ident = cpool.tile([H, H], F32)
        nc.sync.dma_start(out=ident[:, :], in_=ident_d[:, :])
        eyema = cpool.tile([H, H], F32)
        nc.sync.dma_start(out=eyema[:, :], in_=eyema_d[:, :])
        blocktr = cpool.tile([H, H], F32)
        nc.sync.dma_start(out=blocktr[:, :], in_=blocktr_d[:, :])
        seed = cpool.tile([H, 2 * LA], F32R)
        nc.sync.dma_start(out=seed[:, :], in_=seed_d[:, :])
        onescol = cpool.tile([H, 1], F32)
        nc.any.memset(onescol[:, :], 1.0)

        ats, s0s, s1s, vs = [], [], [], []
        # ---- phase L: loads, A, A^T, G (per matrix; pipelines across b) ----
        sp2 = tp2 = t2 = None
        for b in range(bpc):
            if b % 2 == 0:
                # one DMA per tensor loads a PAIR of matrices [128, 1024]
                sp2 = inpool.tile([H, 4 * R], F32R, tag="sp")
                tp2 = inpool.tile([H, 4 * R], F32R, tag="tp")
                t2 = inpool.tile([H, 4 * R], F32, tag="t")
                for dst, src, eng in ((sp2, sp_d, nc.sync), (tp2, tp_d, nc.gpsimd),
                                      (t2, t_d, nc.gpsimd)):
                    eng.dma_start(
                        out=dst[:, :].rearrange("p (b hh c) -> p b hh c",
                                                b=2, hh=2, c=R),
                        in_=src[b:b + 2].rearrange("b (hh p) c -> p b hh c", p=H),
                    )
            m = b % 2
            sp_t = sp2[:, 2 * R * m: 2 * R * m + 2 * R]
            tp_t = tp2[:, 2 * R * m: 2 * R * m + 2 * R]
            t_t = t2[:, 2 * R * m: 2 * R * m + 2 * R]

            x1 = tpool.tile([H, 2 * R], F32, tag="x1")
            nc.vector.tensor_mul(x1[:, :], w4[:, :], tp_t[:, :].bitcast(F32))
            x2 = tpool.tile([H, 2 * R], F32, tag="x2")
            nc.vector.tensor_mul(x2[:, :], w3[:, :], t_t[:, :])
            nc.vector.tensor_add(x1[:, :], x1[:, :], x2[:, :])
            a_t = tpool.tile([H, 2 * R], F32R, tag="a")
            for mh in range(2):
                ps = pmm.tile([H, 2 * R], F32, tag="wide")
                for kh in range(2):
                    nc.tensor.matmul(
                        ps[:, 0:R],
                        w1t[:, R * kh + H * mh: R * kh + H * mh + H],
                        sp_t[:, R * kh: R * kh + R],
                        start=(kh == 0), stop=False,
                    )
                for kh in range(2):
                    nc.tensor.matmul(
                        ps[:, 0:R],
                        w2t[:, R * kh + H * mh: R * kh + H * mh + H],
                        tp_t[:, R * kh: R * kh + R],
                        start=False, stop=(kh == 1),
                    )
                nc.vector.tensor_add(
                    a_t[:, R * mh: R * mh + R], ps[:, 0:R],
                    x1[:, R * mh: R * mh + R],
                )
            at_t = keep.tile([H, 2 * R], F32R, tag=f"at{b}")
            _transpose_256(nc, ptr, at_t, a_t, ident)
            s0_t = keep.tile([H, 2 * R], F32R, tag=f"s0_{b}")
            _mm256_wide(nc, pmm, s0_t, a_t, a_t)          # G
            s1_t = keep.tile([H, 2 * R], F32R, tag=f"s1_{b}")
            v_t = keep.tile([H, 2 * LA], F32R, tag=f"v{b}")
            nc.vector.tensor_copy(v_t[:, :], seed[:, :].bitcast(F32))
            ats.append(at_t); s0s.append(s0_t); s1s.append(s1_t); vs.append(v_t)

        # ---- phase C: chain G2..G16, step-major so the PE never stalls ----
        for b in range(bpc):                               # G2 = (G^2)*2^-21
            _mm256_wide(nc, pmm, s1s[b], s0s[b], s0s[b], scale=G2_SCALE)
        for b in range(bpc):                               # G4
            _mm256_wide(nc, pmm, s0s[b], s1s[b], s1s[b])
        for b in range(bpc):                               # G8
            _mm256_wide(nc, pmm, s1s[b], s0s[b], s0s[b])
        for b in range(bpc):                               # G16 -> hs = s0s
            _mm256_wide(nc, pmm, s0s[b], s1s[b], s1s[b])
        hs = s0s

        # ---- phase R: rungs, the 4 packs' NS chains interleaved ----
        a_c, b_c, c_c = MUO
        for r in range(n_rungs):
            mbds, cts, yts = [], [], []
            for p in range(n_pack):
                mbd = sone.tile([H, H], F32, tag=f"mbd{p}")
                nc.any.memset(mbd[:, :], 0.0)
                mbds.append(mbd)
            for b in range(bpc):
                p, sl = b // GRP, (b % GRP) * 32
                yt_ps = psm.tile([LA, R], F32, tag="sm")
                for kh in range(2):
                    nc.tensor.matmul(
                        yt_ps[:, :],
                        vs[b][:, LA * kh: LA * kh + LA],
                        hs[b][:, R * kh: R * kh + R],
                        start=(kh == 0), stop=(kh == 1),
                    )
                yt_t = sone.tile([LA, R], F32, tag=f"ytt{b}")
                nc.vector.tensor_copy(yt_t[:, :], yt_ps[:, :])
                y_t = spool.tile([H, 2 * LA], F32, tag="yy")
                for hh in range(2):
                    tr_ps = ptr.tile([H, LA], F32, tag="tr")
                    nc.tensor.transpose(
                        tr_ps[:, :],
                        yt_t[:, H * hh: H * hh + H],
                        ident[:LA, :LA],
                    )
                    nc.vector.tensor_copy(y_t[:, LA * hh: LA * hh + LA], tr_ps[:, :])
                # gram into the pack's block-diag tile
                m_ps = psb.tile([H, H], F32, tag="smb")
                for kh in range(2):
                    nc.tensor.matmul(
                        m_ps[sl:sl + LA, sl:sl + LA],
                        y_t[:, LA * kh: LA * kh + LA],
                        y_t[:, LA * kh: LA * kh + LA],
                        start=(kh == 0), stop=(kh == 1),
                        tile_position=(0, sl),
                    )
                nc.vector.tensor_copy(
                    mbds[p][sl:sl + LA, sl:sl + LA], m_ps[sl:sl + LA, sl:sl + LA]
                )
                yts.append(yt_t)

            # trace normalization, p-interleaved
            mns, rrvs = [], []
            for p in range(n_pack):
                masked = spool.tile([H, H], F32, tag="masked")
                nc.vector.tensor_mul(masked[:, :], mbds[p][:, :], ident[:, :])
                dr_ps = psb.tile([1, H], F32, tag="smb")
                nc.tensor.matmul(dr_ps[:, :], onescol[:, :], masked[:, :],
                                 start=True, stop=True)
                drow = spool.tile([1, H], F32, tag="drow")
                nc.vector.tensor_copy(drow[:, :], dr_ps[:, :])
                dg_ps = psb.tile([H, 1], F32, tag="smb")
                nc.tensor.transpose(dg_ps[:, :], drow[:, :], ident[:1, :1])
                diag = spool.tile([H, 1], F32, tag="diag")
                nc.vector.tensor_copy(diag[:, :], dg_ps[:, :])
                tr_ps = psb.tile([H, 1], F32, tag="smb")
                nc.tensor.matmul(tr_ps[:, :], blocktr[:, :], diag[:, :],
                                 start=True, stop=True)
                tre = spool.tile([H, 1], F32, tag="tre")
                nc.vector.tensor_scalar_add(tre[:, :], tr_ps[:, :], 1e-30)
                itv = spool.tile([H, 1], F32, tag="itv")
                nc.vector.reciprocal(itv[:, :], tre[:, :])
                sq = spool.tile([H, 1], F32, tag="sq")
                nc.scalar.activation(
                    sq[:, :], tre[:, :], mybir.ActivationFunctionType.Sqrt,
                )
                rrv = sone.tile([H, 1], F32, tag=f"rrv{p}")
                nc.vector.reciprocal(rrv[:, :], sq[:, :])
                mn = sone.tile([H, H], F32, tag=f"mn{p}")
                nc.vector.tensor_scalar_mul(mn[:, :], mbds[p][:, :], itv[:, :])
                mns.append(mn); rrvs.append(rrv)

            # quintic NS, steps interleaved across the 4 packs
            mcurs = list(mns)
            cts = [sone.tile([H, H], F32, tag=f"ct{p}", name=f"ct{p}")
                   for p in range(n_pack)]
            for st in range(ns_steps):
                m2_pss, csts = [], []
                for p in range(n_pack):
                    m2_ps = psb.tile([H, H], F32, tag="smb")
                    nc.tensor.matmul(m2_ps[:, :], mcurs[p][:, :], mcurs[p][:, :],
                                     start=True, stop=True)
                    m2_pss.append(m2_ps)
                for p in range(n_pack):
                    cst = sone.tile([H, H], F32, tag=f"cst{p}")
                    nc.vector.tensor_scalar_mul(cst[:, :], mcurs[p][:, :], b_c)
                    nc.vector.tensor_add(cst[:, :], cst[:, :], eyema[:, :])
                    m2s = spool.tile([H, H], F32, tag="m2s")
                    nc.scalar.mul(m2s[:, :], m2_pss[p][:, :], c_c)
                    nc.vector.tensor_add(cst[:, :], cst[:, :], m2s[:, :])
                    csts.append(cst)
                if st < ns_steps - 1:
                    cms = []
                    for p in range(n_pack):
                        cm_ps = psb.tile([H, H], F32, tag="smb")
                        nc.tensor.matmul(cm_ps[:, :], csts[p][:, :], mcurs[p][:, :],
                                         start=True, stop=True)
                        cm = spool.tile([H, H], F32, tag=f"cm{p}")
                        nc.vector.tensor_copy(cm[:, :], cm_ps[:, :])
                        cms.append(cm)
                    for p in range(n_pack):
                        mn2_ps = psb.tile([H, H], F32, tag="smb")
                        nc.tensor.matmul(mn2_ps[:, :], cms[p][:, :], csts[p][:, :],
                                         start=True, stop=True)
                        mnew = sone.tile([H, H], F32, tag=f"mnew{p}_{st}")
                        nc.vector.tensor_copy(mnew[:, :], mn2_ps[:, :])
                        mcurs[p] = mnew
                for p in range(n_pack):
                    if st == 0:
                        nc.vector.tensor_copy(cts[p][:, :], csts[p][:, :])
                    else:
                        ct_ps = psb.tile([H, H], F32, tag="smb")
                        nc.tensor.matmul(ct_ps[:, :], cts[p][:, :], csts[p][:, :],
                                         start=True, stop=True)
                        nc.vector.tensor_copy(cts[p][:, :], ct_ps[:, :])
            for p in range(n_pack):
                nc.vector.tensor_scalar_mul(cts[p][:, :], cts[p][:, :], rrvs[p][:, :])

            # extract each pack's diag blocks to partition base 0 via an
            # identity matmul (operands share base sl; out lands at base 0)
            ct0s = []
            for p in range(n_pack):
                for kk in range(GRP):
                    sl = kk * 32
                    c0_ps = psb.tile([LA, LA], F32, tag="smb")
                    nc.tensor.matmul(
                        c0_ps[:, :],
                        ident[sl:sl + LA, sl:sl + LA],
                        cts[p][sl:sl + LA, sl:sl + LA],
                        start=True, stop=True,
                        tile_position=(sl, 0),
                    )
                    ct0 = sone.tile([LA, LA], F32, tag=f"ct0_{p}_{kk}",
                                    name=f"ct0_{p}_{kk}")
                    nc.vector.tensor_copy(ct0[:, :], c0_ps[:, :])
                    ct0s.append(ct0)
            # apply: V_b = Y_b @ Ct0_b (all operands at base 0)
            for b in range(bpc):
                for hh in range(2):
                    vp = ptr.tile([H, LA], F32, tag="tr")
                    nc.tensor.matmul(
                        vp[:, :],
                        yts[b][:, H * hh: H * hh + H],
                        ct0s[b][:, :],
                        start=True, stop=True,
                    )
                    nc.vector.tensor_copy(
                        vs[b][:, LA * hh: LA * hh + LA], vp[:, :]
                    )

        # ---- phase O: outputs Gh, Bh, Vt, Wpt (pipelines across b) ----
        for b in range(bpc):
            zt_ps = psm.tile([LA, R], F32, tag="sm")
            for kh in range(2):
                nc.tensor.matmul(
                    zt_ps[:, :],
                    vs[b][:, LA * kh: LA * kh + LA],
                    hs[b][:, R * kh: R * kh + R],
                    start=(kh == 0), stop=(kh == 1),
                )
            zt_t = spool.tile([LA, R], F32, tag="ztt")
            nc.vector.tensor_copy(zt_t[:, :], zt_ps[:, :])
            z_t = spool.tile([H, 2 * LA], F32, tag="zz")
            for hh in range(2):
                tr_ps = ptr.tile([H, LA], F32, tag="tr")
                nc.tensor.transpose(
                    tr_ps[:, :], zt_t[:, H * hh: H * hh + H],
                    ident[:LA, :LA],
                )
                nc.vector.tensor_copy(z_t[:, LA * hh: LA * hh + LA], tr_ps[:, :])
            ghbh_t = spool.tile([LA, 2 * LA], F32, tag="ghbh")
            gh_ps = psb.tile([LA, LA], F32, tag="smb")
            for kh in range(2):
                nc.tensor.matmul(
                    gh_ps[:, :],
                    z_t[:, LA * kh: LA * kh + LA],
                    vs[b][:, LA * kh: LA * kh + LA].bitcast(F32),
                    start=(kh == 0), stop=(kh == 1),
                )
            nc.vector.tensor_copy(ghbh_t[:, 0:LA], gh_ps[:, :])
            bh_ps = psb.tile([LA, LA], F32, tag="smb")
            for kh in range(2):
                nc.tensor.matmul(
                    bh_ps[:, :],
                    vs[b][:, LA * kh: LA * kh + LA].bitcast(F32),
                    vs[b][:, LA * kh: LA * kh + LA].bitcast(F32),
                    start=(kh == 0), stop=(kh == 1),
                )
            nc.vector.tensor_copy(ghbh_t[:, LA:2 * LA], bh_ps[:, :])
            nc.sync.dma_start(out=ghbh_out[b], in_=ghbh_t[:, :])

            vw_t = spool.tile([LA, 2 * R], F32R, tag="vw")
            for hh in range(2):
                tr_ps = psm.tile([LA, H], F32, tag="sm")
                nc.tensor.transpose(
                    tr_ps[:, :],
                    vs[b][:, LA * hh: LA * hh + LA].bitcast(F32),
                    ident[:, :],
                )
                nc.vector.tensor_copy(vw_t[:, H * hh: H * hh + H], tr_ps[:, :])
            wpt_ps = psm.tile([LA, R], F32, tag="sm")
            for kh in range(2):
                nc.tensor.matmul(
                    wpt_ps[:, :],
                    vs[b][:, LA * kh: LA * kh + LA],
                    ats[b][:, R * kh: R * kh + R],
                    start=(kh == 0), stop=(kh == 1),
                )
            nc.vector.tensor_copy(vw_t[:, R:2 * R], wpt_ps[:, :])
            nc.sync.dma_start(out=vtwpt_out[b], in_=vw_t[:, :])
    nc.compile()
    return nc


def build_k2(bpc=BPC, stage=3):
    nc = bacc.Bacc("TRN2", target_bir_lowering=False)
    sp_d = nc.dram_tensor("sp", [bpc, R, R], F32, kind="ExternalInput")
    # packed per-matrix smalls: [vt | wpt | ms] = [16, 256+256+16]
    vwm_d = nc.dram_tensor("vwm", [bpc, LA, 2 * R + LA], F32R,
                           kind="ExternalInput")
    ident_d = nc.dram_tensor("ident", [H, H], F32, kind="ExternalInput")
    invc_d = nc.dram_tensor("invc", [1, 511], F32, kind="ExternalInput")
    tpn_out = nc.dram_tensor("tpn_out", [bpc, R, R], F32, kind="ExternalOutput")
    spn_out = nc.dram_tensor("spn_out", [bpc, R, R], F32, kind="ExternalOutput")
    scr1 = nc.dram_tensor("scr1", [bpc, SHEAR_N], BF16)
    scrq = nc.dram_tensor("scrq", [bpc, QBUF_N], F32)

    with tile.TileContext(nc) as tc, ExitStack() as ctx:
        cpool = ctx.enter_context(tc.tile_pool(name="consts", bufs=1))
        inpool = ctx.enter_context(tc.tile_pool(name="inp", bufs=2))
        tpool = ctx.enter_context(tc.tile_pool(name="trans", bufs=2))
        keep = ctx.enter_context(tc.tile_pool(name="keep", bufs=1))
        spool = ctx.enter_context(tc.tile_pool(name="small", bufs=3))
        pmm = ctx.enter_context(tc.tile_pool(name="pmm", bufs=2, space="PSUM"))
        ptr = ctx.enter_context(tc.tile_pool(name="ptr", bufs=2, space="PSUM"))
        psm = ctx.enter_context(tc.tile_pool(name="psm", bufs=2, space="PSUM"))

        ident = cpool.tile([H, H], F32)
        nc.sync.dma_start(out=ident[:, :], in_=ident_d[:, :])
        invc = cpool.tile([1, 511], F32)
        nc.sync.dma_start(out=invc[:, :], in_=invc_d[:, :])
        ones = cpool.tile([H, 1], BF16)
        nc.any.memset(ones[:, :], 1.0)
        onesr = cpool.tile([1, H], BF16)
        nc.any.memset(onesr[:, :], 1.0)

        # per-matrix zero-padded staging tiles (pads stay zero) + results
        m2zs, tpns, sps = [], [], []
        for b in range(bpc):
            m2z = keep.tile([H, 1024], BF16, tag=f"m2z{b}", name=f"m2z{b}")
            nc.any.memset(m2z[:, :], 0.0)
            m2zs.append(m2z)
        # one DMA zeroes the shear-gap head [0,255) of every matrix slot
        nc.sync.dma_start(out=scr1[:, 0:255], in_=m2zs[0][0:bpc, 256:511])

        # ---- phase A: tpn + shear writes (pipelines across b) ----
        sp2 = None
        for b in range(bpc):
            if b % 2 == 0:
                sp2 = keep.tile([H, 4 * R], F32, tag=f"sp{b}", name=f"sp{b}")
                nc.sync.dma_start(
                    out=sp2[:, :].rearrange("p (b hh c) -> p b hh c",
                                            b=2, hh=2, c=R),
                    in_=sp_d[b:b + 2].rearrange("b (hh p) c -> p b hh c", p=H),
                )
                sps.append(sp2)
            m = b % 2
            sp_t = sp2[:, 2 * R * m: 2 * R * m + 2 * R]
            vwm_t = inpool.tile([LA, 2 * R + LA], F32R, tag="vwm")
            nc.gpsimd.dma_start(out=vwm_t[:, :], in_=vwm_d[b])
            vt_t = vwm_t[:, 0:R]
            wpt_t = vwm_t[:, R:2 * R]
            ms_t = vwm_t[:, 2 * R:2 * R + LA]

            # Sp^T via PE transposes
            spt_t = tpool.tile([H, 2 * R], F32, tag="spt")
            _transpose_256f(nc, ptr, spt_t, sp_t, ident)

            # P1 = Ms @ Wpt ; P2 = Ms @ Vt   (Ms symmetric)
            p1_ps = psm.tile([LA, R], F32, tag="sm")
            nc.tensor.matmul(p1_ps[:, :], ms_t, wpt_t, start=True, stop=True)
            p1_t = spool.tile([LA, R], F32R, tag="p1")
            nc.vector.tensor_copy(p1_t[:, :], p1_ps[:, :])
            p2_ps = psm.tile([LA, R], F32, tag="sm")
            nc.tensor.matmul(p2_ps[:, :], ms_t, vt_t, start=True, stop=True)
            p2_t = spool.tile([LA, R], F32R, tag="p2")
            nc.vector.tensor_copy(p2_t[:, :], p2_ps[:, :])

            # TpnewT = V P1 ; Tpnew = W' P2  (fp32r, 256-wide)
            tpnT = tpool.tile([H, 2 * R], F32, tag="tpnT")
            tpn = keep.tile([H, 2 * R], F32, tag=f"tpn{b}", name=f"tpn{b}")
            for hh in range(2):
                ps = pmm.tile([H, 2 * R], F32, tag="wide")
                nc.tensor.matmul(
                    ps[:, 0:R],
                    vt_t[:, H * hh: H * hh + H],
                    p1_t[:, :],
                    start=True, stop=True,
                )
                nc.vector.tensor_copy(tpnT[:, R * hh: R * hh + R], ps[:, 0:R])
                ps2 = pmm.tile([H, 2 * R], F32, tag="wide")
                nc.tensor.matmul(
                    ps2[:, 0:R],
                    wpt_t[:, H * hh: H * hh + H],
                    p2_t[:, :],
                    start=True, stop=True,
                )
                nc.vector.tensor_copy(tpn[:, R * hh: R * hh + R], ps2[:, 0:R])
            tpns.append(tpn)
            # Tpnew out, one DMA (natural layout)
            nc.sync.dma_start(
                out=tpn_out[b].rearrange("(hh p) c -> p hh c", p=H),
                in_=tpn[:, :].rearrange("p (hh c) -> p hh c", hh=2),
            )
            if stage < 2:
                continue
            # M2T = 2*TpnewT - SpT into this matrix's bf16 staging tile
            m2z = m2zs[b]
            for hh in range(2):
                nc.vector.tensor_scalar_mul(
                    m2z[:, 512 * hh: 512 * hh + R],
                    tpnT[:, R * hh: R * hh + R], 2.0,
                )
                nc.vector.tensor_tensor(
                    out=m2z[:, 512 * hh: 512 * hh + R],
                    in0=m2z[:, 512 * hh: 512 * hh + R],
                    in1=spt_t[:, R * hh: R * hh + R],
                    op=mybir.AluOpType.subtract,
                )
            # shear-write both halves in one DMA (row i at 511*i + 255)
            nc.gpsimd.dma_start(
                out=scr1[b][255: 255 + 511 * 2 * H].rearrange(
                    "(hh p f) -> p hh f", p=H, hh=2),
                in_=m2z[:, :].rearrange("p (hh x) -> p hh x", hh=2)[:, :, 0:511],
            )

        # ---- phase B: diagonal sums -> periodic Q buffer ----
        if stage >= 2:
            for b in range(bpc):
                shm = tpool.tile([H, 1022], BF16, tag="shm")
                nc.gpsimd.dma_start(
                    out=shm[:, :].rearrange("p (hh f) -> p hh f", hh=2),
                    in_=scr1[b][0: 512 * 2 * H].rearrange(
                        "(hh p f) -> p hh f", p=H, hh=2)[:, :, 0:511],
                )
                sums_ps = psm.tile([1, 511], F32, tag="sm3")
                for hh in range(2):
                    nc.tensor.matmul(sums_ps[:, :], ones[:, :],
                                     shm[:, 511 * hh: 511 * hh + 511],
                                     start=(hh == 0), stop=(hh == 1))
                avg = spool.tile([1, 511], BF16, tag="avg")
                nc.vector.tensor_mul(avg[:, :], sums_ps[:1, :], invc[:, :])
                avgb_ps = pmm.tile([H, 2 * R], F32, tag="wide")
                nc.tensor.matmul(avgb_ps[:, 0:511], onesr[:, :], avg[:, :],
                                 start=True, stop=True)
                avgb = spool.tile([H, 511], F32, tag="avgb")
                nc.vector.tensor_copy(avgb[:, :], avgb_ps[:, 0:511])
                # periodic Q: 128 rows + 1 wrap row of avg at stride 511
                nc.sync.dma_start(
                    out=scrq[b][0: 511 * H].rearrange("(p f) -> p f", p=H),
                    in_=avgb[:, :],
                )
                nc.sync.dma_start(
                    out=scrq[b][511 * H: 511 * (H + 1)].rearrange(
                        "(p f) -> p f", p=1),
                    in_=avgb[0:1, :],
                )

        # ---- phase C: Toeplitz + Spnew ----
        if stage >= 3:
            for b in range(bpc):
                sp_t = sps[b // 2][:, 2 * R * (b % 2): 2 * R * (b % 2) + 2 * R]
                # toepT[p, f] = avg[f - p + 255] via mod-511 reads:
                # cols [0,256) = output rows 128..255 (base 127),
                # cols [256,512) = rows 0..127 (base 255)
                ttT = tpool.tile([H, 2 * R], F32, tag="ttT")
                for hh, base in ((1, 255), (0, 127)):
                    nc.gpsimd.dma_start(
                        out=ttT[:, R * hh: R * hh + R],
                        in_=scrq[b][base: base + 510 * H].rearrange(
                            "(p f) -> p f", p=H)[:, 0:R],
                    )
                # toep natural = transpose(toepT) (halves swapped in ttT)
                ttN = tpool.tile([H, 2 * R], F32, tag="ttN")
                for i in range(2):
                    for j in range(2):
                        ps = ptr.tile([H, H], F32, tag="tr")
                        nc.tensor.transpose(
                            ps[:, :],
                            ttT[:, R * (1 - j) + H * i: R * (1 - j) + H * i + H],
                            ident[:, :],
                        )
                        nc.vector.tensor_copy(
                            ttN[:, R * i + H * j: R * i + H * j + H], ps[:, :])
                # Spnew = Sp - Tpnew + toep_nat ; one DMA out
                spn = tpool.tile([H, 2 * R], F32, tag="spn")
                nc.vector.tensor_tensor(
                    out=spn[:, :], in0=sp_t, in1=tpns[b][:, :],
                    op=mybir.AluOpType.subtract,
                )
                nc.vector.tensor_add(spn[:, :], spn[:, :], ttN[:, :])
                nc.sync.dma_start(
                    out=spn_out[b].rearrange("(hh p) c -> p hh c", p=H),
                    in_=spn[:, :].rearrange("p (hh c) -> p hh c", hh=2),
                )
    nc.compile()
    return nc


def _transpose_256f(nc, ptr_pool, out_t, in_t, ident):
    """out = in^T for a 256x256 [128,512] fp32 tile (4 PE transposes)."""
    for i in range(2):
        for j in range(2):
            ps = ptr_pool.tile([H, H], F32, tag="tr")
            nc.tensor.transpose(
                ps[:, :], in_t[:, R * j + H * i: R * j + H * i + H], ident[:, :]
            )
            nc.vector.tensor_copy(out_t[:, R * i + H * j: R * i + H * j + H], ps[:, :])


def _host_consts():
    ident = np.eye(H, dtype=np.float32)
    eyema = (MUO[0] * np.eye(H)).astype(np.float32)
    blocktr = np.zeros((H, H), np.float32)
    for g in range(4):
        blocktr[g * 32: g * 32 + LA, g * 32: g * 32 + 32] = 1.0
    i = np.arange(R, dtype=np.float32)[:, None]
    j = np.arange(LA, dtype=np.float32)[None, :]
    v0 = np.cos(0.37 * (i + 1) * (j + 1) + 0.11 * i).astype(np.float32)
    seed = np.concatenate([v0[0:H, :], v0[H:R, :]], axis=1)  # [128, 32]
    counts = (R - np.abs(np.arange(511) - 255)).astype(np.float32)
    invc = (1.0 / counts)[None, :].astype(np.float32)
    return ident, eyema, blocktr, seed, invc


def _host_bridge(gh, bh, Kv):
    """Robust whitened generalized eig; returns Ms = Z10 Z10^T per matrix."""
    n = gh.shape[0]
    ms = np.zeros((n, LA, LA), np.float32)
    for b in range(n):
        Gs = 0.5 * (gh[b] + gh[b].T).astype(np.float64)
        Bs = 0.5 * (bh[b] + bh[b].T).astype(np.float64)
        lb, Ub = np.linalg.eigh(Bs)
        lmax = max(float(lb.max()), 0.0)
        keep = lb > lmax * 1e-7 if lmax > 0 else lb > -1.0
        if not np.any(keep):
            continue
        Wh = Ub[:, keep] / np.sqrt(np.maximum(lb[keep], 1e-300))[None, :]
        Gw = Wh.T @ Gs @ Wh
        d, Qw = np.linalg.eigh(Gw)
        Z = Wh @ Qw[:, ::-1][:, :Kv]
        ms[b] = (Z @ Z.T).astype(np.float32)
    return ms


def _host_fallback(T, Tp, Sp, w1, w2, w3, w4, Kv):
    """Numpy implementation (used only if the device path fails)."""
    f32 = np.float32
    A = (np.einsum('rk,bkc->brc', w1, Sp) + np.einsum('rk,bkc->brc', w2, Tp)
         + w4[None] * Tp + w3[None] * T).astype(f32)
    G = np.einsum('brc,brd->bcd', A, A).astype(f32)
    d, q = np.linalg.eigh(G.astype(np.float64))
    Vs = q[:, :, ::-1][:, :, :Kv]
    AV = np.einsum('brc,bcl->brl', A.astype(np.float64), Vs)
    Tpnew = np.einsum('brl,bcl->brc', AV, Vs).astype(f32)
    m, n = R, R
    D = m + n - 1
    ii = np.arange(m)[:, None]; jj = np.arange(n)[None, :]
    dd = jj - ii + (m - 1)
    M2 = (2.0 * Tpnew - Sp).astype(f32)
    Z = np.zeros((M2.shape[0], m, D), f32)
    Z[:, ii, dd] = M2
    sums = Z.sum(axis=1)
    counts = (m - np.abs(np.arange(D) - (m - 1))).astype(f32)
    avg = sums / counts
    Spnew = (Sp - Tpnew + avg[:, dd]).astype(f32)
    return (T, Tpnew, Spnew)


_K1 = None
_K2 = None


def _get_kernels():
    global _K1, _K2
    if _K1 is None:
        _K1 = build_k1()
    if _K2 is None:
        _K2 = build_k2()
    return _K1, _K2


def _run_k2(Sp, vt_all, wpt_all, ms_all, nc2=None):
    ident, eyema, blocktr, seed, invc = _host_consts()
    if nc2 is None:
        nc2 = build_k2()
    vwm = np.concatenate([vt_all, wpt_all, ms_all], axis=2)  # [B, 16, 528]
    vwm = np.ascontiguousarray(vwm, dtype=np.float32)
    in_maps = []
    for c in range(N_CORES):
        sl = slice(c * BPC, (c + 1) * BPC)
        in_maps.append({
            "sp": Sp[sl], "vwm": vwm[sl], "ident": ident, "invc": invc,
        })
    r2 = run_bass_kernel_spmd(nc2, in_maps, list(range(N_CORES)))
    LAST_EXEC_NS[1] = r2.exec_time_ns
    res2 = r2.results
    Tpnew = np.concatenate([res2[c]["tpn_out"] for c in range(N_CORES)], axis=0)
    Spnew = np.concatenate([res2[c]["spn_out"] for c in range(N_CORES)], axis=0)
    return Tpnew, Spnew


def _kernel_device(T, Tp, Sp, w1, w2, w3, w4, Kv):
    ident, eyema, blocktr, seed, invc = _host_consts()
    w1t = np.ascontiguousarray(w1.T)
    w2t = np.ascontiguousarray(w2.T)
    nc1, nc2 = _get_kernels()
    in_maps1 = []
    for c in range(N_CORES):
        sl = slice(c * BPC, (c + 1) * BPC)
        in_maps1.append({
            "sp": Sp[sl], "tp": Tp[sl], "t": T[sl],
            "w1t": w1t, "w2t": w2t, "w3": w3, "w4": w4,
            "ident": ident, "eyema": eyema, "blocktr": blocktr, "seed": seed,
        })
    r1 = run_bass_kernel_spmd(nc1, in_maps1, list(range(N_CORES)))
    LAST_EXEC_NS[0] = r1.exec_time_ns
    res1 = r1.results
    ghbh = np.concatenate([res1[c]["ghbh_out"] for c in range(N_CORES)], axis=0)
    vtwpt = np.concatenate([res1[c]["vtwpt_out"] for c in range(N_CORES)], axis=0)
    gh, bh = ghbh[:, :, 0:LA], ghbh[:, :, LA:2 * LA]
    vt_all, wpt_all = vtwpt[:, :, 0:R], vtwpt[:, :, R:2 * R]
    ms_all = _host_bridge(gh, bh, Kv)
    Tpnew, Spnew = _run_k2(Sp, vt_all, wpt_all, ms_all, nc2=nc2)
    return (T, Tpnew, Spnew)


def _kernel_hybrid(T, Tp, Sp, w1, w2, w3, w4, Kv):
    """Host eigensolve for the subspace + device K2 for apply/averaging."""
    f32 = np.float32
    A = (np.einsum('rk,bkc->brc', w1, Sp) + np.einsum('rk,bkc->brc', w2, Tp)
         + w4[None] * Tp + w3[None] * T).astype(f32)
    G = np.einsum('brc,brd->bcd', A, A)
    d, q = np.linalg.eigh(G.astype(np.float64))
    Vs = q[:, :, ::-1][:, :, :Kv]                       # [B, 256, K]
    vt_all = np.zeros((B_FULL, LA, R), f32)
    vt_all[:, :Kv, :] = Vs.transpose(0, 2, 1).astype(f32)
    AV = np.einsum('brc,bcl->brl', A.astype(np.float64), Vs)
    wpt_all = np.zeros((B_FULL, LA, R), f32)
    wpt_all[:, :Kv, :] = AV.transpose(0, 2, 1).astype(f32)
    ms_all = np.zeros((B_FULL, LA, LA), f32)
    ms_all[:, :Kv, :Kv] = np.eye(Kv, dtype=f32)[None]
    Tpnew, Spnew = _run_k2(Sp, vt_all, wpt_all, ms_all)
    return (T, Tpnew, Spnew)


def kernel(T, Tp, Sp, w1, w2, w3, w4, K):
    T = np.ascontiguousarray(np.asarray(T, dtype=np.float32))
    Tp = np.ascontiguousarray(np.asarray(Tp, dtype=np.float32))
    Sp = np.ascontiguousarray(np.asarray(Sp, dtype=np.float32))
    w1 = np.asarray(w1, dtype=np.float32); w2 = np.asarray(w2, dtype=np.float32)
    w3 = np.asarray(w3, dtype=np.float32); w4 = np.asarray(w4, dtype=np.float32)
    Kv = int(np.asarray(K))
    try:
        return _kernel_device(T, Tp, Sp, w1, w2, w3, w4, Kv)
    except Exception:
        import traceback
        traceback.print_exc()
        print("K1 device path failed; host eigensolve + device K2")
    try:
        return _kernel_hybrid(T, Tp, Sp, w1, w2, w3, w4, Kv)
    except Exception:
        import traceback
        traceback.print_exc()
        print("hybrid path failed; full host fallback")
        return _host_fallback(T, Tp, Sp, w1, w2, w3, w4, Kv)


LAST_EXEC_NS = [None, None]


# revision 38
# speedup vs baseline: 1.6919x; 1.2539x over previous
"""Cadzow update (batched rank-K truncation + Toeplitz averaging) on 8 trn2 cores.

Data-parallel over the batch of 128 matrices (16 per core). Per matrix:
  A = w1@Sp + w2@Tp + w4*Tp + w3*T
  rank-K via oversampled subspace iteration + host Rayleigh-Ritz:
    K1 (device): G = A^T A; chain G2=(G^2*2^-21), G4, G8, G16 (fp32r matmuls);
      3 rungs V <- orth(G16 V) with a quintic Newton-Schulz Gram conditioner
      (4 matrices packed per 128x128 block-diag tile); ships per matrix
      Gh = V^T G16 V, Bh = V^T V (16x16), Vt = V^T and Wpt = (A V)^T.
    host: robust whitened generalized eig of (Gh, Bh); top-K selector
      Ms = Z10 Z10^T (16x16).
    K2 (device): Tpnew = Wpt^T Ms Vt (both orientations from the small
      factors); Spnew = Sp - Tpnew + avgdiag(2 Tpnew - Sp) where the
      diagonal averaging runs via a shear-DMA layout (diag sums by
      ones-matmul) and the Toeplitz broadcast is read back from a
      mod-511 periodic DRAM buffer with all-positive strides.

All big matmuls run as fp32r (~4x PE throughput at >=256-wide outputs);
the 16x16 Grams / Newton-Schulz stay fp32. Outputs are written in natural
layout (no 4-byte-granular transposed DMA anywhere).
"""
import os
import numpy as np
from contextlib import ExitStack

# The axon ntff profile hook (antenv.axon_hooks) is absent in this image;
# a set BASS_TRACE would crash run_bass_kernel_spmd, so clear it.
os.environ.pop("BASS_TRACE", None)

import concourse.bass as bass
import concourse.bacc as bacc
import concourse.mybir as mybir
from concourse import tile
from concourse.bass_utils import run_bass_kernel_spmd

F32 = mybir.dt.float32
F32R = mybir.dt.float32r
BF16 = mybir.dt.bfloat16
N_CORES = 8
B_FULL = 128
BPC = B_FULL // N_CORES          # 16 matrices per core
R = 256
LA = 32                          # subspace dim (oversampled, 4x32 pack)
H = 128                          # partitions
GRP = 4                          # matrices packed per 128x128 Gram tile
N_RUNGS = 1
NS_STEPS = 3
MUO = (3.4445, -4.7750, 2.0315)  # quintic NS coefficients
G2_SCALE = 2.0 ** -21

SHEAR_N = 512 * 257              # shear scratch elems per matrix
QBUF_N = 511 * 129               # periodic Toeplitz buffer elems per matrix


def _halfslc(hh, w=R):
    return slice(w * hh, w * hh + w)


def _load_256(nc, dst, src_b):
    """DRAM (256, X) -> SBUF [128, 2X] (row halves side by side)."""
    X = src_b.shape[-1]
    nc.sync.dma_start(out=dst[:, 0:X], in_=src_b[0:H, :])
    nc.sync.dma_start(out=dst[:, X:2 * X], in_=src_b[H:2 * H, :])


def _mm256_wide(nc, psum_pool, out_t, lhs_t, rhs_t, scale=None):
    """out = L^T @ Rhs for 256x256 [128,512]-tiled operands (4 matmuls)."""
    for mh in range(2):
        ps = psum_pool.tile([H, 2 * R], F32, tag="wide")
        for kh in range(2):
            nc.tensor.matmul(
                ps[:, 0:R],
                lhs_t[:, R * kh + H * mh: R * kh + H * mh + H],
                rhs_t[:, R * kh: R * kh + R],
                start=(kh == 0), stop=(kh == 1),
            )
        if scale is None:
            if mh == 0:
                nc.vector.tensor_copy(out_t[:, R * mh: R * mh + R], ps[:, 0:R])
            else:
                nc.scalar.mul(out_t[:, R * mh: R * mh + R], ps[:, 0:R], 1.0)
        else:
            eng = nc.scalar if mh == 0 else nc.vector
            if mh == 0:
                nc.scalar.mul(out_t[:, R * mh: R * mh + R], ps[:, 0:R], scale)
            else:
                nc.vector.tensor_scalar_mul(
                    out_t[:, R * mh: R * mh + R], ps[:, 0:R], scale)


def _transpose_256(nc, ptr_pool, out_t, in_t, ident):
    """out = in^T for a 256x256 [128,512] tile (4 PE transposes)."""
    for i in range(2):
        for j in range(2):
            ps = ptr_pool.tile([H, H], F32, tag="tr")
            nc.tensor.transpose(
                ps[:, :],
                in_t[:, R * j + H * i: R * j + H * i + H].bitcast(F32),
                ident[:, :],
            )
            nc.vector.tensor_copy(out_t[:, R * i + H * j: R * i + H * j + H], ps[:, :])


def build_k1(bpc=BPC, n_rungs=N_RUNGS, ns_steps=NS_STEPS):
    nc = bacc.Bacc("TRN2", target_bir_lowering=False)
    sp_d = nc.dram_tensor("sp", [bpc, R, R], F32R, kind="ExternalInput")
    tp_d = nc.dram_tensor("tp", [bpc, R, R], F32R, kind="ExternalInput")
    t_d = nc.dram_tensor("t", [bpc, R, R], F32, kind="ExternalInput")
    w1t_d = nc.dram_tensor("w1t", [R, R], F32R, kind="ExternalInput")
    w2t_d = nc.dram_tensor("w2t", [R, R], F32R, kind="ExternalInput")
    w3_d = nc.dram_tensor("w3", [R, R], F32, kind="ExternalInput")
    w4_d = nc.dram_tensor("w4", [R, R], F32, kind="ExternalInput")
    ident_d = nc.dram_tensor("ident", [H, H], F32, kind="ExternalInput")
    eyema_d = nc.dram_tensor("eyema", [H, H], F32, kind="ExternalInput")  # MUO[0]*I
    blocktr_d = nc.dram_tensor("blocktr", [H, H], F32, kind="ExternalInput")
    seed_d = nc.dram_tensor("seed", [H, 2 * LA], F32R, kind="ExternalInput")
    ghbh_out = nc.dram_tensor("ghbh_out", [bpc, LA, 2 * LA], F32,
                              kind="ExternalOutput")
    vtwpt_out = nc.dram_tensor("vtwpt_out", [bpc, LA, 2 * R], F32R,
                               kind="ExternalOutput")

    n_pack = (bpc + GRP - 1) // GRP
    with tile.TileContext(nc) as tc, ExitStack() as ctx:
        cpool = ctx.enter_context(tc.tile_pool(name="consts", bufs=1))
        inpool = ctx.enter_context(tc.tile_pool(name="inp", bufs=2))
        tpool = ctx.enter_context(tc.tile_pool(name="trans", bufs=2))
        keep = ctx.enter_context(tc.tile_pool(name="keep", bufs=1))
        spool = ctx.enter_context(tc.tile_pool(name="small", bufs=2))
        sone = ctx.enter_context(tc.tile_pool(name="sone", bufs=1))
        pmm = ctx.enter_context(tc.tile_pool(name="pmm", bufs=2, space="PSUM"))
        ptr = ctx.enter_context(tc.tile_pool(name="ptr", bufs=2, space="PSUM"))
        psm = ctx.enter_context(tc.tile_pool(name="psm", bufs=2, space="PSUM"))
        psb = ctx.enter_context(tc.tile_pool(name="psb", bufs=2, space="PSUM"))

        w1t = cpool.tile([H, 2 * R], F32R); _load_256(nc, w1t, w1t_d)
        w2t = cpool.tile([H, 2 * R], F32R); _load_256(nc, w2t, w2t_d)
        w3 = cpool.tile([H, 2 * R], F32); _load_256(nc, w3, w3_d)
        w4 = cpool.tile([H, 2 * R], F32); _load_256(nc, w4, w4_d)
        ident = cpool.tile([H, H], F32)
        nc.sync.dma_start(out=ident[:, :], in_=ident_d[:, :])
        eyema = cpool.tile([H, H], F32)
        nc.sync.dma_start(out=eyema[:, :], in_=eyema_d[:, :])
        blocktr = cpool.tile([H, H], F32)
        nc.sync.dma_start(out=blocktr[:, :], in_=blocktr_d[:, :])
        seed = cpool.tile([H, 2 * LA], F32R)
        nc.sync.dma_start(out=seed[:, :], in_=seed_d[:, :])
        onescol = cpool.tile([H, 1], F32)
        nc.any.memset(onescol[:, :], 1.0)
        eyema_bf = cpool.tile([H, H], BF16)
        nc.vector.tensor_copy(eyema_bf[:, :], eyema[:, :])
        ident_bf = cpool.tile([H, H], BF16)
        nc.vector.tensor_copy(ident_bf[:, :], ident[:, :])

        ats, s0s, s1s, vs = [], [], [], []
        # ---- phase L: loads, A, A^T, G (per matrix; pipelines across b) ----
        sp2 = tp2 = t2 = None
        for b in range(bpc):
            if b % 2 == 0:
                # one DMA per tensor loads a PAIR of matrices [128, 1024]
                sp2 = inpool.tile([H, 4 * R], F32R, tag="sp")
                tp2 = inpool.tile([H, 4 * R], F32R, tag="tp")
                t2 = inpool.tile([H, 4 * R], F32, tag="t")
                for dst, src, eng in ((sp2, sp_d, nc.sync), (tp2, tp_d, nc.gpsimd),
                                      (t2, t_d, nc.gpsimd)):
                    eng.dma_start(
                        out=dst[:, :].rearrange("p (b hh c) -> p b hh c",
                                                b=2, hh=2, c=R),
                        in_=src[b:b + 2].rearrange("b (hh p) c -> p b hh c", p=H),
                    )
            m = b % 2
            sp_t = sp2[:, 2 * R * m: 2 * R * m + 2 * R]
            tp_t = tp2[:, 2 * R * m: 2 * R * m + 2 * R]
            t_t = t2[:, 2 * R * m: 2 * R * m + 2 * R]

            x1 = tpool.tile([H, 2 * R], F32, tag="x1")
            nc.vector.tensor_mul(x1[:, :], w4[:, :], tp_t[:, :].bitcast(F32))
            x2 = tpool.tile([H, 2 * R], F32, tag="x2")
            nc.vector.tensor_mul(x2[:, :], w3[:, :], t_t[:, :])
            nc.vector.tensor_add(x1[:, :], x1[:, :], x2[:, :])
            a_t = tpool.tile([H, 2 * R], F32R, tag="a")
            for mh in range(2):
                ps = pmm.tile([H, 2 * R], F32, tag="wide")
                for kh in range(2):
                    nc.tensor.matmul(
                        ps[:, 0:R],
                        w1t[:, R * kh + H * mh: R * kh + H * mh + H],
                        sp_t[:, R * kh: R * kh + R],
                        start=(kh == 0), stop=False,
                    )
                for kh in range(2):
                    nc.tensor.matmul(
                        ps[:, 0:R],
                        w2t[:, R * kh + H * mh: R * kh + H * mh + H],
                        tp_t[:, R * kh: R * kh + R],
                        start=False, stop=(kh == 1),
                    )
                nc.vector.tensor_add(
                    a_t[:, R * mh: R * mh + R], ps[:, 0:R],
                    x1[:, R * mh: R * mh + R],
                )
            at_t = keep.tile([H, 2 * R], F32R, tag=f"at{b}")
            _transpose_256(nc, ptr, at_t, a_t, ident)
            s0_t = keep.tile([H, 2 * R], F32R, tag=f"s0_{b}")
            _mm256_wide(nc, pmm, s0_t, a_t, a_t)          # G
            s1_t = keep.tile([H, 2 * R], F32R, tag=f"s1_{b}")
            v_t = keep.tile([H, 2 * LA], F32R, tag=f"v{b}")
            nc.vector.tensor_copy(v_t[:, :], seed[:, :].bitcast(F32))
            ats.append(at_t); s0s.append(s0_t); s1s.append(s1_t); vs.append(v_t)

        # ---- phase C: chain G2..G16, step-major so the PE never stalls ----
        for b in range(bpc):                               # G2 = (G^2)*2^-21
            _mm256_wide(nc, pmm, s1s[b], s0s[b], s0s[b], scale=G2_SCALE)
        for b in range(bpc):                               # G4
            _mm256_wide(nc, pmm, s0s[b], s1s[b], s1s[b])
        for b in range(bpc):                               # G8
            _mm256_wide(nc, pmm, s1s[b], s0s[b], s0s[b])
        for b in range(bpc):                               # G16 -> hs = s0s
            _mm256_wide(nc, pmm, s0s[b], s1s[b], s1s[b])
        hs = s0s

        # ---- phase R: rungs, the 4 packs' NS chains interleaved ----
        a_c, b_c, c_c = MUO
        for r in range(n_rungs):
            mbds, cts, yts = [], [], []
            for p in range(n_pack):
                mbd = sone.tile([H, H], F32, tag=f"mbd{p}")
                nc.any.memset(mbd[:, :], 0.0)
                mbds.append(mbd)
            for b in range(bpc):
                p, sl = b // GRP, (b % GRP) * 32
                yt_ps = psm.tile([LA, R], F32, tag="sm")
                for kh in range(2):
                    nc.tensor.matmul(
                        yt_ps[:, :],
                        vs[b][:, LA * kh: LA * kh + LA],
                        hs[b][:, R * kh: R * kh + R],
                        start=(kh == 0), stop=(kh == 1),
                    )
                yt_t = sone.tile([LA, R], F32, tag=f"ytt{b}")
                nc.vector.tensor_copy(yt_t[:, :], yt_ps[:, :])
                y_t = spool.tile([H, 2 * LA], F32, tag="yy")
                for hh in range(2):
                    tr_ps = ptr.tile([H, LA], F32, tag="tr")
                    nc.tensor.transpose(
                        tr_ps[:, :],
                        yt_t[:, H * hh: H * hh + H],
                        ident[:LA, :LA],
                    )
                    nc.vector.tensor_copy(y_t[:, LA * hh: LA * hh + LA], tr_ps[:, :])
                # gram into the pack's block-diag tile
                m_ps = psb.tile([H, H], F32, tag="smb")
                for kh in range(2):
                    nc.tensor.matmul(
                        m_ps[sl:sl + LA, sl:sl + LA],
                        y_t[:, LA * kh: LA * kh + LA],
                        y_t[:, LA * kh: LA * kh + LA],
                        start=(kh == 0), stop=(kh == 1),
                        tile_position=(0, sl),
                    )
                nc.vector.tensor_copy(
                    mbds[p][sl:sl + LA, sl:sl + LA], m_ps[sl:sl + LA, sl:sl + LA]
                )
                yts.append(yt_t)

            # trace normalization, p-interleaved
            mns, rrvs = [], []
            for p in range(n_pack):
                masked = spool.tile([H, H], F32, tag="masked")
                nc.vector.tensor_mul(masked[:, :], mbds[p][:, :], ident[:, :])
                dr_ps = psb.tile([1, H], F32, tag="smb")
                nc.tensor.matmul(dr_ps[:, :], onescol[:, :], masked[:, :],
                                 start=True, stop=True)
                drow = spool.tile([1, H], F32, tag="drow")
                nc.vector.tensor_copy(drow[:, :], dr_ps[:, :])
                dg_ps = psb.tile([H, 1], F32, tag="smb")
                nc.tensor.transpose(dg_ps[:, :], drow[:, :], ident[:1, :1])
                diag = spool.tile([H, 1], F32, tag="diag")
                nc.vector.tensor_copy(diag[:, :], dg_ps[:, :])
                tr_ps = psb.tile([H, 1], F32, tag="smb")
                nc.tensor.matmul(tr_ps[:, :], blocktr[:, :], diag[:, :],
                                 start=True, stop=True)
                tre = spool.tile([H, 1], F32, tag="tre")
                nc.vector.tensor_scalar_add(tre[:, :], tr_ps[:, :], 1e-30)
                itv = spool.tile([H, 1], F32, tag="itv")
                nc.vector.reciprocal(itv[:, :], tre[:, :])
                sq = spool.tile([H, 1], F32, tag="sq")
                nc.scalar.activation(
                    sq[:, :], tre[:, :], mybir.ActivationFunctionType.Sqrt,
                )
                rrv = sone.tile([H, 1], F32, tag=f"rrv{p}")
                nc.vector.reciprocal(rrv[:, :], sq[:, :])
                mn = sone.tile([H, H], BF16, tag=f"mn{p}")
                nc.vector.tensor_scalar_mul(mn[:, :], mbds[p][:, :], itv[:, :])
                mns.append(mn); rrvs.append(rrv)

            # quintic NS, steps interleaved across the 4 packs
            mcurs = list(mns)
            cts = [sone.tile([H, H], BF16, tag=f"ct{p}", name=f"ct{p}")
                   for p in range(n_pack)]
            for st in range(ns_steps):
                m2_pss, csts = [], []
                for p in range(n_pack):
                    m2_ps = psb.tile([H, H], F32, tag="smb")
                    nc.tensor.matmul(m2_ps[:, :], mcurs[p][:, :], mcurs[p][:, :],
                                     start=True, stop=True)
                    m2_pss.append(m2_ps)
                for p in range(n_pack):
                    cst = sone.tile([H, H], BF16, tag=f"cst{p}")
                    nc.vector.tensor_scalar_mul(
                        cst[:, :], mcurs[p][:, :].bitcast(BF16), b_c)
                    nc.vector.tensor_add(cst[:, :], cst[:, :], eyema_bf[:, :])
                    m2s = spool.tile([H, H], BF16, tag="m2s")
                    nc.scalar.mul(m2s[:, :], m2_pss[p][:, :], c_c)
                    nc.vector.tensor_add(cst[:, :], cst[:, :], m2s[:, :])
                    csts.append(cst)
                if st < ns_steps - 1:
                    cms = []
                    for p in range(n_pack):
                        cm_ps = psb.tile([H, H], F32, tag="smb")
                        nc.tensor.matmul(cm_ps[:, :], csts[p][:, :], mcurs[p][:, :],
                                         start=True, stop=True)
                        cm = spool.tile([H, H], BF16, tag=f"cm{p}")
                        nc.vector.tensor_copy(cm[:, :], cm_ps[:, :])
                        cms.append(cm)
                    for p in range(n_pack):
                        mn2_ps = psb.tile([H, H], F32, tag="smb")
                        nc.tensor.matmul(mn2_ps[:, :], cms[p][:, :], csts[p][:, :],
                                         start=True, stop=True)
                        mnew = sone.tile([H, H], BF16, tag=f"mnew{p}_{st}")
                        nc.vector.tensor_copy(mnew[:, :], mn2_ps[:, :])
                        mcurs[p] = mnew
                for p in range(n_pack):
                    if st == 0:
                        nc.vector.tensor_copy(cts[p][:, :], csts[p][:, :])
                    else:
                        ct_ps = psb.tile([H, H], F32, tag="smb")
                        nc.tensor.matmul(ct_ps[:, :], cts[p][:, :], csts[p][:, :],
                                         start=True, stop=True)
                        nc.vector.tensor_copy(cts[p][:, :], ct_ps[:, :])
            for p in range(n_pack):
                nc.vector.tensor_scalar_mul(cts[p][:, :], cts[p][:, :], rrvs[p][:, :])

            # extract each pack's diag blocks to partition base 0 via an
            # identity matmul (operands share base sl; out lands at base 0)
            ct0s = []
            for p in range(n_pack):
                for kk in range(GRP):
                    sl = kk * 32
                    c0_ps = psb.tile([LA, LA], F32, tag="smb")
                    nc.tensor.matmul(
                        c0_ps[:, :],
                        ident_bf[sl:sl + LA, sl:sl + LA],
                        cts[p][sl:sl + LA, sl:sl + LA],
                        start=True, stop=True,
                        tile_position=(sl, 0),
                    )
                    ct0 = sone.tile([LA, LA], F32, tag=f"ct0_{p}_{kk}",
                                    name=f"ct0_{p}_{kk}")
                    nc.vector.tensor_copy(ct0[:, :], c0_ps[:, :])
                    ct0s.append(ct0)
            # apply: V_b = Y_b @ Ct0_b (all operands at base 0)
            for b in range(bpc):
                for hh in range(2):
                    vp = ptr.tile([H, LA], F32, tag="tr")
                    nc.tensor.matmul(
                        vp[:, :],
                        yts[b][:, H * hh: H * hh + H],
                        ct0s[b][:, :],
                        start=True, stop=True,
                    )
                    nc.vector.tensor_copy(
                        vs[b][:, LA * hh: LA * hh + LA], vp[:, :]
                    )

        # ---- phase O: outputs Gh, Bh, Vt, Wpt (pipelines across b) ----
        for b in range(bpc):
            zt_ps = psm.tile([LA, R], F32, tag="sm")
            for kh in range(2):
                nc.tensor.matmul(
                    zt_ps[:, :],
                    vs[b][:, LA * kh: LA * kh + LA],
                    hs[b][:, R * kh: R * kh + R],
                    start=(kh == 0), stop=(kh == 1),
                )
            zt_t = spool.tile([LA, R], F32, tag="ztt")
            nc.vector.tensor_copy(zt_t[:, :], zt_ps[:, :])
            z_t = spool.tile([H, 2 * LA], F32, tag="zz")
            for hh in range(2):
                tr_ps = ptr.tile([H, LA], F32, tag="tr")
                nc.tensor.transpose(
                    tr_ps[:, :], zt_t[:, H * hh: H * hh + H],
                    ident[:LA, :LA],
                )
                nc.vector.tensor_copy(z_t[:, LA * hh: LA * hh + LA], tr_ps[:, :])
            ghbh_t = spool.tile([LA, 2 * LA], F32, tag="ghbh")
            gh_ps = psb.tile([LA, LA], F32, tag="smb")
            for kh in range(2):
                nc.tensor.matmul(
                    gh_ps[:, :],
                    z_t[:, LA * kh: LA * kh + LA],
                    vs[b][:, LA * kh: LA * kh + LA].bitcast(F32),
                    start=(kh == 0), stop=(kh == 1),
                )
            nc.vector.tensor_copy(ghbh_t[:, 0:LA], gh_ps[:, :])
            bh_ps = psb.tile([LA, LA], F32, tag="smb")
            for kh in range(2):
                nc.tensor.matmul(
                    bh_ps[:, :],
                    vs[b][:, LA * kh: LA * kh + LA].bitcast(F32),
                    vs[b][:, LA * kh: LA * kh + LA].bitcast(F32),
                    start=(kh == 0), stop=(kh == 1),
                )
            nc.vector.tensor_copy(ghbh_t[:, LA:2 * LA], bh_ps[:, :])
            nc.sync.dma_start(out=ghbh_out[b], in_=ghbh_t[:, :])

            vw_t = spool.tile([LA, 2 * R], F32R, tag="vw")
            for hh in range(2):
                tr_ps = psm.tile([LA, H], F32, tag="sm")
                nc.tensor.transpose(
                    tr_ps[:, :],
                    vs[b][:, LA * hh: LA * hh + LA].bitcast(F32),
                    ident[:, :],
                )
                nc.vector.tensor_copy(vw_t[:, H * hh: H * hh + H], tr_ps[:, :])
            wpt_ps = psm.tile([LA, R], F32, tag="sm")
            for kh in range(2):
                nc.tensor.matmul(
                    wpt_ps[:, :],
                    vs[b][:, LA * kh: LA * kh + LA],
                    ats[b][:, R * kh: R * kh + R],
                    start=(kh == 0), stop=(kh == 1),
                )
            nc.vector.tensor_copy(vw_t[:, R:2 * R], wpt_ps[:, :])
            nc.sync.dma_start(out=vtwpt_out[b], in_=vw_t[:, :])
    nc.compile()
    return nc


def build_k2(bpc=BPC, stage=3):
    nc = bacc.Bacc("TRN2", target_bir_lowering=False)
    sp_d = nc.dram_tensor("sp", [bpc, R, R], F32, kind="ExternalInput")
    # packed per-matrix smalls: [vt | wpt | ms] = [16, 256+256+16]
    vwm_d = nc.dram_tensor("vwm", [bpc, LA, 2 * R + LA], F32R,
                           kind="ExternalInput")
    ident_d = nc.dram_tensor("ident", [H, H], F32, kind="ExternalInput")
    invc_d = nc.dram_tensor("invc", [1, 511], F32, kind="ExternalInput")
    tpn_out = nc.dram_tensor("tpn_out", [bpc, R, R], F32, kind="ExternalOutput")
    spn_out = nc.dram_tensor("spn_out", [bpc, R, R], F32, kind="ExternalOutput")
    scr1 = nc.dram_tensor("scr1", [bpc, SHEAR_N], BF16)
    scrq = nc.dram_tensor("scrq", [bpc, QBUF_N], F32)

    with tile.TileContext(nc) as tc, ExitStack() as ctx:
        cpool = ctx.enter_context(tc.tile_pool(name="consts", bufs=1))
        inpool = ctx.enter_context(tc.tile_pool(name="inp", bufs=2))
        tpool = ctx.enter_context(tc.tile_pool(name="trans", bufs=2))
        keep = ctx.enter_context(tc.tile_pool(name="keep", bufs=1))
        spool = ctx.enter_context(tc.tile_pool(name="small", bufs=3))
        pmm = ctx.enter_context(tc.tile_pool(name="pmm", bufs=2, space="PSUM"))
        ptr = ctx.enter_context(tc.tile_pool(name="ptr", bufs=2, space="PSUM"))
        psm = ctx.enter_context(tc.tile_pool(name="psm", bufs=2, space="PSUM"))

        ident = cpool.tile([H, H], F32)
        nc.sync.dma_start(out=ident[:, :], in_=ident_d[:, :])
        invc = cpool.tile([1, 511], F32)
        nc.sync.dma_start(out=invc[:, :], in_=invc_d[:, :])
        ones = cpool.tile([H, 1], BF16)
        nc.any.memset(ones[:, :], 1.0)
        onesr = cpool.tile([1, H], BF16)
        nc.any.memset(onesr[:, :], 1.0)

        # per-matrix zero-padded staging tiles (pads stay zero) + results
        m2zs, tpns, sps = [], [], []
        for b in range(bpc):
            m2z = keep.tile([H, 1024], BF16, tag=f"m2z{b}", name=f"m2z{b}")
            nc.any.memset(m2z[:, :], 0.0)
            m2zs.append(m2z)
        # one DMA zeroes the shear-gap head [0,255) of every matrix slot
        nc.sync.dma_start(out=scr1[:, 0:255], in_=m2zs[0][0:bpc, 256:511])

        # ---- phase A: tpn + shear writes (pipelines across b) ----
        sp2 = None
        for b in range(bpc):
            if b % 2 == 0:
                sp2 = keep.tile([H, 4 * R], F32, tag=f"sp{b}", name=f"sp{b}")
                nc.sync.dma_start(
                    out=sp2[:, :].rearrange("p (b hh c) -> p b hh c",
                                            b=2, hh=2, c=R),
                    in_=sp_d[b:b + 2].rearrange("b (hh p) c -> p b hh c", p=H),
                )
                sps.append(sp2)
            m = b % 2
            sp_t = sp2[:, 2 * R * m: 2 * R * m + 2 * R]
            vwm_t = inpool.tile([LA, 2 * R + LA], F32R, tag="vwm")
            nc.gpsimd.dma_start(out=vwm_t[:, :], in_=vwm_d[b])
            vt_t = vwm_t[:, 0:R]
            wpt_t = vwm_t[:, R:2 * R]
            ms_t = vwm_t[:, 2 * R:2 * R + LA]

            # Sp^T via PE transposes
            spt_t = tpool.tile([H, 2 * R], F32, tag="spt")
            _transpose_256f(nc, ptr, spt_t, sp_t, ident)

            # P1 = Ms @ Wpt ; P2 = Ms @ Vt   (Ms symmetric)
            p1_ps = psm.tile([LA, R], F32, tag="sm")
            nc.tensor.matmul(p1_ps[:, :], ms_t, wpt_t, start=True, stop=True)
            p1_t = spool.tile([LA, R], F32R, tag="p1")
            nc.vector.tensor_copy(p1_t[:, :], p1_ps[:, :])
            p2_ps = psm.tile([LA, R], F32, tag="sm")
            nc.tensor.matmul(p2_ps[:, :], ms_t, vt_t, start=True, stop=True)
            p2_t = spool.tile([LA, R], F32R, tag="p2")
            nc.vector.tensor_copy(p2_t[:, :], p2_ps[:, :])

            # TpnewT = V P1 ; Tpnew = W' P2  (fp32r, 256-wide)
            tpnT = tpool.tile([H, 2 * R], F32, tag="tpnT")
            tpn = keep.tile([H, 2 * R], F32, tag=f"tpn{b}", name=f"tpn{b}")
            for hh in range(2):
                ps = pmm.tile([H, 2 * R], F32, tag="wide")
                nc.tensor.matmul(
                    ps[:, 0:R],
                    vt_t[:, H * hh: H * hh + H],
                    p1_t[:, :],
                    start=True, stop=True,
                )
                nc.vector.tensor_copy(tpnT[:, R * hh: R * hh + R], ps[:, 0:R])
                ps2 = pmm.tile([H, 2 * R], F32, tag="wide")
                nc.tensor.matmul(
                    ps2[:, 0:R],
                    wpt_t[:, H * hh: H * hh + H],
                    p2_t[:, :],
                    start=True, stop=True,
                )
                nc.vector.tensor_copy(tpn[:, R * hh: R * hh + R], ps2[:, 0:R])
            tpns.append(tpn)
            # Tpnew out, one DMA (natural layout)
            nc.sync.dma_start(
                out=tpn_out[b].rearrange("(hh p) c -> p hh c", p=H),
                in_=tpn[:, :].rearrange("p (hh c) -> p hh c", hh=2),
            )
            if stage < 2:
                continue
            # M2T = 2*TpnewT - SpT into this matrix's bf16 staging tile
            m2z = m2zs[b]
            for hh in range(2):
                nc.vector.tensor_scalar_mul(
                    m2z[:, 512 * hh: 512 * hh + R],
                    tpnT[:, R * hh: R * hh + R], 2.0,
                )
                nc.vector.tensor_tensor(
                    out=m2z[:, 512 * hh: 512 * hh + R],
                    in0=m2z[:, 512 * hh: 512 * hh + R],
                    in1=spt_t[:, R * hh: R * hh + R],
                    op=mybir.AluOpType.subtract,
                )
            # shear-write both halves in one DMA (row i at 511*i + 255)
            nc.gpsimd.dma_start(
                out=scr1[b][255: 255 + 511 * 2 * H].rearrange(
                    "(hh p f) -> p hh f", p=H, hh=2),
                in_=m2z[:, :].rearrange("p (hh x) -> p hh x", hh=2)[:, :, 0:511],
            )

        # ---- phase B: diagonal sums -> periodic Q buffer ----
        if stage >= 2:
            for b in range(bpc):
                shm = tpool.tile([H, 1022], BF16, tag="shm")
                nc.gpsimd.dma_start(
                    out=shm[:, :].rearrange("p (hh f) -> p hh f", hh=2),
                    in_=scr1[b][0: 512 * 2 * H].rearrange(
                        "(hh p f) -> p hh f", p=H, hh=2)[:, :, 0:511],
                )
                sums_ps = psm.tile([1, 511], F32, tag="sm3")
                for hh in range(2):
                    nc.tensor.matmul(sums_ps[:, :], ones[:, :],
                                     shm[:, 511 * hh: 511 * hh + 511],
                                     start=(hh == 0), stop=(hh == 1))
                avg = spool.tile([1, 511], BF16, tag="avg")
                nc.vector.tensor_mul(avg[:, :], sums_ps[:1, :], invc[:, :])
                avgb_ps = pmm.tile([H, 2 * R], F32, tag="wide")
                nc.tensor.matmul(avgb_ps[:, 0:511], onesr[:, :], avg[:, :],
                                 start=True, stop=True)
                avgb = spool.tile([H, 511], F32, tag="avgb")
                nc.vector.tensor_copy(avgb[:, :], avgb_ps[:, 0:511])
                # periodic Q: 128 rows + 1 wrap row of avg at stride 511
                nc.sync.dma_start(
                    out=scrq[b][0: 511 * H].rearrange("(p f) -> p f", p=H),
                    in_=avgb[:, :],
                )
                nc.sync.dma_start(
                    out=scrq[b][511 * H: 511 * (H + 1)].rearrange(
                        "(p f) -> p f", p=1),
                    in_=avgb[0:1, :],
                )

        # ---- phase C: Toeplitz + Spnew ----
        if stage >= 3:
            for b in range(bpc):
                sp_t = sps[b // 2][:, 2 * R * (b % 2): 2 * R * (b % 2) + 2 * R]
                # toepT[p, f] = avg[f - p + 255] via mod-511 reads:
                # cols [0,256) = output rows 128..255 (base 127),
                # cols [256,512) = rows 0..127 (base 255)
                ttT = tpool.tile([H, 2 * R], F32, tag="ttT")
                for hh, base in ((1, 255), (0, 127)):
                    nc.gpsimd.dma_start(
                        out=ttT[:, R * hh: R * hh + R],
                        in_=scrq[b][base: base + 510 * H].rearrange(
                            "(p f) -> p f", p=H)[:, 0:R],
                    )
                # toep natural = transpose(toepT) (halves swapped in ttT)
                ttN = tpool.tile([H, 2 * R], F32, tag="ttN")
                for i in range(2):
                    for j in range(2):
                        ps = ptr.tile([H, H], F32, tag="tr")
                        nc.tensor.transpose(
                            ps[:, :],
                            ttT[:, R * (1 - j) + H * i: R * (1 - j) + H * i + H],
                            ident[:, :],
                        )
                        nc.vector.tensor_copy(
                            ttN[:, R * i + H * j: R * i + H * j + H], ps[:, :])
                # Spnew = Sp - Tpnew + toep_nat ; one DMA out
                spn = tpool.tile([H, 2 * R], F32, tag="spn")
                nc.vector.tensor_tensor(
                    out=spn[:, :], in0=sp_t, in1=tpns[b][:, :],
                    op=mybir.AluOpType.subtract,
                )
                nc.vector.tensor_add(spn[:, :], spn[:, :], ttN[:, :])
                nc.sync.dma_start(
                    out=spn_out[b].rearrange("(hh p) c -> p hh c", p=H),
                    in_=spn[:, :].rearrange("p (hh c) -> p hh c", hh=2),
                )
    nc.compile()
    return nc


def _transpose_256f(nc, ptr_pool, out_t, in_t, ident):
    """out = in^T for a 256x256 [128,512] fp32 tile (4 PE transposes)."""
    for i in range(2):
        for j in range(2):
            ps = ptr_pool.tile([H, H], F32, tag="tr")
            nc.tensor.transpose(
                ps[:, :], in_t[:, R * j + H * i: R * j + H * i + H], ident[:, :]
            )
            nc.vector.tensor_copy(out_t[:, R * i + H * j: R * i + H * j + H], ps[:, :])


def _host_consts():
    ident = np.eye(H, dtype=np.float32)
    eyema = (MUO[0] * np.eye(H)).astype(np.float32)
    blocktr = np.zeros((H, H), np.float32)
    for g in range(4):
        blocktr[g * 32: g * 32 + LA, g * 32: g * 32 + 32] = 1.0
    i = np.arange(R, dtype=np.float32)[:, None]
    j = np.arange(LA, dtype=np.float32)[None, :]
    v0 = np.cos(0.37 * (i + 1) * (j + 1) + 0.11 * i).astype(np.float32)
    seed = np.concatenate([v0[0:H, :], v0[H:R, :]], axis=1)  # [128, 32]
    counts = (R - np.abs(np.arange(511) - 255)).astype(np.float32)
    invc = (1.0 / counts)[None, :].astype(np.float32)
    return ident, eyema, blocktr, seed, invc


def _host_bridge(gh, bh, Kv):
    """Robust whitened generalized eig; returns Ms = Z10 Z10^T per matrix."""
    n = gh.shape[0]
    ms = np.zeros((n, LA, LA), np.float32)
    for b in range(n):
        Gs = 0.5 * (gh[b] + gh[b].T).astype(np.float64)
        Bs = 0.5 * (bh[b] + bh[b].T).astype(np.float64)
        lb, Ub = np.linalg.eigh(Bs)
        lmax = max(float(lb.max()), 0.0)
        keep = lb > lmax * 1e-7 if lmax > 0 else lb > -1.0
        if not np.any(keep):
            continue
        Wh = Ub[:, keep] / np.sqrt(np.maximum(lb[keep], 1e-300))[None, :]
        Gw = Wh.T @ Gs @ Wh
        d, Qw = np.linalg.eigh(Gw)
        Z = Wh @ Qw[:, ::-1][:, :Kv]
        ms[b] = (Z @ Z.T).astype(np.float32)
    return ms


def _host_fallback(T, Tp, Sp, w1, w2, w3, w4, Kv):
    """Numpy implementation (used only if the device path fails)."""
    f32 = np.float32
    A = (np.einsum('rk,bkc->brc', w1, Sp) + np.einsum('rk,bkc->brc', w2, Tp)
         + w4[None] * Tp + w3[None] * T).astype(f32)
    G = np.einsum('brc,brd->bcd', A, A).astype(f32)
    d, q = np.linalg.eigh(G.astype(np.float64))
    Vs = q[:, :, ::-1][:, :, :Kv]
    AV = np.einsum('brc,bcl->brl', A.astype(np.float64), Vs)
    Tpnew = np.einsum('brl,bcl->brc', AV, Vs).astype(f32)
    m, n = R, R
    D = m + n - 1
    ii = np.arange(m)[:, None]; jj = np.arange(n)[None, :]
    dd = jj - ii + (m - 1)
    M2 = (2.0 * Tpnew - Sp).astype(f32)
    Z = np.zeros((M2.shape[0], m, D), f32)
    Z[:, ii, dd] = M2
    sums = Z.sum(axis=1)
    counts = (m - np.abs(np.arange(D) - (m - 1))).astype(f32)
    avg = sums / counts
    Spnew = (Sp - Tpnew + avg[:, dd]).astype(f32)
    return (T, Tpnew, Spnew)


_K1 = None
_K2 = None


def _get_kernels():
    global _K1, _K2
    if _K1 is None:
        _K1 = build_k1()
    if _K2 is None:
        _K2 = build_k2()
    return _K1, _K2


def _run_k2(Sp, vt_all, wpt_all, ms_all, nc2=None):
    ident, eyema, blocktr, seed, invc = _host_consts()
    if nc2 is None:
        nc2 = build_k2()
    vwm = np.concatenate([vt_all, wpt_all, ms_all], axis=2)  # [B, 16, 528]
    vwm = np.ascontiguousarray(vwm, dtype=np.float32)
    in_maps = []
    for c in range(N_CORES):
        sl = slice(c * BPC, (c + 1) * BPC)
        in_maps.append({
            "sp": Sp[sl], "vwm": vwm[sl], "ident": ident, "invc": invc,
        })
    r2 = run_bass_kernel_spmd(nc2, in_maps, list(range(N_CORES)))
    LAST_EXEC_NS[1] = r2.exec_time_ns
    res2 = r2.results
    Tpnew = np.concatenate([res2[c]["tpn_out"] for c in range(N_CORES)], axis=0)
    Spnew = np.concatenate([res2[c]["spn_out"] for c in range(N_CORES)], axis=0)
    return Tpnew, Spnew


def _kernel_device(T, Tp, Sp, w1, w2, w3, w4, Kv):
    ident, eyema, blocktr, seed, invc = _host_consts()
    w1t = np.ascontiguousarray(w1.T)
    w2t = np.ascontiguousarray(w2.T)
    nc1, nc2 = _get_kernels()
    in_maps1 = []
    for c in range(N_CORES):
        sl = slice(c * BPC, (c + 1) * BPC)
        in_maps1.append({
            "sp": Sp[sl], "tp": Tp[sl], "t": T[sl],
            "w1t": w1t, "w2t": w2t, "w3": w3, "w4": w4,
            "ident": ident, "eyema": eyema, "blocktr": blocktr, "seed": seed,
        })
    r1 = run_bass_kernel_spmd(nc1, in_maps1, list(range(N_CORES)))
    LAST_EXEC_NS[0] = r1.exec_time_ns
    res1 = r1.results
    ghbh = np.concatenate([res1[c]["ghbh_out"] for c in range(N_CORES)], axis=0)
    vtwpt = np.concatenate([res1[c]["vtwpt_out"] for c in range(N_CORES)], axis=0)
    gh, bh = ghbh[:, :, 0:LA], ghbh[:, :, LA:2 * LA]
    vt_all, wpt_all = vtwpt[:, :, 0:R], vtwpt[:, :, R:2 * R]
    ms_all = _host_bridge(gh, bh, Kv)
    Tpnew, Spnew = _run_k2(Sp, vt_all, wpt_all, ms_all, nc2=nc2)
    return (T, Tpnew, Spnew)


def _kernel_hybrid(T, Tp, Sp, w1, w2, w3, w4, Kv):
    """Host eigensolve for the subspace + device K2 for apply/averaging."""
    f32 = np.float32
    A = (np.einsum('rk,bkc->brc', w1, Sp) + np.einsum('rk,bkc->brc', w2, Tp)
         + w4[None] * Tp + w3[None] * T).astype(f32)
    G = np.einsum('brc,brd->bcd', A, A)
    d, q = np.linalg.eigh(G.astype(np.float64))
    Vs = q[:, :, ::-1][:, :, :Kv]                       # [B, 256, K]
    vt_all = np.zeros((B_FULL, LA, R), f32)
    vt_all[:, :Kv, :] = Vs.transpose(0, 2, 1).astype(f32)
    AV = np.einsum('brc,bcl->brl', A.astype(np.float64), Vs)
    wpt_all = np.zeros((B_FULL, LA, R), f32)
    wpt_all[:, :Kv, :] = AV.transpose(0, 2, 1).astype(f32)
    ms_all = np.zeros((B_FULL, LA, LA), f32)
    ms_all[:, :Kv, :Kv] = np.eye(Kv, dtype=f32)[None]
    Tpnew, Spnew = _run_k2(Sp, vt_all, wpt_all, ms_all)
    return (T, Tpnew, Spnew)


def kernel(T, Tp, Sp, w1, w2, w3, w4, K):
    T = np.ascontiguousarray(np.asarray(T, dtype=np.float32))
    Tp = np.ascontiguousarray(np.asarray(Tp, dtype=np.float32))
    Sp = np.ascontiguousarray(np.asarray(Sp, dtype=np.float32))
    w1 = np.asarray(w1, dtype=np.float32); w2 = np.asarray(w2, dtype=np.float32)
    w3 = np.asarray(w3, dtype=np.float32); w4 = np.asarray(w4, dtype=np.float32)
    Kv = int(np.asarray(K))
    try:
        return _kernel_device(T, Tp, Sp, w1, w2, w3, w4, Kv)
    except Exception:
        import traceback
        traceback.print_exc()
        print("K1 device path failed; host eigensolve + device K2")
    try:
        return _kernel_hybrid(T, Tp, Sp, w1, w2, w3, w4, Kv)
    except Exception:
        import traceback
        traceback.print_exc()
        print("hybrid path failed; full host fallback")
        return _host_fallback(T, Tp, Sp, w1, w2, w3, w4, Kv)


LAST_EXEC_NS = [None, None]


# revision 43
# speedup vs baseline: 1.8663x; 1.1031x over previous
"""Cadzow update (batched rank-K truncation + Toeplitz averaging) on 8 trn2 cores.

Data-parallel over the batch of 128 matrices (16 per core). Per matrix:
  A = w1@Sp + w2@Tp + w4*Tp + w3*T
  rank-K via oversampled subspace iteration + host Rayleigh-Ritz:
    K1 (device): G = A^T A; chain G2=(G^2*2^-21), G4, G8, G16 (fp32r matmuls);
      3 rungs V <- orth(G16 V) with a quintic Newton-Schulz Gram conditioner
      (4 matrices packed per 128x128 block-diag tile); ships per matrix
      Gh = V^T G16 V, Bh = V^T V (16x16), Vt = V^T and Wpt = (A V)^T.
    host: robust whitened generalized eig of (Gh, Bh); top-K selector
      Ms = Z10 Z10^T (16x16).
    K2 (device): Tpnew = Wpt^T Ms Vt (both orientations from the small
      factors); Spnew = Sp - Tpnew + avgdiag(2 Tpnew - Sp) where the
      diagonal averaging runs via a shear-DMA layout (diag sums by
      ones-matmul) and the Toeplitz broadcast is read back from a
      mod-511 periodic DRAM buffer with all-positive strides.

All big matmuls run as fp32r (~4x PE throughput at >=256-wide outputs);
the 16x16 Grams / Newton-Schulz stay fp32. Outputs are written in natural
layout (no 4-byte-granular transposed DMA anywhere).
"""
import os
import numpy as np
from contextlib import ExitStack

# The axon ntff profile hook (antenv.axon_hooks) is absent in this image;
# a set BASS_TRACE would crash run_bass_kernel_spmd, so clear it.
os.environ.pop("BASS_TRACE", None)

import concourse.bass as bass
import concourse.bacc as bacc
import concourse.mybir as mybir
from concourse import tile
from concourse.bass_utils import run_bass_kernel_spmd

F32 = mybir.dt.float32
F32R = mybir.dt.float32r
BF16 = mybir.dt.bfloat16
N_CORES = 8
B_FULL = 128
BPC = B_FULL // N_CORES          # 16 matrices per core
R = 256
LA = 32                          # subspace dim (oversampled, 4x32 pack)
H = 128                          # partitions
GRP = 4                          # matrices packed per 128x128 Gram tile
N_RUNGS = 1
NS_STEPS = 3
MUO = (3.4445, -4.7750, 2.0315)  # quintic NS coefficients
G2_SCALE = 2.0 ** -21

SHEAR_N = 512 * 257              # shear scratch elems per matrix
QBUF_N = 511 * 129               # periodic Toeplitz buffer elems per matrix


def _halfslc(hh, w=R):
    return slice(w * hh, w * hh + w)


def _load_256(nc, dst, src_b):
    """DRAM (256, X) -> SBUF [128, 2X] (row halves side by side)."""
    X = src_b.shape[-1]
    nc.sync.dma_start(out=dst[:, 0:X], in_=src_b[0:H, :])
    nc.sync.dma_start(out=dst[:, X:2 * X], in_=src_b[H:2 * H, :])


def _mm256_wide(nc, psum_pool, out_t, lhs_t, rhs_t, scale=None):
    """out = L^T @ Rhs for 256x256 [128,512]-tiled operands (4 matmuls)."""
    for mh in range(2):
        ps = psum_pool.tile([H, 2 * R], F32, tag="wide")
        for kh in range(2):
            nc.tensor.matmul(
                ps[:, 0:R],
                lhs_t[:, R * kh + H * mh: R * kh + H * mh + H],
                rhs_t[:, R * kh: R * kh + R],
                start=(kh == 0), stop=(kh == 1),
            )
        if scale is None:
            if mh == 0:
                nc.vector.tensor_copy(out_t[:, R * mh: R * mh + R], ps[:, 0:R])
            else:
                nc.scalar.mul(out_t[:, R * mh: R * mh + R], ps[:, 0:R], 1.0)
        else:
            eng = nc.scalar if mh == 0 else nc.vector
            if mh == 0:
                nc.scalar.mul(out_t[:, R * mh: R * mh + R], ps[:, 0:R], scale)
            else:
                nc.vector.tensor_scalar_mul(
                    out_t[:, R * mh: R * mh + R], ps[:, 0:R], scale)


def _transpose_256(nc, ptr_pool, out_t, in_t, ident):
    """out = in^T for a 256x256 [128,512] tile (4 PE transposes)."""
    for i in range(2):
        for j in range(2):
            ps = ptr_pool.tile([H, H], F32, tag="tr")
            nc.tensor.transpose(
                ps[:, :],
                in_t[:, R * j + H * i: R * j + H * i + H].bitcast(F32),
                ident[:, :],
            )
            nc.vector.tensor_copy(out_t[:, R * i + H * j: R * i + H * j + H], ps[:, :])


def build_k1(bpc=BPC, n_rungs=N_RUNGS, ns_steps=NS_STEPS):
    nc = bacc.Bacc("TRN2", target_bir_lowering=False)
    sp_d = nc.dram_tensor("sp", [bpc, R, R], F32R, kind="ExternalInput")
    tp_d = nc.dram_tensor("tp", [bpc, R, R], F32R, kind="ExternalInput")
    t_d = nc.dram_tensor("t", [bpc, R, R], F32, kind="ExternalInput")
    w1t_d = nc.dram_tensor("w1t", [R, R], F32R, kind="ExternalInput")
    w2t_d = nc.dram_tensor("w2t", [R, R], F32R, kind="ExternalInput")
    w3_d = nc.dram_tensor("w3", [R, R], F32, kind="ExternalInput")
    w4_d = nc.dram_tensor("w4", [R, R], F32, kind="ExternalInput")
    ident_d = nc.dram_tensor("ident", [H, H], F32, kind="ExternalInput")
    eyema_d = nc.dram_tensor("eyema", [H, H], F32, kind="ExternalInput")  # MUO[0]*I
    blocktr_d = nc.dram_tensor("blocktr", [H, H], F32, kind="ExternalInput")
    seed_d = nc.dram_tensor("seed", [H, 2 * LA], F32R, kind="ExternalInput")
    ghbh_out = nc.dram_tensor("ghbh_out", [bpc, LA, 2 * LA], F32,
                              kind="ExternalOutput")
    vtwpt_out = nc.dram_tensor("vtwpt_out", [bpc, LA, 2 * R], F32R,
                               kind="ExternalOutput")

    n_pack = (bpc + GRP - 1) // GRP
    with tile.TileContext(nc) as tc, ExitStack() as ctx:
        cpool = ctx.enter_context(tc.tile_pool(name="consts", bufs=1))
        inpool = ctx.enter_context(tc.tile_pool(name="inp", bufs=2))
        tpool = ctx.enter_context(tc.tile_pool(name="trans", bufs=2))
        keep = ctx.enter_context(tc.tile_pool(name="keep", bufs=1))
        spool = ctx.enter_context(tc.tile_pool(name="small", bufs=2))
        sone = ctx.enter_context(tc.tile_pool(name="sone", bufs=1))
        pmm = ctx.enter_context(tc.tile_pool(name="pmm", bufs=2, space="PSUM"))
        ptr = ctx.enter_context(tc.tile_pool(name="ptr", bufs=2, space="PSUM"))
        psm = ctx.enter_context(tc.tile_pool(name="psm", bufs=2, space="PSUM"))
        psb = ctx.enter_context(tc.tile_pool(name="psb", bufs=2, space="PSUM"))

        w1t = cpool.tile([H, 2 * R], F32R); _load_256(nc, w1t, w1t_d)
        w2t = cpool.tile([H, 2 * R], F32R); _load_256(nc, w2t, w2t_d)
        w3 = cpool.tile([H, 2 * R], F32); _load_256(nc, w3, w3_d)
        w4 = cpool.tile([H, 2 * R], F32); _load_256(nc, w4, w4_d)
        ident = cpool.tile([H, H], F32)
        nc.sync.dma_start(out=ident[:, :], in_=ident_d[:, :])
        eyema = cpool.tile([H, H], F32)
        nc.sync.dma_start(out=eyema[:, :], in_=eyema_d[:, :])
        blocktr = cpool.tile([H, H], F32)
        nc.sync.dma_start(out=blocktr[:, :], in_=blocktr_d[:, :])
        seed = cpool.tile([H, 2 * LA], F32R)
        nc.sync.dma_start(out=seed[:, :], in_=seed_d[:, :])
        onescol = cpool.tile([H, 1], F32)
        nc.any.memset(onescol[:, :], 1.0)
        eyema_bf = cpool.tile([H, H], BF16)
        nc.vector.tensor_copy(eyema_bf[:, :], eyema[:, :])
        ident_bf = cpool.tile([H, H], BF16)
        nc.vector.tensor_copy(ident_bf[:, :], ident[:, :])

        ats, s0s, s1s, vs = [], [], [], []
        # ---- phase L: loads, A, A^T, G (per matrix; pipelines across b) ----
        sp2 = tp2 = t2 = None
        for b in range(bpc):
            if b % 2 == 0:
                # one DMA per tensor loads a PAIR of matrices [128, 1024]
                sp2 = inpool.tile([H, 4 * R], F32R, tag="sp")
                tp2 = inpool.tile([H, 4 * R], F32R, tag="tp")
                t2 = inpool.tile([H, 4 * R], F32, tag="t")
                for dst, src, eng in ((sp2, sp_d, nc.sync), (tp2, tp_d, nc.gpsimd),
                                      (t2, t_d, nc.gpsimd)):
                    eng.dma_start(
                        out=dst[:, :].rearrange("p (b hh c) -> p b hh c",
                                                b=2, hh=2, c=R),
                        in_=src[b:b + 2].rearrange("b (hh p) c -> p b hh c", p=H),
                    )
            m = b % 2
            sp_t = sp2[:, 2 * R * m: 2 * R * m + 2 * R]
            tp_t = tp2[:, 2 * R * m: 2 * R * m + 2 * R]
            t_t = t2[:, 2 * R * m: 2 * R * m + 2 * R]

            x1 = tpool.tile([H, 2 * R], F32, tag="x1")
            nc.vector.tensor_mul(x1[:, :], w4[:, :], tp_t[:, :].bitcast(F32))
            x2 = tpool.tile([H, 2 * R], F32, tag="x2")
            nc.vector.tensor_mul(x2[:, :], w3[:, :], t_t[:, :])
            nc.vector.tensor_add(x1[:, :], x1[:, :], x2[:, :])
            a_t = tpool.tile([H, 2 * R], F32R, tag="a")
            for mh in range(2):
                ps = pmm.tile([H, 2 * R], F32, tag="wide")
                for kh in range(2):
                    nc.tensor.matmul(
                        ps[:, 0:R],
                        w1t[:, R * kh + H * mh: R * kh + H * mh + H],
                        sp_t[:, R * kh: R * kh + R],
                        start=(kh == 0), stop=False,
                    )
                for kh in range(2):
                    nc.tensor.matmul(
                        ps[:, 0:R],
                        w2t[:, R * kh + H * mh: R * kh + H * mh + H],
                        tp_t[:, R * kh: R * kh + R],
                        start=False, stop=(kh == 1),
                    )
                nc.vector.tensor_add(
                    a_t[:, R * mh: R * mh + R], ps[:, 0:R],
                    x1[:, R * mh: R * mh + R],
                )
            at_t = keep.tile([H, 2 * R], F32R, tag=f"at{b}")
            _transpose_256(nc, ptr, at_t, a_t, ident)
            s0_t = keep.tile([H, 2 * R], F32R, tag=f"s0_{b}")
            _mm256_wide(nc, pmm, s0_t, a_t, a_t)          # G
            s1_t = keep.tile([H, 2 * R], F32R, tag=f"s1_{b}")
            v_t = keep.tile([H, 2 * LA], F32R, tag=f"v{b}")
            nc.vector.tensor_copy(v_t[:, :], seed[:, :].bitcast(F32))
            ats.append(at_t); s0s.append(s0_t); s1s.append(s1_t); vs.append(v_t)

        # ---- phase C: chain G2..G16, step-major so the PE never stalls ----
        for b in range(bpc):                               # G2 = (G^2)*2^-21
            _mm256_wide(nc, pmm, s1s[b], s0s[b], s0s[b], scale=G2_SCALE)
        for b in range(bpc):                               # G4
            _mm256_wide(nc, pmm, s0s[b], s1s[b], s1s[b])
        for b in range(bpc):                               # G8
            _mm256_wide(nc, pmm, s1s[b], s0s[b], s0s[b])
        for b in range(bpc):                               # G16 -> hs = s0s
            _mm256_wide(nc, pmm, s0s[b], s1s[b], s1s[b])
        hs = s0s

        # ---- phase R: rungs, the 4 packs' NS chains interleaved ----
        a_c, b_c, c_c = MUO
        for r in range(n_rungs):
            mbds, cts, yts = [], [], []
            for p in range(n_pack):
                mbd = sone.tile([H, H], F32, tag=f"mbd{p}")
                nc.any.memset(mbd[:, :], 0.0)
                mbds.append(mbd)
            for b in range(bpc):
                p, sl = b // GRP, (b % GRP) * 32
                yt_ps = psm.tile([LA, R], F32, tag="sm")
                for kh in range(2):
                    nc.tensor.matmul(
                        yt_ps[:, :],
                        vs[b][:, LA * kh: LA * kh + LA],
                        hs[b][:, R * kh: R * kh + R],
                        start=(kh == 0), stop=(kh == 1),
                    )
                yt_t = sone.tile([LA, R], F32, tag=f"ytt{b}")
                nc.vector.tensor_copy(yt_t[:, :], yt_ps[:, :])
                y_t = spool.tile([H, 2 * LA], F32, tag="yy")
                for hh in range(2):
                    tr_ps = ptr.tile([H, LA], F32, tag="tr")
                    nc.tensor.transpose(
                        tr_ps[:, :],
                        yt_t[:, H * hh: H * hh + H],
                        ident[:LA, :LA],
                    )
                    nc.vector.tensor_copy(y_t[:, LA * hh: LA * hh + LA], tr_ps[:, :])
                # gram into the pack's block-diag tile
                m_ps = psb.tile([H, H], F32, tag="smb")
                for kh in range(2):
                    nc.tensor.matmul(
                        m_ps[sl:sl + LA, sl:sl + LA],
                        y_t[:, LA * kh: LA * kh + LA],
                        y_t[:, LA * kh: LA * kh + LA],
                        start=(kh == 0), stop=(kh == 1),
                        tile_position=(0, sl),
                    )
                nc.vector.tensor_copy(
                    mbds[p][sl:sl + LA, sl:sl + LA], m_ps[sl:sl + LA, sl:sl + LA]
                )
                yts.append(yt_t)

            # trace normalization, p-interleaved
            mns, rrvs = [], []
            for p in range(n_pack):
                masked = spool.tile([H, H], F32, tag="masked")
                nc.vector.tensor_mul(masked[:, :], mbds[p][:, :], ident[:, :])
                dr_ps = psb.tile([1, H], F32, tag="smb")
                nc.tensor.matmul(dr_ps[:, :], onescol[:, :], masked[:, :],
                                 start=True, stop=True)
                drow = spool.tile([1, H], F32, tag="drow")
                nc.vector.tensor_copy(drow[:, :], dr_ps[:, :])
                dg_ps = psb.tile([H, 1], F32, tag="smb")
                nc.tensor.transpose(dg_ps[:, :], drow[:, :], ident[:1, :1])
                diag = spool.tile([H, 1], F32, tag="diag")
                nc.vector.tensor_copy(diag[:, :], dg_ps[:, :])
                tr_ps = psb.tile([H, 1], F32, tag="smb")
                nc.tensor.matmul(tr_ps[:, :], blocktr[:, :], diag[:, :],
                                 start=True, stop=True)
                tre = spool.tile([H, 1], F32, tag="tre")
                nc.vector.tensor_scalar_add(tre[:, :], tr_ps[:, :], 1e-30)
                itv = spool.tile([H, 1], F32, tag="itv")
                nc.vector.reciprocal(itv[:, :], tre[:, :])
                sq = spool.tile([H, 1], F32, tag="sq")
                nc.scalar.activation(
                    sq[:, :], tre[:, :], mybir.ActivationFunctionType.Sqrt,
                )
                rrv = sone.tile([H, 1], F32, tag=f"rrv{p}")
                nc.vector.reciprocal(rrv[:, :], sq[:, :])
                mn = sone.tile([H, H], BF16, tag=f"mn{p}")
                nc.vector.tensor_scalar_mul(mn[:, :], mbds[p][:, :], itv[:, :])
                mns.append(mn); rrvs.append(rrv)

            # quintic NS, steps interleaved across the 4 packs
            mcurs = list(mns)
            cts = [sone.tile([H, H], BF16, tag=f"ct{p}", name=f"ct{p}")
                   for p in range(n_pack)]
            for st in range(ns_steps):
                m2_pss, csts = [], []
                for p in range(n_pack):
                    m2_ps = psb.tile([H, H], F32, tag="smb")
                    nc.tensor.matmul(m2_ps[:, :], mcurs[p][:, :], mcurs[p][:, :],
                                     start=True, stop=True)
                    m2_pss.append(m2_ps)
                for p in range(n_pack):
                    cst = sone.tile([H, H], BF16, tag=f"cst{p}")
                    nc.vector.tensor_scalar_mul(
                        cst[:, :], mcurs[p][:, :].bitcast(BF16), b_c)
                    nc.vector.tensor_add(cst[:, :], cst[:, :], eyema_bf[:, :])
                    m2s = spool.tile([H, H], BF16, tag="m2s")
                    nc.scalar.mul(m2s[:, :], m2_pss[p][:, :], c_c)
                    nc.vector.tensor_add(cst[:, :], cst[:, :], m2s[:, :])
                    csts.append(cst)
                if st < ns_steps - 1:
                    cms = []
                    for p in range(n_pack):
                        cm_ps = psb.tile([H, H], F32, tag="smb")
                        nc.tensor.matmul(cm_ps[:, :], csts[p][:, :], mcurs[p][:, :],
                                         start=True, stop=True)
                        cm = spool.tile([H, H], BF16, tag=f"cm{p}")
                        nc.vector.tensor_copy(cm[:, :], cm_ps[:, :])
                        cms.append(cm)
                    for p in range(n_pack):
                        mn2_ps = psb.tile([H, H], F32, tag="smb")
                        nc.tensor.matmul(mn2_ps[:, :], cms[p][:, :], csts[p][:, :],
                                         start=True, stop=True)
                        mnew = sone.tile([H, H], BF16, tag=f"mnew{p}_{st}")
                        nc.vector.tensor_copy(mnew[:, :], mn2_ps[:, :])
                        mcurs[p] = mnew
                for p in range(n_pack):
                    if st == 0:
                        nc.vector.tensor_copy(cts[p][:, :], csts[p][:, :])
                    else:
                        ct_ps = psb.tile([H, H], F32, tag="smb")
                        nc.tensor.matmul(ct_ps[:, :], cts[p][:, :], csts[p][:, :],
                                         start=True, stop=True)
                        nc.vector.tensor_copy(cts[p][:, :], ct_ps[:, :])
            for p in range(n_pack):
                nc.vector.tensor_scalar_mul(cts[p][:, :], cts[p][:, :], rrvs[p][:, :])

            # extract each pack's diag blocks to partition base 0 via an
            # identity matmul (operands share base sl; out lands at base 0)
            ct0s = []
            for p in range(n_pack):
                for kk in range(GRP):
                    sl = kk * 32
                    c0_ps = psb.tile([LA, LA], F32, tag="smb")
                    nc.tensor.matmul(
                        c0_ps[:, :],
                        ident_bf[sl:sl + LA, sl:sl + LA],
                        cts[p][sl:sl + LA, sl:sl + LA],
                        start=True, stop=True,
                        tile_position=(sl, 0),
                    )
                    ct0 = sone.tile([LA, LA], F32, tag=f"ct0_{p}_{kk}",
                                    name=f"ct0_{p}_{kk}")
                    nc.vector.tensor_copy(ct0[:, :], c0_ps[:, :])
                    ct0s.append(ct0)
            # apply: V_b = Y_b @ Ct0_b (all operands at base 0)
            for b in range(bpc):
                for hh in range(2):
                    vp = ptr.tile([H, LA], F32, tag="tr")
                    nc.tensor.matmul(
                        vp[:, :],
                        yts[b][:, H * hh: H * hh + H],
                        ct0s[b][:, :],
                        start=True, stop=True,
                    )
                    nc.vector.tensor_copy(
                        vs[b][:, LA * hh: LA * hh + LA], vp[:, :]
                    )

        # ---- phase O: outputs Gh, Bh, Vt, Wpt (pipelines across b) ----
        for b in range(bpc):
            zt_ps = psm.tile([LA, R], F32, tag="sm")
            for kh in range(2):
                nc.tensor.matmul(
                    zt_ps[:, :],
                    vs[b][:, LA * kh: LA * kh + LA],
                    hs[b][:, R * kh: R * kh + R],
                    start=(kh == 0), stop=(kh == 1),
                )
            zt_t = spool.tile([LA, R], F32, tag="ztt")
            nc.vector.tensor_copy(zt_t[:, :], zt_ps[:, :])
            z_t = spool.tile([H, 2 * LA], F32, tag="zz")
            for hh in range(2):
                tr_ps = ptr.tile([H, LA], F32, tag="tr")
                nc.tensor.transpose(
                    tr_ps[:, :], zt_t[:, H * hh: H * hh + H],
                    ident[:LA, :LA],
                )
                nc.vector.tensor_copy(z_t[:, LA * hh: LA * hh + LA], tr_ps[:, :])
            ghbh_t = spool.tile([LA, 2 * LA], F32, tag="ghbh")
            gh_ps = psb.tile([LA, LA], F32, tag="smb")
            for kh in range(2):
                nc.tensor.matmul(
                    gh_ps[:, :],
                    z_t[:, LA * kh: LA * kh + LA],
                    vs[b][:, LA * kh: LA * kh + LA].bitcast(F32),
                    start=(kh == 0), stop=(kh == 1),
                )
            nc.vector.tensor_copy(ghbh_t[:, 0:LA], gh_ps[:, :])
            bh_ps = psb.tile([LA, LA], F32, tag="smb")
            for kh in range(2):
                nc.tensor.matmul(
                    bh_ps[:, :],
                    vs[b][:, LA * kh: LA * kh + LA].bitcast(F32),
                    vs[b][:, LA * kh: LA * kh + LA].bitcast(F32),
                    start=(kh == 0), stop=(kh == 1),
                )
            nc.vector.tensor_copy(ghbh_t[:, LA:2 * LA], bh_ps[:, :])
            nc.sync.dma_start(out=ghbh_out[b], in_=ghbh_t[:, :])

            vw_t = spool.tile([LA, 2 * R], F32R, tag="vw")
            for hh in range(2):
                tr_ps = psm.tile([LA, H], F32, tag="sm")
                nc.tensor.transpose(
                    tr_ps[:, :],
                    vs[b][:, LA * hh: LA * hh + LA].bitcast(F32),
                    ident[:, :],
                )
                nc.vector.tensor_copy(vw_t[:, H * hh: H * hh + H], tr_ps[:, :])
            wpt_ps = psm.tile([LA, R], F32, tag="sm")
            for kh in range(2):
                nc.tensor.matmul(
                    wpt_ps[:, :],
                    vs[b][:, LA * kh: LA * kh + LA],
                    ats[b][:, R * kh: R * kh + R],
                    start=(kh == 0), stop=(kh == 1),
                )
            nc.vector.tensor_copy(vw_t[:, R:2 * R], wpt_ps[:, :])
            nc.sync.dma_start(out=vtwpt_out[b], in_=vw_t[:, :])
    nc.compile()
    return nc


def build_k2(bpc=BPC, stage=3):
    nc = bacc.Bacc("TRN2", target_bir_lowering=False)
    sp_d = nc.dram_tensor("sp", [bpc, R, R], F32, kind="ExternalInput")
    # packed per-matrix smalls: [vt | wpt | ms] = [32, 256+256+32]
    vwm_d = nc.dram_tensor("vwm", [bpc, LA, 2 * R + LA], F32R,
                           kind="ExternalInput")
    ident_d = nc.dram_tensor("ident", [H, H], F32, kind="ExternalInput")
    invc2_d = nc.dram_tensor("invc2", [1, 1022], F32, kind="ExternalInput")
    tpn_out = nc.dram_tensor("tpn_out", [bpc, R, R], F32, kind="ExternalOutput")
    spn_out = nc.dram_tensor("spn_out", [bpc, R, R], F32, kind="ExternalOutput")
    scr1 = nc.dram_tensor("scr1", [bpc, SHEAR_N], BF16)
    scrq = nc.dram_tensor("scrq", [bpc, QBUF_N], F32)
    npair = bpc // 2

    with tile.TileContext(nc) as tc, ExitStack() as ctx:
        cpool = ctx.enter_context(tc.tile_pool(name="consts", bufs=1))
        inpool = ctx.enter_context(tc.tile_pool(name="inp", bufs=2))
        tpool = ctx.enter_context(tc.tile_pool(name="trans", bufs=2))
        keep = ctx.enter_context(tc.tile_pool(name="keep", bufs=1))
        spool = ctx.enter_context(tc.tile_pool(name="small", bufs=3))
        pmm = ctx.enter_context(tc.tile_pool(name="pmm", bufs=2, space="PSUM"))
        ptr = ctx.enter_context(tc.tile_pool(name="ptr", bufs=2, space="PSUM"))
        psm = ctx.enter_context(tc.tile_pool(name="psm", bufs=2, space="PSUM"))

        ident = cpool.tile([H, H], F32)
        nc.sync.dma_start(out=ident[:, :], in_=ident_d[:, :])
        invc2 = cpool.tile([1, 1022], F32)
        nc.sync.dma_start(out=invc2[:, :], in_=invc2_d[:, :])
        ones = cpool.tile([H, 1], BF16)
        nc.any.memset(ones[:, :], 1.0)
        onesr = cpool.tile([1, H], BF16)
        nc.any.memset(onesr[:, :], 1.0)

        # per-pair zero-padded staging tiles (pads stay zero) + results
        m2zs, tpns, sps = [], [], []
        for q in range(npair):
            m2z = keep.tile([H, 2048], BF16, tag=f"m2z{q}", name=f"m2z{q}")
            nc.any.memset(m2z[:, :], 0.0)
            m2zs.append(m2z)
            tpn = keep.tile([H, 4 * R], F32, tag=f"tpn{q}", name=f"tpn{q}")
            tpns.append(tpn)
        # one DMA zeroes the shear-gap head [0,255) of every matrix slot
        nc.sync.dma_start(out=scr1[:, 0:255], in_=m2zs[0][0:bpc, 256:511])

        # ---- phase A: tpn + shear writes (pipelines across pairs) ----
        for q in range(npair):
            b0 = 2 * q
            sp2 = keep.tile([H, 4 * R], F32, tag=f"sp{q}", name=f"sp{q}")
            nc.sync.dma_start(
                out=sp2[:, :].rearrange("p (b hh c) -> p b hh c",
                                        b=2, hh=2, c=R),
                in_=sp_d[b0:b0 + 2].rearrange("b (hh p) c -> p b hh c", p=H),
            )
            sps.append(sp2)
            vwm2 = inpool.tile([LA, 2 * (2 * R + LA)], F32R, tag="vwm")
            nc.gpsimd.dma_start(
                out=vwm2[:, :].rearrange("p (b c) -> p b c", b=2),
                in_=vwm_d[b0:b0 + 2].rearrange("b p c -> p b c"),
            )
            for m in range(2):
                b = b0 + m
                W = 2 * R + LA
                sp_t = sp2[:, 2 * R * m: 2 * R * m + 2 * R]
                vt_t = vwm2[:, W * m: W * m + R]
                wpt_t = vwm2[:, W * m + R: W * m + 2 * R]
                ms_t = vwm2[:, W * m + 2 * R: W * m + 2 * R + LA]

                # Sp^T via PE transposes (copies split DVE/Act)
                spt_t = tpool.tile([H, 2 * R], F32, tag="spt")
                for i in range(2):
                    for j in range(2):
                        ps = ptr.tile([H, H], F32, tag="tr")
                        nc.tensor.transpose(
                            ps[:, :],
                            sp_t[:, R * j + H * i: R * j + H * i + H],
                            ident[:, :],
                        )
                        if (i + j) % 2 == 0:
                            nc.vector.tensor_copy(
                                spt_t[:, R * i + H * j: R * i + H * j + H],
                                ps[:, :])
                        else:
                            nc.scalar.mul(
                                spt_t[:, R * i + H * j: R * i + H * j + H],
                                ps[:, :], 1.0)

                # P1 = Ms @ Wpt ; P2 = Ms @ Vt   (Ms symmetric)
                p1_ps = psm.tile([LA, R], F32, tag="sm")
                nc.tensor.matmul(p1_ps[:, :], ms_t, wpt_t, start=True, stop=True)
                p1_t = spool.tile([LA, R], F32R, tag="p1")
                nc.vector.tensor_copy(p1_t[:, :], p1_ps[:, :])
                p2_ps = psm.tile([LA, R], F32, tag="sm")
                nc.tensor.matmul(p2_ps[:, :], ms_t, vt_t, start=True, stop=True)
                p2_t = spool.tile([LA, R], F32R, tag="p2")
                nc.vector.tensor_copy(p2_t[:, :], p2_ps[:, :])

                # TpnewT = V P1 ; Tpnew = W' P2  (fp32r, 256-wide)
                tpnT = tpool.tile([H, 2 * R], F32, tag="tpnT")
                tpn = tpns[q]
                for hh in range(2):
                    ps = pmm.tile([H, 2 * R], F32, tag="wide")
                    nc.tensor.matmul(
                        ps[:, 0:R],
                        vt_t[:, H * hh: H * hh + H],
                        p1_t[:, :],
                        start=True, stop=True,
                    )
                    nc.vector.tensor_copy(tpnT[:, R * hh: R * hh + R], ps[:, 0:R])
                    ps2 = pmm.tile([H, 2 * R], F32, tag="wide")
                    nc.tensor.matmul(
                        ps2[:, 0:R],
                        wpt_t[:, H * hh: H * hh + H],
                        p2_t[:, :],
                        start=True, stop=True,
                    )
                    nc.scalar.mul(
                        tpn[:, 2 * R * m + R * hh: 2 * R * m + R * hh + R],
                        ps2[:, 0:R], 1.0)
                if stage >= 2:
                    # M2T = 2*TpnewT - SpT into this pair's bf16 staging
                    m2z = m2zs[q]
                    for hh in range(2):
                        o = 1024 * m + 512 * hh
                        nc.vector.tensor_scalar_mul(
                            m2z[:, o: o + R],
                            tpnT[:, R * hh: R * hh + R], 2.0,
                        )
                        nc.vector.tensor_tensor(
                            out=m2z[:, o: o + R],
                            in0=m2z[:, o: o + R],
                            in1=spt_t[:, R * hh: R * hh + R],
                            op=mybir.AluOpType.subtract,
                        )
            # Tpnew out, one DMA per pair (natural layout)
            nc.sync.dma_start(
                out=tpn_out[b0:b0 + 2].rearrange("b (hh p) c -> p b hh c", p=H),
                in_=tpns[q][:, :].rearrange("p (b hh c) -> p b hh c",
                                            b=2, hh=2, c=R),
            )
            if stage < 2:
                continue
            # shear-write the pair, one DMA per row-half (3D APs balance)
            for hh in range(2):
                o = 255 + 511 * H * hh
                nc.gpsimd.dma_start(
                    out=scr1[b0:b0 + 2, o: o + 511 * H].rearrange(
                        "b (p f) -> p b f", p=H),
                    in_=m2zs[q][:, :].rearrange(
                        "p (b hh x) -> p b hh x", b=2, hh=2)[:, :, hh, 0:511],
                )

        # ---- phase B: diagonal sums -> periodic Q buffer (per pair) ----
        if stage >= 2:
            for q in range(npair):
                b0 = 2 * q
                shm = tpool.tile([H, 2044], BF16, tag="shm")
                for hh in range(2):
                    nc.gpsimd.dma_start(
                        out=shm[:, :].rearrange("p (b hh f) -> p b hh f",
                                                b=2, hh=2)[:, :, hh],
                        in_=scr1[b0:b0 + 2, 512 * H * hh: 512 * H * hh
                                 + 512 * H].rearrange(
                            "b (p f) -> p b f", p=H)[:, :, 0:511],
                    )
                avg = spool.tile([1, 1022], BF16, tag="avg")
                for m in range(2):
                    sums_ps = psm.tile([1, 511], F32, tag="sm3")
                    for hh in range(2):
                        nc.tensor.matmul(
                            sums_ps[:, :], ones[:, :],
                            shm[:, 1022 * m + 511 * hh: 1022 * m + 511 * hh + 511],
                            start=(hh == 0), stop=(hh == 1))
                    nc.vector.tensor_mul(avg[:, 511 * m: 511 * m + 511],
                                         sums_ps[:1, :],
                                         invc2[:, 511 * m: 511 * m + 511])
                avgb = spool.tile([H, 1022], F32, tag="avgb")
                for m in range(2):
                    avgb_ps = pmm.tile([H, 2 * R], F32, tag="wide")
                    nc.tensor.matmul(avgb_ps[:, 0:511], onesr[:, :],
                                     avg[:, 511 * m: 511 * m + 511],
                                     start=True, stop=True)
                    if m == 0:
                        nc.vector.tensor_copy(avgb[:, 0:511], avgb_ps[:, 0:511])
                    else:
                        nc.scalar.mul(avgb[:, 511:1022], avgb_ps[:, 0:511], 1.0)
                # periodic Q: 128 rows + 1 wrap row of avg, per pair
                nc.sync.dma_start(
                    out=scrq[b0:b0 + 2, 0: 511 * H].rearrange(
                        "b (p f) -> p b f", p=H),
                    in_=avgb[:, :].rearrange("p (b f) -> p b f", b=2),
                )
                nc.sync.dma_start(
                    out=scrq[b0:b0 + 2, 511 * H: 511 * (H + 1)],
                    in_=avgb[0:1, :],
                )

        # ---- phase C: Toeplitz + Spnew (per pair) ----
        if stage >= 3:
            for q in range(npair):
                b0 = 2 * q
                # toepT[p, f] = avg[f - p + 255] via mod-511 reads, 1 DMA/pair
                # per matrix: cols [0,256) = rows 128..255 (base 127),
                # cols [256,512) = rows 0..127 (base 255)
                ttT = tpool.tile([H, 4 * R], F32, tag="ttT")
                for hh, base in ((1, 255), (0, 127)):
                    nc.gpsimd.dma_start(
                        out=ttT[:, :].rearrange("p (b hh f) -> p b hh f",
                                                b=2, hh=2, f=R)[:, :, hh],
                        in_=scrq[b0:b0 + 2, base: base + 510 * H].rearrange(
                            "b (p f) -> p b f", p=H, f=510)[:, :, 0:R],
                    )
                spn2 = tpool.tile([H, 4 * R], F32, tag="spn2")
                for m in range(2):
                    b = b0 + m
                    sp_t = sps[q][:, 2 * R * m: 2 * R * m + 2 * R]
                    # toep natural = transpose(toepT) (halves swapped in ttT)
                    ttN = tpool.tile([H, 2 * R], F32, tag="ttN")
                    for i in range(2):
                        for j in range(2):
                            ps = ptr.tile([H, H], F32, tag="tr")
                            nc.tensor.transpose(
                                ps[:, :],
                                ttT[:, 2 * R * m + R * (1 - j) + H * i:
                                    2 * R * m + R * (1 - j) + H * i + H],
                                ident[:, :],
                            )
                            if (i + j) % 2 == 0:
                                nc.vector.tensor_copy(
                                    ttN[:, R * i + H * j: R * i + H * j + H],
                                    ps[:, :])
                            else:
                                nc.scalar.mul(
                                    ttN[:, R * i + H * j: R * i + H * j + H],
                                    ps[:, :], 1.0)
                    # Spnew = Sp - Tpnew + toep_nat
                    o = 2 * R * m
                    nc.vector.tensor_tensor(
                        out=spn2[:, o: o + 2 * R], in0=sp_t,
                        in1=tpns[q][:, o: o + 2 * R],
                        op=mybir.AluOpType.subtract,
                    )
                    nc.vector.tensor_add(
                        spn2[:, o: o + 2 * R], spn2[:, o: o + 2 * R], ttN[:, :])
                nc.sync.dma_start(
                    out=spn_out[b0:b0 + 2].rearrange(
                        "b (hh p) c -> p b hh c", p=H),
                    in_=spn2[:, :].rearrange("p (b hh c) -> p b hh c",
                                             b=2, hh=2, c=R),
                )
    nc.compile()
    return nc


def _transpose_256f(nc, ptr_pool, out_t, in_t, ident):
    """out = in^T for a 256x256 [128,512] fp32 tile (4 PE transposes)."""
    for i in range(2):
        for j in range(2):
            ps = ptr_pool.tile([H, H], F32, tag="tr")
            nc.tensor.transpose(
                ps[:, :], in_t[:, R * j + H * i: R * j + H * i + H], ident[:, :]
            )
            nc.vector.tensor_copy(out_t[:, R * i + H * j: R * i + H * j + H], ps[:, :])


def _host_consts():
    ident = np.eye(H, dtype=np.float32)
    eyema = (MUO[0] * np.eye(H)).astype(np.float32)
    blocktr = np.zeros((H, H), np.float32)
    for g in range(4):
        blocktr[g * 32: g * 32 + LA, g * 32: g * 32 + 32] = 1.0
    i = np.arange(R, dtype=np.float32)[:, None]
    j = np.arange(LA, dtype=np.float32)[None, :]
    v0 = np.cos(0.37 * (i + 1) * (j + 1) + 0.11 * i).astype(np.float32)
    seed = np.concatenate([v0[0:H, :], v0[H:R, :]], axis=1)  # [128, 32]
    counts = (R - np.abs(np.arange(511) - 255)).astype(np.float32)
    invc = (1.0 / counts)[None, :].astype(np.float32)
    return ident, eyema, blocktr, seed, invc


def _host_bridge(gh, bh, Kv):
    """Robust whitened generalized eig; returns Ms = Z10 Z10^T per matrix."""
    n = gh.shape[0]
    ms = np.zeros((n, LA, LA), np.float32)
    for b in range(n):
        Gs = 0.5 * (gh[b] + gh[b].T).astype(np.float64)
        Bs = 0.5 * (bh[b] + bh[b].T).astype(np.float64)
        lb, Ub = np.linalg.eigh(Bs)
        lmax = max(float(lb.max()), 0.0)
        keep = lb > lmax * 1e-7 if lmax > 0 else lb > -1.0
        if not np.any(keep):
            continue
        Wh = Ub[:, keep] / np.sqrt(np.maximum(lb[keep], 1e-300))[None, :]
        Gw = Wh.T @ Gs @ Wh
        d, Qw = np.linalg.eigh(Gw)
        Z = Wh @ Qw[:, ::-1][:, :Kv]
        ms[b] = (Z @ Z.T).astype(np.float32)
    return ms


def _host_fallback(T, Tp, Sp, w1, w2, w3, w4, Kv):
    """Numpy implementation (used only if the device path fails)."""
    f32 = np.float32
    A = (np.einsum('rk,bkc->brc', w1, Sp) + np.einsum('rk,bkc->brc', w2, Tp)
         + w4[None] * Tp + w3[None] * T).astype(f32)
    G = np.einsum('brc,brd->bcd', A, A).astype(f32)
    d, q = np.linalg.eigh(G.astype(np.float64))
    Vs = q[:, :, ::-1][:, :, :Kv]
    AV = np.einsum('brc,bcl->brl', A.astype(np.float64), Vs)
    Tpnew = np.einsum('brl,bcl->brc', AV, Vs).astype(f32)
    m, n = R, R
    D = m + n - 1
    ii = np.arange(m)[:, None]; jj = np.arange(n)[None, :]
    dd = jj - ii + (m - 1)
    M2 = (2.0 * Tpnew - Sp).astype(f32)
    Z = np.zeros((M2.shape[0], m, D), f32)
    Z[:, ii, dd] = M2
    sums = Z.sum(axis=1)
    counts = (m - np.abs(np.arange(D) - (m - 1))).astype(f32)
    avg = sums / counts
    Spnew = (Sp - Tpnew + avg[:, dd]).astype(f32)
    return (T, Tpnew, Spnew)


_K1 = None
_K2 = None


def _get_kernels():
    global _K1, _K2
    if _K1 is None:
        _K1 = build_k1()
    if _K2 is None:
        _K2 = build_k2()
    return _K1, _K2


def _run_k2(Sp, vt_all, wpt_all, ms_all, nc2=None):
    ident, eyema, blocktr, seed, invc = _host_consts()
    if nc2 is None:
        nc2 = build_k2()
    vwm = np.concatenate([vt_all, wpt_all, ms_all], axis=2)  # [B, 16, 528]
    vwm = np.ascontiguousarray(vwm, dtype=np.float32)
    in_maps = []
    for c in range(N_CORES):
        sl = slice(c * BPC, (c + 1) * BPC)
        in_maps.append({
            "sp": Sp[sl], "vwm": vwm[sl], "ident": ident,
            "invc2": np.concatenate([invc, invc], axis=1),
        })
    r2 = run_bass_kernel_spmd(nc2, in_maps, list(range(N_CORES)))
    LAST_EXEC_NS[1] = r2.exec_time_ns
    res2 = r2.results
    Tpnew = np.concatenate([res2[c]["tpn_out"] for c in range(N_CORES)], axis=0)
    Spnew = np.concatenate([res2[c]["spn_out"] for c in range(N_CORES)], axis=0)
    return Tpnew, Spnew


def _kernel_device(T, Tp, Sp, w1, w2, w3, w4, Kv):
    ident, eyema, blocktr, seed, invc = _host_consts()
    w1t = np.ascontiguousarray(w1.T)
    w2t = np.ascontiguousarray(w2.T)
    nc1, nc2 = _get_kernels()
    in_maps1 = []
    for c in range(N_CORES):
        sl = slice(c * BPC, (c + 1) * BPC)
        in_maps1.append({
            "sp": Sp[sl], "tp": Tp[sl], "t": T[sl],
            "w1t": w1t, "w2t": w2t, "w3": w3, "w4": w4,
            "ident": ident, "eyema": eyema, "blocktr": blocktr, "seed": seed,
        })
    r1 = run_bass_kernel_spmd(nc1, in_maps1, list(range(N_CORES)))
    LAST_EXEC_NS[0] = r1.exec_time_ns
    res1 = r1.results
    ghbh = np.concatenate([res1[c]["ghbh_out"] for c in range(N_CORES)], axis=0)
    vtwpt = np.concatenate([res1[c]["vtwpt_out"] for c in range(N_CORES)], axis=0)
    gh, bh = ghbh[:, :, 0:LA], ghbh[:, :, LA:2 * LA]
    vt_all, wpt_all = vtwpt[:, :, 0:R], vtwpt[:, :, R:2 * R]
    ms_all = _host_bridge(gh, bh, Kv)
    Tpnew, Spnew = _run_k2(Sp, vt_all, wpt_all, ms_all, nc2=nc2)
    return (T, Tpnew, Spnew)


def _kernel_hybrid(T, Tp, Sp, w1, w2, w3, w4, Kv):
    """Host eigensolve for the subspace + device K2 for apply/averaging."""
    f32 = np.float32
    A = (np.einsum('rk,bkc->brc', w1, Sp) + np.einsum('rk,bkc->brc', w2, Tp)
         + w4[None] * Tp + w3[None] * T).astype(f32)
    G = np.einsum('brc,brd->bcd', A, A)
    d, q = np.linalg.eigh(G.astype(np.float64))
    Vs = q[:, :, ::-1][:, :, :Kv]                       # [B, 256, K]
    vt_all = np.zeros((B_FULL, LA, R), f32)
    vt_all[:, :Kv, :] = Vs.transpose(0, 2, 1).astype(f32)
    AV = np.einsum('brc,bcl->brl', A.astype(np.float64), Vs)
    wpt_all = np.zeros((B_FULL, LA, R), f32)
    wpt_all[:, :Kv, :] = AV.transpose(0, 2, 1).astype(f32)
    ms_all = np.zeros((B_FULL, LA, LA), f32)
    ms_all[:, :Kv, :Kv] = np.eye(Kv, dtype=f32)[None]
    Tpnew, Spnew = _run_k2(Sp, vt_all, wpt_all, ms_all)
    return (T, Tpnew, Spnew)


def kernel(T, Tp, Sp, w1, w2, w3, w4, K):
    T = np.ascontiguousarray(np.asarray(T, dtype=np.float32))
    Tp = np.ascontiguousarray(np.asarray(Tp, dtype=np.float32))
    Sp = np.ascontiguousarray(np.asarray(Sp, dtype=np.float32))
    w1 = np.asarray(w1, dtype=np.float32); w2 = np.asarray(w2, dtype=np.float32)
    w3 = np.asarray(w3, dtype=np.float32); w4 = np.asarray(w4, dtype=np.float32)
    Kv = int(np.asarray(K))
    try:
        return _kernel_device(T, Tp, Sp, w1, w2, w3, w4, Kv)
    except Exception:
        import traceback
        traceback.print_exc()
        print("K1 device path failed; host eigensolve + device K2")
    try:
        return _kernel_hybrid(T, Tp, Sp, w1, w2, w3, w4, Kv)
    except Exception:
        import traceback
        traceback.print_exc()
        print("hybrid path failed; full host fallback")
        return _host_fallback(T, Tp, Sp, w1, w2, w3, w4, Kv)


LAST_EXEC_NS = [None, None]


# revision 46
# speedup vs baseline: 2.0946x; 1.1224x over previous
"""Cadzow update (batched rank-K truncation + Toeplitz averaging) on 8 trn2 cores.

Data-parallel over the batch of 128 matrices (16 per core). Per matrix:
  A = w1@Sp + w2@Tp + w4*Tp + w3*T
  rank-K via oversampled subspace iteration + host Rayleigh-Ritz:
    K1 (device): G = A^T A; chain G2=(G^2*2^-21), G4, G8, G16 (fp32r matmuls);
      3 rungs V <- orth(G16 V) with a quintic Newton-Schulz Gram conditioner
      (4 matrices packed per 128x128 block-diag tile); ships per matrix
      Gh = V^T G16 V, Bh = V^T V (16x16), Vt = V^T and Wpt = (A V)^T.
    host: robust whitened generalized eig of (Gh, Bh); top-K selector
      Ms = Z10 Z10^T (16x16).
    K2 (device): Tpnew = Wpt^T Ms Vt (both orientations from the small
      factors); Spnew = Sp - Tpnew + avgdiag(2 Tpnew - Sp) where the
      diagonal averaging runs via a shear-DMA layout (diag sums by
      ones-matmul) and the Toeplitz broadcast is read back from a
      mod-511 periodic DRAM buffer with all-positive strides.

All big matmuls run as fp32r (~4x PE throughput at >=256-wide outputs);
the 16x16 Grams / Newton-Schulz stay fp32. Outputs are written in natural
layout (no 4-byte-granular transposed DMA anywhere).
"""
import os
import numpy as np
from contextlib import ExitStack

# The axon ntff profile hook (antenv.axon_hooks) is absent in this image;
# a set BASS_TRACE would crash run_bass_kernel_spmd, so clear it.
os.environ.pop("BASS_TRACE", None)

import concourse.bass as bass
import concourse.bacc as bacc
import concourse.mybir as mybir
from concourse import tile
from concourse.bass_utils import run_bass_kernel_spmd

F32 = mybir.dt.float32
F32R = mybir.dt.float32r
BF16 = mybir.dt.bfloat16
N_CORES = 8
B_FULL = 128
BPC = B_FULL // N_CORES          # 16 matrices per core
R = 256
LA = 32                          # subspace dim (oversampled, 4x32 pack)
H = 128                          # partitions
GRP = 4                          # matrices packed per 128x128 Gram tile
N_RUNGS = 1
NS_STEPS = 3
MUO = (3.4445, -4.7750, 2.0315)  # quintic NS coefficients
G2_SCALE = 2.0 ** -21

SHEAR_N = 512 * 257              # shear scratch elems per matrix
QBUF_N = 511 * 129               # periodic Toeplitz buffer elems per matrix


def _halfslc(hh, w=R):
    return slice(w * hh, w * hh + w)


def _load_256(nc, dst, src_b):
    """DRAM (256, X) -> SBUF [128, 2X] (row halves side by side)."""
    X = src_b.shape[-1]
    nc.sync.dma_start(out=dst[:, 0:X], in_=src_b[0:H, :])
    nc.sync.dma_start(out=dst[:, X:2 * X], in_=src_b[H:2 * H, :])


def _mm256_wide(nc, psum_pool, out_t, lhs_t, rhs_t, scale=None, alt=0):
    """out = L^T @ Rhs for 256x256 [128,512]-tiled operands (4 matmuls,
    both output halves in one psum bank, a single copy out)."""
    ps = psum_pool.tile([H, 2 * R], F32, tag="wide")
    for mh in range(2):
        for kh in range(2):
            nc.tensor.matmul(
                ps[:, R * mh: R * mh + R],
                lhs_t[:, R * kh + H * mh: R * kh + H * mh + H],
                rhs_t[:, R * kh: R * kh + R],
                start=(kh == 0), stop=(kh == 1),
            )
    if scale is None:
        if alt == 0:
            nc.vector.tensor_copy(out_t[:, :], ps[:, :])
        else:
            nc.scalar.mul(out_t[:, :], ps[:, :], 1.0)
    else:
        if alt == 0:
            nc.vector.tensor_scalar_mul(out_t[:, :], ps[:, :], scale)
        else:
            nc.scalar.mul(out_t[:, :], ps[:, :], scale)


def _transpose_256(nc, ptr_pool, out_t, in_t, ident, alt=0):
    """out = in^T for a 256x256 [128,512] tile (4 PE transposes, 2 copies)."""
    for i in range(2):
        ps = ptr_pool.tile([H, 2 * H], F32, tag="tr")
        for j in range(2):
            nc.tensor.transpose(
                ps[:, H * j: H * j + H],
                in_t[:, R * j + H * i: R * j + H * i + H].bitcast(F32),
                ident[:, :],
            )
        if (i + alt) % 2 == 0:
            nc.vector.tensor_copy(out_t[:, R * i: R * i + R], ps[:, :])
        else:
            nc.scalar.mul(out_t[:, R * i: R * i + R], ps[:, :], 1.0)


def build_k1(bpc=BPC, n_rungs=N_RUNGS, ns_steps=NS_STEPS):
    nc = bacc.Bacc("TRN2", target_bir_lowering=False)
    sp_d = nc.dram_tensor("sp", [bpc, R, R], F32R, kind="ExternalInput")
    tp_d = nc.dram_tensor("tp", [bpc, R, R], F32R, kind="ExternalInput")
    t_d = nc.dram_tensor("t", [bpc, R, R], F32, kind="ExternalInput")
    w1t_d = nc.dram_tensor("w1t", [R, R], F32R, kind="ExternalInput")
    w2t_d = nc.dram_tensor("w2t", [R, R], F32R, kind="ExternalInput")
    w3_d = nc.dram_tensor("w3", [R, R], F32, kind="ExternalInput")
    w4_d = nc.dram_tensor("w4", [R, R], F32, kind="ExternalInput")
    ident_d = nc.dram_tensor("ident", [H, H], F32, kind="ExternalInput")
    eyema_d = nc.dram_tensor("eyema", [H, H], F32, kind="ExternalInput")  # MUO[0]*I
    blocktr_d = nc.dram_tensor("blocktr", [H, H], F32, kind="ExternalInput")
    seed_d = nc.dram_tensor("seed", [H, 2 * LA], F32R, kind="ExternalInput")
    ghbh_out = nc.dram_tensor("ghbh_out", [bpc, LA, 2 * LA], F32,
                              kind="ExternalOutput")
    vtwpt_out = nc.dram_tensor("vtwpt_out", [bpc, LA, 2 * R], F32R,
                               kind="ExternalOutput")

    n_pack = (bpc + GRP - 1) // GRP
    with tile.TileContext(nc) as tc, ExitStack() as ctx:
        cpool = ctx.enter_context(tc.tile_pool(name="consts", bufs=1))
        inpool = ctx.enter_context(tc.tile_pool(name="inp", bufs=2))
        tpool = ctx.enter_context(tc.tile_pool(name="trans", bufs=2))
        keep = ctx.enter_context(tc.tile_pool(name="keep", bufs=1))
        spool = ctx.enter_context(tc.tile_pool(name="small", bufs=2))
        sone = ctx.enter_context(tc.tile_pool(name="sone", bufs=1))
        pmm = ctx.enter_context(tc.tile_pool(name="pmm", bufs=2, space="PSUM"))
        ptr = ctx.enter_context(tc.tile_pool(name="ptr", bufs=2, space="PSUM"))
        psm = ctx.enter_context(tc.tile_pool(name="psm", bufs=2, space="PSUM"))
        psb = ctx.enter_context(tc.tile_pool(name="psb", bufs=2, space="PSUM"))

        w1t = cpool.tile([H, 2 * R], F32R); _load_256(nc, w1t, w1t_d)
        w2t = cpool.tile([H, 2 * R], F32R); _load_256(nc, w2t, w2t_d)
        w3 = cpool.tile([H, 2 * R], F32); _load_256(nc, w3, w3_d)
        w4 = cpool.tile([H, 2 * R], F32); _load_256(nc, w4, w4_d)
        ident = cpool.tile([H, H], F32)
        nc.sync.dma_start(out=ident[:, :], in_=ident_d[:, :])
        eyema = cpool.tile([H, H], F32)
        nc.sync.dma_start(out=eyema[:, :], in_=eyema_d[:, :])
        blocktr = cpool.tile([H, H], F32)
        nc.sync.dma_start(out=blocktr[:, :], in_=blocktr_d[:, :])
        seed = cpool.tile([H, 2 * LA], F32R)
        nc.sync.dma_start(out=seed[:, :], in_=seed_d[:, :])
        onescol = cpool.tile([H, 1], F32)
        nc.any.memset(onescol[:, :], 1.0)
        eyema_bf = cpool.tile([H, H], BF16)
        nc.vector.tensor_copy(eyema_bf[:, :], eyema[:, :])
        ident_bf = cpool.tile([H, H], BF16)
        nc.vector.tensor_copy(ident_bf[:, :], ident[:, :])

        ats, s0s, s1s, vs = [], [], [], []
        # ---- phase L: loads, A, A^T, G (per matrix; pipelines across b) ----
        sp2 = tp2 = t2 = None
        for b in range(bpc):
            if b % 2 == 0:
                # one DMA per tensor loads a PAIR of matrices [128, 1024]
                sp2 = inpool.tile([H, 4 * R], F32R, tag="sp")
                tp2 = inpool.tile([H, 4 * R], F32R, tag="tp")
                t2 = inpool.tile([H, 4 * R], F32, tag="t")
                for dst, src, eng in ((sp2, sp_d, nc.sync), (tp2, tp_d, nc.gpsimd),
                                      (t2, t_d, nc.gpsimd)):
                    eng.dma_start(
                        out=dst[:, :].rearrange("p (b hh c) -> p b hh c",
                                                b=2, hh=2, c=R),
                        in_=src[b:b + 2].rearrange("b (hh p) c -> p b hh c", p=H),
                    )
            m = b % 2
            sp_t = sp2[:, 2 * R * m: 2 * R * m + 2 * R]
            tp_t = tp2[:, 2 * R * m: 2 * R * m + 2 * R]
            t_t = t2[:, 2 * R * m: 2 * R * m + 2 * R]

            x1 = tpool.tile([H, 2 * R], F32, tag="x1")
            nc.vector.tensor_mul(x1[:, :], w4[:, :], tp_t[:, :].bitcast(F32))
            x2 = tpool.tile([H, 2 * R], F32, tag="x2")
            nc.vector.tensor_mul(x2[:, :], w3[:, :], t_t[:, :])
            nc.vector.tensor_add(x1[:, :], x1[:, :], x2[:, :])
            a_t = tpool.tile([H, 2 * R], F32R, tag="a")
            ps = pmm.tile([H, 2 * R], F32, tag="wide")
            for mh in range(2):
                for kh in range(2):
                    nc.tensor.matmul(
                        ps[:, R * mh: R * mh + R],
                        w1t[:, R * kh + H * mh: R * kh + H * mh + H],
                        sp_t[:, R * kh: R * kh + R],
                        start=(kh == 0), stop=False,
                    )
                for kh in range(2):
                    nc.tensor.matmul(
                        ps[:, R * mh: R * mh + R],
                        w2t[:, R * kh + H * mh: R * kh + H * mh + H],
                        tp_t[:, R * kh: R * kh + R],
                        start=False, stop=(kh == 1),
                    )
            nc.vector.tensor_add(a_t[:, :], ps[:, :], x1[:, :])
            at_t = keep.tile([H, 2 * R], F32R, tag=f"at{b}")
            _transpose_256(nc, ptr, at_t, a_t, ident, alt=b % 2)
            s0_t = keep.tile([H, 2 * R], F32R, tag=f"s0_{b}")
            _mm256_wide(nc, pmm, s0_t, a_t, a_t, alt=b % 2)  # G
            s1_t = keep.tile([H, 2 * R], F32R, tag=f"s1_{b}")
            v_t = keep.tile([H, 2 * LA], F32R, tag=f"v{b}")
            nc.vector.tensor_copy(v_t[:, :], seed[:, :].bitcast(F32))
            ats.append(at_t); s0s.append(s0_t); s1s.append(s1_t); vs.append(v_t)

        # ---- phase C: chain G2..G16, step-major so the PE never stalls ----
        for b in range(bpc):                               # G2 = (G^2)*2^-21
            _mm256_wide(nc, pmm, s1s[b], s0s[b], s0s[b], scale=G2_SCALE,
                        alt=b % 2)
        for b in range(bpc):                               # G4
            _mm256_wide(nc, pmm, s0s[b], s1s[b], s1s[b], alt=b % 2)
        for b in range(bpc):                               # G8
            _mm256_wide(nc, pmm, s1s[b], s0s[b], s0s[b], alt=b % 2)
        for b in range(bpc):                               # G16 -> hs = s0s
            _mm256_wide(nc, pmm, s0s[b], s1s[b], s1s[b], alt=b % 2)
        hs = s0s

        # ---- phase R: rungs, the 4 packs' NS chains interleaved ----
        a_c, b_c, c_c = MUO
        for r in range(n_rungs):
            mbds, cts, yts = [], [], []
            for p in range(n_pack):
                mbd = sone.tile([H, H], F32, tag=f"mbd{p}")
                nc.any.memset(mbd[:, :], 0.0)
                mbds.append(mbd)
            for b in range(bpc):
                p, sl = b // GRP, (b % GRP) * 32
                yt_ps = psm.tile([LA, R], F32, tag="sm")
                for kh in range(2):
                    nc.tensor.matmul(
                        yt_ps[:, :],
                        vs[b][:, LA * kh: LA * kh + LA],
                        hs[b][:, R * kh: R * kh + R],
                        start=(kh == 0), stop=(kh == 1),
                    )
                yt_t = sone.tile([LA, R], F32, tag=f"ytt{b}")
                if b % 2 == 0:
                    nc.vector.tensor_copy(yt_t[:, :], yt_ps[:, :])
                else:
                    nc.scalar.mul(yt_t[:, :], yt_ps[:, :], 1.0)
                y_t = spool.tile([H, 2 * LA], F32, tag="yy")
                tr_ps = ptr.tile([H, 2 * LA], F32, tag="tr")
                for hh in range(2):
                    nc.tensor.transpose(
                        tr_ps[:, LA * hh: LA * hh + LA],
                        yt_t[:, H * hh: H * hh + H],
                        ident[:LA, :LA],
                    )
                nc.vector.tensor_copy(y_t[:, :], tr_ps[:, :])
                # gram into the pack's block-diag tile
                m_ps = psb.tile([H, H], F32, tag="smb")
                for kh in range(2):
                    nc.tensor.matmul(
                        m_ps[sl:sl + LA, sl:sl + LA],
                        y_t[:, LA * kh: LA * kh + LA],
                        y_t[:, LA * kh: LA * kh + LA],
                        start=(kh == 0), stop=(kh == 1),
                        tile_position=(0, sl),
                    )
                nc.vector.tensor_copy(
                    mbds[p][sl:sl + LA, sl:sl + LA], m_ps[sl:sl + LA, sl:sl + LA]
                )
                yts.append(yt_t)

            # trace normalization, p-interleaved
            mns, rrvs = [], []
            for p in range(n_pack):
                masked = spool.tile([H, H], F32, tag="masked")
                nc.vector.tensor_mul(masked[:, :], mbds[p][:, :], ident[:, :])
                dr_ps = psb.tile([1, H], F32, tag="smb")
                nc.tensor.matmul(dr_ps[:, :], onescol[:, :], masked[:, :],
                                 start=True, stop=True)
                drow = spool.tile([1, H], F32, tag="drow")
                nc.vector.tensor_copy(drow[:, :], dr_ps[:, :])
                dg_ps = psb.tile([H, 1], F32, tag="smb")
                nc.tensor.transpose(dg_ps[:, :], drow[:, :], ident[:1, :1])
                diag = spool.tile([H, 1], F32, tag="diag")
                nc.vector.tensor_copy(diag[:, :], dg_ps[:, :])
                tr_ps = psb.tile([H, 1], F32, tag="smb")
                nc.tensor.matmul(tr_ps[:, :], blocktr[:, :], diag[:, :],
                                 start=True, stop=True)
                tre = spool.tile([H, 1], F32, tag="tre")
                nc.vector.tensor_scalar_add(tre[:, :], tr_ps[:, :], 1e-30)
                itv = spool.tile([H, 1], F32, tag="itv")
                nc.vector.reciprocal(itv[:, :], tre[:, :])
                sq = spool.tile([H, 1], F32, tag="sq")
                nc.scalar.activation(
                    sq[:, :], tre[:, :], mybir.ActivationFunctionType.Sqrt,
                )
                rrv = sone.tile([H, 1], F32, tag=f"rrv{p}")
                nc.vector.reciprocal(rrv[:, :], sq[:, :])
                mn = sone.tile([H, H], BF16, tag=f"mn{p}")
                nc.vector.tensor_scalar_mul(mn[:, :], mbds[p][:, :], itv[:, :])
                mns.append(mn); rrvs.append(rrv)

            # quintic NS, steps interleaved across the 4 packs
            mcurs = list(mns)
            cts = [sone.tile([H, H], BF16, tag=f"ct{p}", name=f"ct{p}")
                   for p in range(n_pack)]
            for st in range(ns_steps):
                m2_pss, csts = [], []
                for p in range(n_pack):
                    m2_ps = psb.tile([H, H], F32, tag="smb")
                    nc.tensor.matmul(m2_ps[:, :], mcurs[p][:, :], mcurs[p][:, :],
                                     start=True, stop=True)
                    m2_pss.append(m2_ps)
                for p in range(n_pack):
                    cst = sone.tile([H, H], BF16, tag=f"cst{p}")
                    nc.vector.tensor_scalar_mul(
                        cst[:, :], mcurs[p][:, :].bitcast(BF16), b_c)
                    nc.vector.tensor_add(cst[:, :], cst[:, :], eyema_bf[:, :])
                    m2s = spool.tile([H, H], BF16, tag="m2s")
                    nc.scalar.mul(m2s[:, :], m2_pss[p][:, :], c_c)
                    nc.vector.tensor_add(cst[:, :], cst[:, :], m2s[:, :])
                    csts.append(cst)
                if st < ns_steps - 1:
                    cms = []
                    for p in range(n_pack):
                        cm_ps = psb.tile([H, H], F32, tag="smb")
                        nc.tensor.matmul(cm_ps[:, :], csts[p][:, :], mcurs[p][:, :],
                                         start=True, stop=True)
                        cm = spool.tile([H, H], BF16, tag=f"cm{p}")
                        nc.vector.tensor_copy(cm[:, :], cm_ps[:, :])
                        cms.append(cm)
                    for p in range(n_pack):
                        mn2_ps = psb.tile([H, H], F32, tag="smb")
                        nc.tensor.matmul(mn2_ps[:, :], cms[p][:, :], csts[p][:, :],
                                         start=True, stop=True)
                        mnew = sone.tile([H, H], BF16, tag=f"mnew{p}_{st}")
                        nc.vector.tensor_copy(mnew[:, :], mn2_ps[:, :])
                        mcurs[p] = mnew
                for p in range(n_pack):
                    if st == 0:
                        nc.vector.tensor_copy(cts[p][:, :], csts[p][:, :])
                    else:
                        ct_ps = psb.tile([H, H], F32, tag="smb")
                        nc.tensor.matmul(ct_ps[:, :], cts[p][:, :], csts[p][:, :],
                                         start=True, stop=True)
                        nc.vector.tensor_copy(cts[p][:, :], ct_ps[:, :])
            for p in range(n_pack):
                nc.vector.tensor_scalar_mul(cts[p][:, :], cts[p][:, :], rrvs[p][:, :])

            # extract each pack's diag blocks to partition base 0 via an
            # identity matmul (operands share base sl; out lands at base 0)
            ct0s = []
            for p in range(n_pack):
                for kk in range(GRP):
                    sl = kk * 32
                    c0_ps = psb.tile([LA, LA], F32, tag="smb")
                    nc.tensor.matmul(
                        c0_ps[:, :],
                        ident_bf[sl:sl + LA, sl:sl + LA],
                        cts[p][sl:sl + LA, sl:sl + LA],
                        start=True, stop=True,
                        tile_position=(sl, 0),
                    )
                    ct0 = sone.tile([LA, LA], F32, tag=f"ct0_{p}_{kk}",
                                    name=f"ct0_{p}_{kk}")
                    nc.vector.tensor_copy(ct0[:, :], c0_ps[:, :])
                    ct0s.append(ct0)
            # apply: V_b = Y_b @ Ct0_b (all operands at base 0)
            for b in range(bpc):
                vp = ptr.tile([H, 2 * LA], F32, tag="tr")
                for hh in range(2):
                    nc.tensor.matmul(
                        vp[:, LA * hh: LA * hh + LA],
                        yts[b][:, H * hh: H * hh + H],
                        ct0s[b][:, :],
                        start=True, stop=True,
                    )
                nc.vector.tensor_copy(vs[b][:, :], vp[:, :])

        # ---- phase O: outputs Gh, Bh, Vt, Wpt (pipelines across b) ----
        for b in range(bpc):
            zt_ps = psm.tile([LA, R], F32, tag="sm")
            for kh in range(2):
                nc.tensor.matmul(
                    zt_ps[:, :],
                    vs[b][:, LA * kh: LA * kh + LA],
                    hs[b][:, R * kh: R * kh + R],
                    start=(kh == 0), stop=(kh == 1),
                )
            zt_t = spool.tile([LA, R], F32, tag="ztt")
            if b % 2 == 0:
                nc.vector.tensor_copy(zt_t[:, :], zt_ps[:, :])
            else:
                nc.scalar.mul(zt_t[:, :], zt_ps[:, :], 1.0)
            z_t = spool.tile([H, 2 * LA], F32, tag="zz")
            ztr_ps = ptr.tile([H, 2 * LA], F32, tag="tr")
            for hh in range(2):
                nc.tensor.transpose(
                    ztr_ps[:, LA * hh: LA * hh + LA],
                    zt_t[:, H * hh: H * hh + H],
                    ident[:LA, :LA],
                )
            nc.vector.tensor_copy(z_t[:, :], ztr_ps[:, :])
            ghbh_t = spool.tile([LA, 2 * LA], F32, tag="ghbh")
            gb_ps = psb.tile([LA, 2 * LA], F32, tag="smb")
            for kh in range(2):
                nc.tensor.matmul(
                    gb_ps[:, 0:LA],
                    z_t[:, LA * kh: LA * kh + LA],
                    vs[b][:, LA * kh: LA * kh + LA].bitcast(F32),
                    start=(kh == 0), stop=(kh == 1),
                )
            for kh in range(2):
                nc.tensor.matmul(
                    gb_ps[:, LA:2 * LA],
                    vs[b][:, LA * kh: LA * kh + LA].bitcast(F32),
                    vs[b][:, LA * kh: LA * kh + LA].bitcast(F32),
                    start=(kh == 0), stop=(kh == 1),
                )
            nc.vector.tensor_copy(ghbh_t[:, :], gb_ps[:, :])
            nc.sync.dma_start(out=ghbh_out[b], in_=ghbh_t[:, :])

            vw_t = spool.tile([LA, 2 * R], F32R, tag="vw")
            vtr_ps = psm.tile([LA, R], F32, tag="sm")
            for hh in range(2):
                nc.tensor.transpose(
                    vtr_ps[:, H * hh: H * hh + H],
                    vs[b][:, LA * hh: LA * hh + LA].bitcast(F32),
                    ident[:, :],
                )
            nc.vector.tensor_copy(vw_t[:, 0:R], vtr_ps[:, :])
            wpt_ps = psm.tile([LA, R], F32, tag="sm")
            for kh in range(2):
                nc.tensor.matmul(
                    wpt_ps[:, :],
                    vs[b][:, LA * kh: LA * kh + LA],
                    ats[b][:, R * kh: R * kh + R],
                    start=(kh == 0), stop=(kh == 1),
                )
            nc.vector.tensor_copy(vw_t[:, R:2 * R], wpt_ps[:, :])
            nc.sync.dma_start(out=vtwpt_out[b], in_=vw_t[:, :])
    nc.compile()
    return nc


def build_k2(bpc=BPC, stage=3):
    nc = bacc.Bacc("TRN2", target_bir_lowering=False)
    sp_d = nc.dram_tensor("sp", [bpc, R, R], F32, kind="ExternalInput")
    # packed per-matrix smalls: [vt | wpt | ms] = [32, 256+256+32]
    vwm_d = nc.dram_tensor("vwm", [bpc, LA, 2 * R + LA], F32R,
                           kind="ExternalInput")
    ident_d = nc.dram_tensor("ident", [H, H], F32, kind="ExternalInput")
    invc2_d = nc.dram_tensor("invc2", [1, 1022], F32, kind="ExternalInput")
    tpn_out = nc.dram_tensor("tpn_out", [bpc, R, R], F32, kind="ExternalOutput")
    spn_out = nc.dram_tensor("spn_out", [bpc, R, R], F32, kind="ExternalOutput")
    scr1 = nc.dram_tensor("scr1", [bpc, SHEAR_N], BF16)
    scrq = nc.dram_tensor("scrq", [bpc, QBUF_N], F32)
    npair = bpc // 2

    with tile.TileContext(nc) as tc, ExitStack() as ctx:
        cpool = ctx.enter_context(tc.tile_pool(name="consts", bufs=1))
        inpool = ctx.enter_context(tc.tile_pool(name="inp", bufs=2))
        tpool = ctx.enter_context(tc.tile_pool(name="trans", bufs=2))
        keep = ctx.enter_context(tc.tile_pool(name="keep", bufs=1))
        spool = ctx.enter_context(tc.tile_pool(name="small", bufs=3))
        pmm = ctx.enter_context(tc.tile_pool(name="pmm", bufs=2, space="PSUM"))
        ptr = ctx.enter_context(tc.tile_pool(name="ptr", bufs=2, space="PSUM"))
        psm = ctx.enter_context(tc.tile_pool(name="psm", bufs=2, space="PSUM"))

        ident = cpool.tile([H, H], F32)
        nc.sync.dma_start(out=ident[:, :], in_=ident_d[:, :])
        invc2 = cpool.tile([1, 1022], F32)
        nc.sync.dma_start(out=invc2[:, :], in_=invc2_d[:, :])
        ones = cpool.tile([H, 1], BF16)
        nc.any.memset(ones[:, :], 1.0)
        onesr = cpool.tile([1, H], BF16)
        nc.any.memset(onesr[:, :], 1.0)

        # per-pair zero-padded staging tiles (pads stay zero) + results
        m2zs, tpns, sps = [], [], []
        for q in range(npair):
            m2z = keep.tile([H, 2048], BF16, tag=f"m2z{q}", name=f"m2z{q}")
            nc.any.memset(m2z[:, :], 0.0)
            m2zs.append(m2z)
            tpn = keep.tile([H, 4 * R], F32, tag=f"tpn{q}", name=f"tpn{q}")
            tpns.append(tpn)
        # one DMA zeroes the shear-gap head [0,255) of every matrix slot
        nc.sync.dma_start(out=scr1[:, 0:255], in_=m2zs[0][0:bpc, 256:511])

        # ---- phase A: tpn + shear writes (pipelines across pairs) ----
        for q in range(npair):
            b0 = 2 * q
            sp2 = keep.tile([H, 4 * R], F32, tag=f"sp{q}", name=f"sp{q}")
            nc.sync.dma_start(
                out=sp2[:, :].rearrange("p (b hh c) -> p b hh c",
                                        b=2, hh=2, c=R),
                in_=sp_d[b0:b0 + 2].rearrange("b (hh p) c -> p b hh c", p=H),
            )
            sps.append(sp2)
            vwm2 = inpool.tile([LA, 2 * (2 * R + LA)], F32R, tag="vwm")
            nc.gpsimd.dma_start(
                out=vwm2[:, :].rearrange("p (b c) -> p b c", b=2),
                in_=vwm_d[b0:b0 + 2].rearrange("b p c -> p b c"),
            )
            for m in range(2):
                b = b0 + m
                W = 2 * R + LA
                sp_t = sp2[:, 2 * R * m: 2 * R * m + 2 * R]
                vt_t = vwm2[:, W * m: W * m + R]
                wpt_t = vwm2[:, W * m + R: W * m + 2 * R]
                ms_t = vwm2[:, W * m + 2 * R: W * m + 2 * R + LA]

                # Sp^T via PE transposes (copies split DVE/Act)
                spt_t = tpool.tile([H, 2 * R], F32, tag="spt")
                for i in range(2):
                    for j in range(2):
                        ps = ptr.tile([H, H], F32, tag="tr")
                        nc.tensor.transpose(
                            ps[:, :],
                            sp_t[:, R * j + H * i: R * j + H * i + H],
                            ident[:, :],
                        )
                        if (i + j) % 2 == 0:
                            nc.vector.tensor_copy(
                                spt_t[:, R * i + H * j: R * i + H * j + H],
                                ps[:, :])
                        else:
                            nc.scalar.mul(
                                spt_t[:, R * i + H * j: R * i + H * j + H],
                                ps[:, :], 1.0)

                # P1 = Ms @ Wpt ; P2 = Ms @ Vt   (Ms symmetric)
                p1_ps = psm.tile([LA, R], F32, tag="sm")
                nc.tensor.matmul(p1_ps[:, :], ms_t, wpt_t, start=True, stop=True)
                p1_t = spool.tile([LA, R], F32R, tag="p1")
                nc.vector.tensor_copy(p1_t[:, :], p1_ps[:, :])
                p2_ps = psm.tile([LA, R], F32, tag="sm")
                nc.tensor.matmul(p2_ps[:, :], ms_t, vt_t, start=True, stop=True)
                p2_t = spool.tile([LA, R], F32R, tag="p2")
                nc.vector.tensor_copy(p2_t[:, :], p2_ps[:, :])

                # TpnewT = V P1 ; Tpnew = W' P2  (fp32r, 256-wide)
                tpnT = tpool.tile([H, 2 * R], F32, tag="tpnT")
                tpn = tpns[q]
                for hh in range(2):
                    ps = pmm.tile([H, 2 * R], F32, tag="wide")
                    nc.tensor.matmul(
                        ps[:, 0:R],
                        vt_t[:, H * hh: H * hh + H],
                        p1_t[:, :],
                        start=True, stop=True,
                    )
                    nc.vector.tensor_copy(tpnT[:, R * hh: R * hh + R], ps[:, 0:R])
                    ps2 = pmm.tile([H, 2 * R], F32, tag="wide")
                    nc.tensor.matmul(
                        ps2[:, 0:R],
                        wpt_t[:, H * hh: H * hh + H],
                        p2_t[:, :],
                        start=True, stop=True,
                    )
                    nc.scalar.mul(
                        tpn[:, 2 * R * m + R * hh: 2 * R * m + R * hh + R],
                        ps2[:, 0:R], 1.0)
                if stage >= 2:
                    # M2T = 2*TpnewT - SpT into this pair's bf16 staging
                    m2z = m2zs[q]
                    for hh in range(2):
                        o = 1024 * m + 512 * hh
                        nc.vector.tensor_scalar_mul(
                            m2z[:, o: o + R],
                            tpnT[:, R * hh: R * hh + R], 2.0,
                        )
                        nc.vector.tensor_tensor(
                            out=m2z[:, o: o + R],
                            in0=m2z[:, o: o + R],
                            in1=spt_t[:, R * hh: R * hh + R],
                            op=mybir.AluOpType.subtract,
                        )
            # Tpnew out, one DMA per pair (natural layout)
            nc.sync.dma_start(
                out=tpn_out[b0:b0 + 2].rearrange("b (hh p) c -> p b hh c", p=H),
                in_=tpns[q][:, :].rearrange("p (b hh c) -> p b hh c",
                                            b=2, hh=2, c=R),
            )
            if stage < 2:
                continue
            # shear-write the pair, one DMA per row-half (3D APs balance)
            for hh in range(2):
                o = 255 + 511 * H * hh
                nc.gpsimd.dma_start(
                    out=scr1[b0:b0 + 2, o: o + 511 * H].rearrange(
                        "b (p f) -> p b f", p=H),
                    in_=m2zs[q][:, :].rearrange(
                        "p (b hh x) -> p b hh x", b=2, hh=2)[:, :, hh, 0:511],
                )

        # ---- phase B: diagonal sums -> periodic Q buffer (per pair) ----
        if stage >= 2:
            for q in range(npair):
                b0 = 2 * q
                shm = tpool.tile([H, 2044], BF16, tag="shm")
                for hh in range(2):
                    nc.gpsimd.dma_start(
                        out=shm[:, :].rearrange("p (b hh f) -> p b hh f",
                                                b=2, hh=2)[:, :, hh],
                        in_=scr1[b0:b0 + 2, 512 * H * hh: 512 * H * hh
                                 + 512 * H].rearrange(
                            "b (p f) -> p b f", p=H)[:, :, 0:511],
                    )
                avg = spool.tile([1, 1022], BF16, tag="avg")
                for m in range(2):
                    sums_ps = psm.tile([1, 511], F32, tag="sm3")
                    for hh in range(2):
                        nc.tensor.matmul(
                            sums_ps[:, :], ones[:, :],
                            shm[:, 1022 * m + 511 * hh: 1022 * m + 511 * hh + 511],
                            start=(hh == 0), stop=(hh == 1))
                    nc.vector.tensor_mul(avg[:, 511 * m: 511 * m + 511],
                                         sums_ps[:1, :],
                                         invc2[:, 511 * m: 511 * m + 511])
                avgb = spool.tile([H, 1022], F32, tag="avgb")
                for m in range(2):
                    avgb_ps = pmm.tile([H, 2 * R], F32, tag="wide")
                    nc.tensor.matmul(avgb_ps[:, 0:511], onesr[:, :],
                                     avg[:, 511 * m: 511 * m + 511],
                                     start=True, stop=True)
                    if m == 0:
                        nc.vector.tensor_copy(avgb[:, 0:511], avgb_ps[:, 0:511])
                    else:
                        nc.scalar.mul(avgb[:, 511:1022], avgb_ps[:, 0:511], 1.0)
                # periodic Q: 128 rows + 1 wrap row of avg, per pair
                nc.sync.dma_start(
                    out=scrq[b0:b0 + 2, 0: 511 * H].rearrange(
                        "b (p f) -> p b f", p=H),
                    in_=avgb[:, :].rearrange("p (b f) -> p b f", b=2),
                )
                nc.sync.dma_start(
                    out=scrq[b0:b0 + 2, 511 * H: 511 * (H + 1)],
                    in_=avgb[0:1, :],
                )

        # ---- phase C: Toeplitz + Spnew (per pair) ----
        if stage >= 3:
            for q in range(npair):
                b0 = 2 * q
                # toepT[p, f] = avg[f - p + 255] via mod-511 reads, 1 DMA/pair
                # per matrix: cols [0,256) = rows 128..255 (base 127),
                # cols [256,512) = rows 0..127 (base 255)
                ttT = tpool.tile([H, 4 * R], F32, tag="ttT")
                for hh, base in ((1, 255), (0, 127)):
                    nc.gpsimd.dma_start(
                        out=ttT[:, :].rearrange("p (b hh f) -> p b hh f",
                                                b=2, hh=2, f=R)[:, :, hh],
                        in_=scrq[b0:b0 + 2, base: base + 510 * H].rearrange(
                            "b (p f) -> p b f", p=H, f=510)[:, :, 0:R],
                    )
                spn2 = tpool.tile([H, 4 * R], F32, tag="spn2")
                for m in range(2):
                    b = b0 + m
                    sp_t = sps[q][:, 2 * R * m: 2 * R * m + 2 * R]
                    # toep natural = transpose(toepT) (halves swapped in ttT)
                    ttN = tpool.tile([H, 2 * R], F32, tag="ttN")
                    for i in range(2):
                        for j in range(2):
                            ps = ptr.tile([H, H], F32, tag="tr")
                            nc.tensor.transpose(
                                ps[:, :],
                                ttT[:, 2 * R * m + R * (1 - j) + H * i:
                                    2 * R * m + R * (1 - j) + H * i + H],
                                ident[:, :],
                            )
                            if (i + j) % 2 == 0:
                                nc.vector.tensor_copy(
                                    ttN[:, R * i + H * j: R * i + H * j + H],
                                    ps[:, :])
                            else:
                                nc.scalar.mul(
                                    ttN[:, R * i + H * j: R * i + H * j + H],
                                    ps[:, :], 1.0)
                    # Spnew = Sp - Tpnew + toep_nat
                    o = 2 * R * m
                    nc.vector.tensor_tensor(
                        out=spn2[:, o: o + 2 * R], in0=sp_t,
                        in1=tpns[q][:, o: o + 2 * R],
                        op=mybir.AluOpType.subtract,
                    )
                    nc.vector.tensor_add(
                        spn2[:, o: o + 2 * R], spn2[:, o: o + 2 * R], ttN[:, :])
                nc.sync.dma_start(
                    out=spn_out[b0:b0 + 2].rearrange(
                        "b (hh p) c -> p b hh c", p=H),
                    in_=spn2[:, :].rearrange("p (b hh c) -> p b hh c",
                                             b=2, hh=2, c=R),
                )
    nc.compile()
    return nc


def _transpose_256f(nc, ptr_pool, out_t, in_t, ident):
    """out = in^T for a 256x256 [128,512] fp32 tile (4 PE transposes)."""
    for i in range(2):
        for j in range(2):
            ps = ptr_pool.tile([H, H], F32, tag="tr")
            nc.tensor.transpose(
                ps[:, :], in_t[:, R * j + H * i: R * j + H * i + H], ident[:, :]
            )
            nc.vector.tensor_copy(out_t[:, R * i + H * j: R * i + H * j + H], ps[:, :])


def _host_consts():
    ident = np.eye(H, dtype=np.float32)
    eyema = (MUO[0] * np.eye(H)).astype(np.float32)
    blocktr = np.zeros((H, H), np.float32)
    for g in range(4):
        blocktr[g * 32: g * 32 + LA, g * 32: g * 32 + 32] = 1.0
    i = np.arange(R, dtype=np.float32)[:, None]
    j = np.arange(LA, dtype=np.float32)[None, :]
    v0 = np.cos(0.37 * (i + 1) * (j + 1) + 0.11 * i).astype(np.float32)
    seed = np.concatenate([v0[0:H, :], v0[H:R, :]], axis=1)  # [128, 32]
    counts = (R - np.abs(np.arange(511) - 255)).astype(np.float32)
    invc = (1.0 / counts)[None, :].astype(np.float32)
    return ident, eyema, blocktr, seed, invc


def _host_bridge(gh, bh, Kv):
    """Robust whitened generalized eig; returns Ms = Z10 Z10^T per matrix."""
    n = gh.shape[0]
    ms = np.zeros((n, LA, LA), np.float32)
    for b in range(n):
        Gs = 0.5 * (gh[b] + gh[b].T).astype(np.float64)
        Bs = 0.5 * (bh[b] + bh[b].T).astype(np.float64)
        lb, Ub = np.linalg.eigh(Bs)
        lmax = max(float(lb.max()), 0.0)
        keep = lb > lmax * 1e-7 if lmax > 0 else lb > -1.0
        if not np.any(keep):
            continue
        Wh = Ub[:, keep] / np.sqrt(np.maximum(lb[keep], 1e-300))[None, :]
        Gw = Wh.T @ Gs @ Wh
        d, Qw = np.linalg.eigh(Gw)
        Z = Wh @ Qw[:, ::-1][:, :Kv]
        ms[b] = (Z @ Z.T).astype(np.float32)
    return ms


def _host_fallback(T, Tp, Sp, w1, w2, w3, w4, Kv):
    """Numpy implementation (used only if the device path fails)."""
    f32 = np.float32
    A = (np.einsum('rk,bkc->brc', w1, Sp) + np.einsum('rk,bkc->brc', w2, Tp)
         + w4[None] * Tp + w3[None] * T).astype(f32)
    G = np.einsum('brc,brd->bcd', A, A).astype(f32)
    d, q = np.linalg.eigh(G.astype(np.float64))
    Vs = q[:, :, ::-1][:, :, :Kv]
    AV = np.einsum('brc,bcl->brl', A.astype(np.float64), Vs)
    Tpnew = np.einsum('brl,bcl->brc', AV, Vs).astype(f32)
    m, n = R, R
    D = m + n - 1
    ii = np.arange(m)[:, None]; jj = np.arange(n)[None, :]
    dd = jj - ii + (m - 1)
    M2 = (2.0 * Tpnew - Sp).astype(f32)
    Z = np.zeros((M2.shape[0], m, D), f32)
    Z[:, ii, dd] = M2
    sums = Z.sum(axis=1)
    counts = (m - np.abs(np.arange(D) - (m - 1))).astype(f32)
    avg = sums / counts
    Spnew = (Sp - Tpnew + avg[:, dd]).astype(f32)
    return (T, Tpnew, Spnew)


_K1 = None
_K2 = None


def _get_kernels():
    global _K1, _K2
    if _K1 is None:
        _K1 = build_k1()
    if _K2 is None:
        _K2 = build_k2()
    return _K1, _K2


def _run_k2(Sp, vt_all, wpt_all, ms_all, nc2=None):
    ident, eyema, blocktr, seed, invc = _host_consts()
    if nc2 is None:
        nc2 = build_k2()
    vwm = np.concatenate([vt_all, wpt_all, ms_all], axis=2)  # [B, 16, 528]
    vwm = np.ascontiguousarray(vwm, dtype=np.float32)
    in_maps = []
    for c in range(N_CORES):
        sl = slice(c * BPC, (c + 1) * BPC)
        in_maps.append({
            "sp": Sp[sl], "vwm": vwm[sl], "ident": ident,
            "invc2": np.concatenate([invc, invc], axis=1),
        })
    r2 = run_bass_kernel_spmd(nc2, in_maps, list(range(N_CORES)))
    LAST_EXEC_NS[1] = r2.exec_time_ns
    res2 = r2.results
    Tpnew = np.concatenate([res2[c]["tpn_out"] for c in range(N_CORES)], axis=0)
    Spnew = np.concatenate([res2[c]["spn_out"] for c in range(N_CORES)], axis=0)
    return Tpnew, Spnew


def _kernel_device(T, Tp, Sp, w1, w2, w3, w4, Kv):
    ident, eyema, blocktr, seed, invc = _host_consts()
    w1t = np.ascontiguousarray(w1.T)
    w2t = np.ascontiguousarray(w2.T)
    nc1, nc2 = _get_kernels()
    in_maps1 = []
    for c in range(N_CORES):
        sl = slice(c * BPC, (c + 1) * BPC)
        in_maps1.append({
            "sp": Sp[sl], "tp": Tp[sl], "t": T[sl],
            "w1t": w1t, "w2t": w2t, "w3": w3, "w4": w4,
            "ident": ident, "eyema": eyema, "blocktr": blocktr, "seed": seed,
        })
    r1 = run_bass_kernel_spmd(nc1, in_maps1, list(range(N_CORES)))
    LAST_EXEC_NS[0] = r1.exec_time_ns
    res1 = r1.results
    ghbh = np.concatenate([res1[c]["ghbh_out"] for c in range(N_CORES)], axis=0)
    vtwpt = np.concatenate([res1[c]["vtwpt_out"] for c in range(N_CORES)], axis=0)
    gh, bh = ghbh[:, :, 0:LA], ghbh[:, :, LA:2 * LA]
    vt_all, wpt_all = vtwpt[:, :, 0:R], vtwpt[:, :, R:2 * R]
    ms_all = _host_bridge(gh, bh, Kv)
    Tpnew, Spnew = _run_k2(Sp, vt_all, wpt_all, ms_all, nc2=nc2)
    return (T, Tpnew, Spnew)


def _kernel_hybrid(T, Tp, Sp, w1, w2, w3, w4, Kv):
    """Host eigensolve for the subspace + device K2 for apply/averaging."""
    f32 = np.float32
    A = (np.einsum('rk,bkc->brc', w1, Sp) + np.einsum('rk,bkc->brc', w2, Tp)
         + w4[None] * Tp + w3[None] * T).astype(f32)
    G = np.einsum('brc,brd->bcd', A, A)
    d, q = np.linalg.eigh(G.astype(np.float64))
    Vs = q[:, :, ::-1][:, :, :Kv]                       # [B, 256, K]
    vt_all = np.zeros((B_FULL, LA, R), f32)
    vt_all[:, :Kv, :] = Vs.transpose(0, 2, 1).astype(f32)
    AV = np.einsum('brc,bcl->brl', A.astype(np.float64), Vs)
    wpt_all = np.zeros((B_FULL, LA, R), f32)
    wpt_all[:, :Kv, :] = AV.transpose(0, 2, 1).astype(f32)
    ms_all = np.zeros((B_FULL, LA, LA), f32)
    ms_all[:, :Kv, :Kv] = np.eye(Kv, dtype=f32)[None]
    Tpnew, Spnew = _run_k2(Sp, vt_all, wpt_all, ms_all)
    return (T, Tpnew, Spnew)


def kernel(T, Tp, Sp, w1, w2, w3, w4, K):
    T = np.ascontiguousarray(np.asarray(T, dtype=np.float32))
    Tp = np.ascontiguousarray(np.asarray(Tp, dtype=np.float32))
    Sp = np.ascontiguousarray(np.asarray(Sp, dtype=np.float32))
    w1 = np.asarray(w1, dtype=np.float32); w2 = np.asarray(w2, dtype=np.float32)
    w3 = np.asarray(w3, dtype=np.float32); w4 = np.asarray(w4, dtype=np.float32)
    Kv = int(np.asarray(K))
    try:
        return _kernel_device(T, Tp, Sp, w1, w2, w3, w4, Kv)
    except Exception:
        import traceback
        traceback.print_exc()
        print("K1 device path failed; host eigensolve + device K2")
    try:
        return _kernel_hybrid(T, Tp, Sp, w1, w2, w3, w4, Kv)
    except Exception:
        import traceback
        traceback.print_exc()
        print("hybrid path failed; full host fallback")
        return _host_fallback(T, Tp, Sp, w1, w2, w3, w4, Kv)


LAST_EXEC_NS = [None, None]


# revision 50
# speedup vs baseline: 2.3146x; 1.1050x over previous
"""Cadzow update (batched rank-K truncation + Toeplitz averaging) on 8 trn2 cores.

Data-parallel over the batch of 128 matrices (16 per core). Per matrix:
  A = w1@Sp + w2@Tp + w4*Tp + w3*T
  rank-K via oversampled subspace iteration + host Rayleigh-Ritz:
    K1 (device): G = A^T A; chain G2=(G^2*2^-21), G4, G8, G16 (fp32r matmuls);
      3 rungs V <- orth(G16 V) with a quintic Newton-Schulz Gram conditioner
      (4 matrices packed per 128x128 block-diag tile); ships per matrix
      Gh = V^T G16 V, Bh = V^T V (16x16), Vt = V^T and Wpt = (A V)^T.
    host: robust whitened generalized eig of (Gh, Bh); top-K selector
      Ms = Z10 Z10^T (16x16).
    K2 (device): Tpnew = Wpt^T Ms Vt (both orientations from the small
      factors); Spnew = Sp - Tpnew + avgdiag(2 Tpnew - Sp) where the
      diagonal averaging runs via a shear-DMA layout (diag sums by
      ones-matmul) and the Toeplitz broadcast is read back from a
      mod-511 periodic DRAM buffer with all-positive strides.

All big matmuls run as fp32r (~4x PE throughput at >=256-wide outputs);
the 16x16 Grams / Newton-Schulz stay fp32. Outputs are written in natural
layout (no 4-byte-granular transposed DMA anywhere).
"""
import os
import numpy as np
from contextlib import ExitStack

# The axon ntff profile hook (antenv.axon_hooks) is absent in this image;
# a set BASS_TRACE would crash run_bass_kernel_spmd, so clear it.
os.environ.pop("BASS_TRACE", None)

import concourse.bass as bass
import concourse.bacc as bacc
import concourse.mybir as mybir
from concourse import tile
from concourse.bass_utils import run_bass_kernel_spmd

F32 = mybir.dt.float32
F32R = mybir.dt.float32r
BF16 = mybir.dt.bfloat16
N_CORES = 8
B_FULL = 128
BPC = B_FULL // N_CORES          # 16 matrices per core
R = 256
LA = 32                          # subspace dim (oversampled, 4x32 pack)
H = 128                          # partitions
GRP = 4                          # matrices packed per 128x128 Gram tile
N_RUNGS = 1
NS_STEPS = 3
MUO = (3.4445, -4.7750, 2.0315)  # quintic NS coefficients
G2_SCALE = 2.0 ** -21

SHEAR_N = 512 * 257              # shear scratch elems per matrix
QBUF_N = 511 * 129               # periodic Toeplitz buffer elems per matrix


def _halfslc(hh, w=R):
    return slice(w * hh, w * hh + w)


def _load_256(nc, dst, src_b):
    """DRAM (256, X) -> SBUF [128, 2X] (row halves side by side)."""
    X = src_b.shape[-1]
    nc.sync.dma_start(out=dst[:, 0:X], in_=src_b[0:H, :])
    nc.sync.dma_start(out=dst[:, X:2 * X], in_=src_b[H:2 * H, :])


def _mm256_wide(nc, psum_pool, out_t, lhs_t, rhs_t, scale=None, alt=0):
    """out = L^T @ Rhs for 256x256 [128,512]-tiled operands (4 matmuls,
    both output halves in one psum bank, a single copy out)."""
    ps = psum_pool.tile([H, 2 * R], F32, tag="wide")
    for mh in range(2):
        for kh in range(2):
            nc.tensor.matmul(
                ps[:, R * mh: R * mh + R],
                lhs_t[:, R * kh + H * mh: R * kh + H * mh + H],
                rhs_t[:, R * kh: R * kh + R],
                start=(kh == 0), stop=(kh == 1),
            )
    if scale is None:
        if alt == 0:
            nc.vector.tensor_copy(out_t[:, :], ps[:, :])
        else:
            nc.scalar.mul(out_t[:, :], ps[:, :], 1.0)
    else:
        if alt == 0:
            nc.vector.tensor_scalar_mul(out_t[:, :], ps[:, :], scale)
        else:
            nc.scalar.mul(out_t[:, :], ps[:, :], scale)


def _transpose_256(nc, ptr_pool, out_t, in_t, ident, alt=0):
    """out = in^T for a 256x256 [128,512] tile (4 PE transposes, 2 copies)."""
    for i in range(2):
        ps = ptr_pool.tile([H, 2 * H], F32, tag="tr")
        for j in range(2):
            nc.tensor.transpose(
                ps[:, H * j: H * j + H],
                in_t[:, R * j + H * i: R * j + H * i + H].bitcast(F32),
                ident[:, :],
            )
        if (i + alt) % 2 == 0:
            nc.vector.tensor_copy(out_t[:, R * i: R * i + R], ps[:, :])
        else:
            nc.scalar.mul(out_t[:, R * i: R * i + R], ps[:, :], 1.0)


def build_k1(bpc=BPC, n_rungs=N_RUNGS, ns_steps=NS_STEPS, fuse_w34=True):
    nc = bacc.Bacc("TRN2", target_bir_lowering=False)
    sp_d = nc.dram_tensor("sp", [bpc, R, R], F32R, kind="ExternalInput")
    tp_d = nc.dram_tensor("tp", [bpc, R, R], F32R, kind="ExternalInput")
    t_d = nc.dram_tensor("t", [bpc, R, R], F32, kind="ExternalInput")
    w1t_d = nc.dram_tensor("w1t", [R, R], F32R, kind="ExternalInput")
    w2t_d = nc.dram_tensor("w2t", [R, R], F32R, kind="ExternalInput")
    w3_d = nc.dram_tensor("w3", [R, R], F32, kind="ExternalInput")
    w4_d = nc.dram_tensor("w4", [R, R], F32, kind="ExternalInput")
    ident_d = nc.dram_tensor("ident", [H, H], F32, kind="ExternalInput")
    eyema_d = nc.dram_tensor("eyema", [H, H], F32, kind="ExternalInput")  # MUO[0]*I
    blocktr_d = nc.dram_tensor("blocktr", [H, H], F32, kind="ExternalInput")
    seed_d = nc.dram_tensor("seed", [H, 2 * LA], F32R, kind="ExternalInput")
    ghbh_out = nc.dram_tensor("ghbh_out", [bpc, LA, 2 * LA], F32,
                              kind="ExternalOutput")
    vtwpt_out = nc.dram_tensor("vtwpt_out", [bpc, LA, 2 * R], F32R,
                               kind="ExternalOutput")

    n_pack = (bpc + GRP - 1) // GRP
    with tile.TileContext(nc) as tc, ExitStack() as ctx:
        cpool = ctx.enter_context(tc.tile_pool(name="consts", bufs=1))
        inpool = ctx.enter_context(tc.tile_pool(name="inp", bufs=2))
        tpool = ctx.enter_context(tc.tile_pool(name="trans", bufs=2))
        keep = ctx.enter_context(tc.tile_pool(name="keep", bufs=1))
        spool = ctx.enter_context(tc.tile_pool(name="small", bufs=2))
        sone = ctx.enter_context(tc.tile_pool(name="sone", bufs=1))
        pmm = ctx.enter_context(tc.tile_pool(name="pmm", bufs=2, space="PSUM"))
        ptr = ctx.enter_context(tc.tile_pool(name="ptr", bufs=2, space="PSUM"))
        psm = ctx.enter_context(tc.tile_pool(name="psm", bufs=2, space="PSUM"))
        psb = ctx.enter_context(tc.tile_pool(name="psb", bufs=2, space="PSUM"))

        w1t = cpool.tile([H, 2 * R], F32R); _load_256(nc, w1t, w1t_d)
        w2t = cpool.tile([H, 2 * R], F32R); _load_256(nc, w2t, w2t_d)
        w3 = cpool.tile([H, 2 * R], F32); _load_256(nc, w3, w3_d)
        w4 = cpool.tile([H, 2 * R], F32); _load_256(nc, w4, w4_d)
        ident = cpool.tile([H, H], F32)
        nc.sync.dma_start(out=ident[:, :], in_=ident_d[:, :])
        eyema = cpool.tile([H, H], F32)
        nc.sync.dma_start(out=eyema[:, :], in_=eyema_d[:, :])
        blocktr = cpool.tile([H, H], F32)
        nc.sync.dma_start(out=blocktr[:, :], in_=blocktr_d[:, :])
        seed = cpool.tile([H, 2 * LA], F32R)
        nc.sync.dma_start(out=seed[:, :], in_=seed_d[:, :])
        onescol = cpool.tile([H, 1], F32)
        nc.any.memset(onescol[:, :], 1.0)
        eyema_bf = cpool.tile([H, H], BF16)
        nc.vector.tensor_copy(eyema_bf[:, :], eyema[:, :])
        ident_bf = cpool.tile([H, H], BF16)
        nc.vector.tensor_copy(ident_bf[:, :], ident[:, :])

        ats, s0s, s1s, vs = [], [], [], []
        # ---- phase L: loads, A, A^T, G (per matrix; pipelines across b) ----
        sp2 = tp2 = t2 = None
        for b in range(bpc):
            if b % 2 == 0:
                # one DMA per tensor loads a PAIR of matrices [128, 1024]
                sp2 = inpool.tile([H, 4 * R], F32R, tag="sp")
                tp2 = inpool.tile([H, 4 * R], F32R, tag="tp")
                t2 = inpool.tile([H, 4 * R], F32, tag="t")
                for dst, src, eng in ((sp2, sp_d, nc.sync), (tp2, tp_d, nc.gpsimd),
                                      (t2, t_d, nc.gpsimd)):
                    eng.dma_start(
                        out=dst[:, :].rearrange("p (b hh c) -> p b hh c",
                                                b=2, hh=2, c=R),
                        in_=src[b:b + 2].rearrange("b (hh p) c -> p b hh c", p=H),
                    )
            m = b % 2
            sp_t = sp2[:, 2 * R * m: 2 * R * m + 2 * R]
            tp_t = tp2[:, 2 * R * m: 2 * R * m + 2 * R]
            t_t = t2[:, 2 * R * m: 2 * R * m + 2 * R]

            x1 = tpool.tile([H, 2 * R], F32, tag="x1")
            if fuse_w34:
                # w4 == -w3 for this model: w4*Tp + w3*T = w3*(T - Tp)
                nc.vector.tensor_tensor(
                    out=x1[:, :], in0=t_t[:, :], in1=tp_t[:, :].bitcast(F32),
                    op=mybir.AluOpType.subtract,
                )
                nc.vector.tensor_mul(x1[:, :], x1[:, :], w3[:, :])
            else:
                nc.vector.tensor_mul(x1[:, :], w4[:, :], tp_t[:, :].bitcast(F32))
                x2 = tpool.tile([H, 2 * R], F32, tag="x2")
                nc.vector.tensor_mul(x2[:, :], w3[:, :], t_t[:, :])
                nc.vector.tensor_add(x1[:, :], x1[:, :], x2[:, :])
            a_t = tpool.tile([H, 2 * R], F32R, tag="a")
            ps = pmm.tile([H, 2 * R], F32, tag="wide")
            for mh in range(2):
                for kh in range(2):
                    nc.tensor.matmul(
                        ps[:, R * mh: R * mh + R],
                        w1t[:, R * kh + H * mh: R * kh + H * mh + H],
                        sp_t[:, R * kh: R * kh + R],
                        start=(kh == 0), stop=False,
                    )
                for kh in range(2):
                    nc.tensor.matmul(
                        ps[:, R * mh: R * mh + R],
                        w2t[:, R * kh + H * mh: R * kh + H * mh + H],
                        tp_t[:, R * kh: R * kh + R],
                        start=False, stop=(kh == 1),
                    )
            nc.vector.tensor_add(a_t[:, :], ps[:, :], x1[:, :])
            at_t = keep.tile([H, 2 * R], F32R, tag=f"at{b}")
            _transpose_256(nc, ptr, at_t, a_t, ident, alt=b % 2)
            s0_t = keep.tile([H, 2 * R], F32R, tag=f"s0_{b}")
            _mm256_wide(nc, pmm, s0_t, a_t, a_t, alt=b % 2)  # G
            s1_t = keep.tile([H, 2 * R], F32R, tag=f"s1_{b}")
            v_t = keep.tile([H, 2 * LA], F32R, tag=f"v{b}")
            nc.vector.tensor_copy(v_t[:, :], seed[:, :].bitcast(F32))
            ats.append(at_t); s0s.append(s0_t); s1s.append(s1_t); vs.append(v_t)

        # ---- phase C: chain G2..G16, step-major so the PE never stalls ----
        for b in range(bpc):                               # G2 = (G^2)*2^-21
            _mm256_wide(nc, pmm, s1s[b], s0s[b], s0s[b], scale=G2_SCALE,
                        alt=b % 2)
        for b in range(bpc):                               # G4
            _mm256_wide(nc, pmm, s0s[b], s1s[b], s1s[b], alt=b % 2)
        for b in range(bpc):                               # G8
            _mm256_wide(nc, pmm, s1s[b], s0s[b], s0s[b], alt=b % 2)
        for b in range(bpc):                               # G16 -> hs = s0s
            _mm256_wide(nc, pmm, s0s[b], s1s[b], s1s[b], alt=b % 2)
        hs = s0s

        # ---- phase R: rungs, the 4 packs' NS chains interleaved ----
        a_c, b_c, c_c = MUO
        for r in range(n_rungs):
            mbds, cts, yts = [], [], []
            for p in range(n_pack):
                mbd = sone.tile([H, H], F32, tag=f"mbd{p}")
                nc.any.memset(mbd[:, :], 0.0)
                mbds.append(mbd)
            for b in range(bpc):
                p, sl = b // GRP, (b % GRP) * 32
                yt_ps = psm.tile([LA, R], F32, tag="sm")
                for kh in range(2):
                    nc.tensor.matmul(
                        yt_ps[:, :],
                        vs[b][:, LA * kh: LA * kh + LA],
                        hs[b][:, R * kh: R * kh + R],
                        start=(kh == 0), stop=(kh == 1),
                    )
                yt_t = sone.tile([LA, R], F32, tag=f"ytt{b}")
                if b % 2 == 0:
                    nc.vector.tensor_copy(yt_t[:, :], yt_ps[:, :])
                else:
                    nc.scalar.mul(yt_t[:, :], yt_ps[:, :], 1.0)
                y_t = spool.tile([H, 2 * LA], F32, tag="yy")
                tr_ps = ptr.tile([H, 2 * LA], F32, tag="tr")
                for hh in range(2):
                    nc.tensor.transpose(
                        tr_ps[:, LA * hh: LA * hh + LA],
                        yt_t[:, H * hh: H * hh + H],
                        ident[:LA, :LA],
                    )
                nc.vector.tensor_copy(y_t[:, :], tr_ps[:, :])
                # gram into the pack's block-diag tile
                m_ps = psb.tile([H, H], F32, tag="smb")
                for kh in range(2):
                    nc.tensor.matmul(
                        m_ps[sl:sl + LA, sl:sl + LA],
                        y_t[:, LA * kh: LA * kh + LA],
                        y_t[:, LA * kh: LA * kh + LA],
                        start=(kh == 0), stop=(kh == 1),
                        tile_position=(0, sl),
                    )
                nc.vector.tensor_copy(
                    mbds[p][sl:sl + LA, sl:sl + LA], m_ps[sl:sl + LA, sl:sl + LA]
                )
                yts.append(yt_t)

            # trace normalization, p-interleaved
            mns, rrvs = [], []
            for p in range(n_pack):
                masked = spool.tile([H, H], F32, tag="masked")
                nc.vector.tensor_mul(masked[:, :], mbds[p][:, :], ident[:, :])
                dr_ps = psb.tile([1, H], F32, tag="smb")
                nc.tensor.matmul(dr_ps[:, :], onescol[:, :], masked[:, :],
                                 start=True, stop=True)
                drow = spool.tile([1, H], F32, tag="drow")
                nc.vector.tensor_copy(drow[:, :], dr_ps[:, :])
                dg_ps = psb.tile([H, 1], F32, tag="smb")
                nc.tensor.transpose(dg_ps[:, :], drow[:, :], ident[:1, :1])
                diag = spool.tile([H, 1], F32, tag="diag")
                nc.vector.tensor_copy(diag[:, :], dg_ps[:, :])
                tr_ps = psb.tile([H, 1], F32, tag="smb")
                nc.tensor.matmul(tr_ps[:, :], blocktr[:, :], diag[:, :],
                                 start=True, stop=True)
                tre = spool.tile([H, 1], F32, tag="tre")
                nc.vector.tensor_scalar_add(tre[:, :], tr_ps[:, :], 1e-30)
                itv = spool.tile([H, 1], F32, tag="itv")
                nc.vector.reciprocal(itv[:, :], tre[:, :])
                sq = spool.tile([H, 1], F32, tag="sq")
                nc.scalar.activation(
                    sq[:, :], tre[:, :], mybir.ActivationFunctionType.Sqrt,
                )
                rrv = sone.tile([H, 1], F32, tag=f"rrv{p}")
                nc.vector.reciprocal(rrv[:, :], sq[:, :])
                mn = sone.tile([H, H], BF16, tag=f"mn{p}")
                nc.vector.tensor_scalar_mul(mn[:, :], mbds[p][:, :], itv[:, :])
                mns.append(mn); rrvs.append(rrv)

            # quintic NS, steps interleaved across the 4 packs
            mcurs = list(mns)
            cts = [sone.tile([H, H], BF16, tag=f"ct{p}", name=f"ct{p}")
                   for p in range(n_pack)]
            for st in range(ns_steps):
                m2_pss, csts = [], []
                for p in range(n_pack):
                    m2_ps = psb.tile([H, H], F32, tag="smb")
                    nc.tensor.matmul(m2_ps[:, :], mcurs[p][:, :], mcurs[p][:, :],
                                     start=True, stop=True)
                    m2_pss.append(m2_ps)
                for p in range(n_pack):
                    cst = sone.tile([H, H], BF16, tag=f"cst{p}")
                    nc.vector.tensor_scalar_mul(
                        cst[:, :], mcurs[p][:, :].bitcast(BF16), b_c)
                    nc.vector.tensor_add(cst[:, :], cst[:, :], eyema_bf[:, :])
                    m2s = spool.tile([H, H], BF16, tag="m2s")
                    nc.scalar.mul(m2s[:, :], m2_pss[p][:, :], c_c)
                    nc.vector.tensor_add(cst[:, :], cst[:, :], m2s[:, :])
                    csts.append(cst)
                if st < ns_steps - 1:
                    cms = []
                    for p in range(n_pack):
                        cm_ps = psb.tile([H, H], F32, tag="smb")
                        nc.tensor.matmul(cm_ps[:, :], csts[p][:, :], mcurs[p][:, :],
                                         start=True, stop=True)
                        cm = spool.tile([H, H], BF16, tag=f"cm{p}")
                        nc.vector.tensor_copy(cm[:, :], cm_ps[:, :])
                        cms.append(cm)
                    for p in range(n_pack):
                        mn2_ps = psb.tile([H, H], F32, tag="smb")
                        nc.tensor.matmul(mn2_ps[:, :], cms[p][:, :], csts[p][:, :],
                                         start=True, stop=True)
                        mnew = sone.tile([H, H], BF16, tag=f"mnew{p}_{st}")
                        nc.vector.tensor_copy(mnew[:, :], mn2_ps[:, :])
                        mcurs[p] = mnew
                for p in range(n_pack):
                    if st == 0:
                        nc.vector.tensor_copy(cts[p][:, :], csts[p][:, :])
                    else:
                        ct_ps = psb.tile([H, H], F32, tag="smb")
                        nc.tensor.matmul(ct_ps[:, :], cts[p][:, :], csts[p][:, :],
                                         start=True, stop=True)
                        nc.vector.tensor_copy(cts[p][:, :], ct_ps[:, :])
            for p in range(n_pack):
                nc.vector.tensor_scalar_mul(cts[p][:, :], cts[p][:, :], rrvs[p][:, :])

            # extract each pack's diag blocks to partition base 0 via an
            # identity matmul (operands share base sl; out lands at base 0)
            ct0s = []
            for p in range(n_pack):
                for kk in range(GRP):
                    sl = kk * 32
                    c0_ps = psb.tile([LA, LA], F32, tag="smb")
                    nc.tensor.matmul(
                        c0_ps[:, :],
                        ident_bf[sl:sl + LA, sl:sl + LA],
                        cts[p][sl:sl + LA, sl:sl + LA],
                        start=True, stop=True,
                        tile_position=(sl, 0),
                    )
                    ct0 = sone.tile([LA, LA], F32, tag=f"ct0_{p}_{kk}",
                                    name=f"ct0_{p}_{kk}")
                    nc.vector.tensor_copy(ct0[:, :], c0_ps[:, :])
                    ct0s.append(ct0)
            # apply: V_b = Y_b @ Ct0_b (all operands at base 0)
            for b in range(bpc):
                vp = ptr.tile([H, 2 * LA], F32, tag="tr")
                for hh in range(2):
                    nc.tensor.matmul(
                        vp[:, LA * hh: LA * hh + LA],
                        yts[b][:, H * hh: H * hh + H],
                        ct0s[b][:, :],
                        start=True, stop=True,
                    )
                nc.vector.tensor_copy(vs[b][:, :], vp[:, :])

        # ---- phase O: outputs Gh, Bh, Vt, Wpt (pipelines across b) ----
        for b in range(bpc):
            zt_ps = psm.tile([LA, R], F32, tag="sm")
            for kh in range(2):
                nc.tensor.matmul(
                    zt_ps[:, :],
                    vs[b][:, LA * kh: LA * kh + LA],
                    hs[b][:, R * kh: R * kh + R],
                    start=(kh == 0), stop=(kh == 1),
                )
            zt_t = spool.tile([LA, R], F32, tag="ztt")
            if b % 2 == 0:
                nc.vector.tensor_copy(zt_t[:, :], zt_ps[:, :])
            else:
                nc.scalar.mul(zt_t[:, :], zt_ps[:, :], 1.0)
            z_t = spool.tile([H, 2 * LA], F32, tag="zz")
            ztr_ps = ptr.tile([H, 2 * LA], F32, tag="tr")
            for hh in range(2):
                nc.tensor.transpose(
                    ztr_ps[:, LA * hh: LA * hh + LA],
                    zt_t[:, H * hh: H * hh + H],
                    ident[:LA, :LA],
                )
            nc.vector.tensor_copy(z_t[:, :], ztr_ps[:, :])
            ghbh_t = spool.tile([LA, 2 * LA], F32, tag="ghbh")
            gb_ps = psb.tile([LA, 2 * LA], F32, tag="smb")
            for kh in range(2):
                nc.tensor.matmul(
                    gb_ps[:, 0:LA],
                    z_t[:, LA * kh: LA * kh + LA],
                    vs[b][:, LA * kh: LA * kh + LA].bitcast(F32),
                    start=(kh == 0), stop=(kh == 1),
                )
            for kh in range(2):
                nc.tensor.matmul(
                    gb_ps[:, LA:2 * LA],
                    vs[b][:, LA * kh: LA * kh + LA].bitcast(F32),
                    vs[b][:, LA * kh: LA * kh + LA].bitcast(F32),
                    start=(kh == 0), stop=(kh == 1),
                )
            nc.vector.tensor_copy(ghbh_t[:, :], gb_ps[:, :])
            nc.sync.dma_start(out=ghbh_out[b], in_=ghbh_t[:, :])

            vw_t = spool.tile([LA, 2 * R], F32R, tag="vw")
            vtr_ps = psm.tile([LA, R], F32, tag="sm")
            for hh in range(2):
                nc.tensor.transpose(
                    vtr_ps[:, H * hh: H * hh + H],
                    vs[b][:, LA * hh: LA * hh + LA].bitcast(F32),
                    ident[:, :],
                )
            nc.vector.tensor_copy(vw_t[:, 0:R], vtr_ps[:, :])
            wpt_ps = psm.tile([LA, R], F32, tag="sm")
            for kh in range(2):
                nc.tensor.matmul(
                    wpt_ps[:, :],
                    vs[b][:, LA * kh: LA * kh + LA],
                    ats[b][:, R * kh: R * kh + R],
                    start=(kh == 0), stop=(kh == 1),
                )
            nc.vector.tensor_copy(vw_t[:, R:2 * R], wpt_ps[:, :])
            nc.sync.dma_start(out=vtwpt_out[b], in_=vw_t[:, :])
    nc.compile()
    return nc


def build_k2(bpc=BPC, stage=3):
    nc = bacc.Bacc("TRN2", target_bir_lowering=False)
    sp_d = nc.dram_tensor("sp", [bpc, R, R], F32, kind="ExternalInput")
    # packed per-matrix smalls: [vt | wpt | ms] = [32, 256+256+32]
    vwm_d = nc.dram_tensor("vwm", [bpc, LA, 2 * R + LA], F32R,
                           kind="ExternalInput")
    ident_d = nc.dram_tensor("ident", [H, H], F32, kind="ExternalInput")
    invc2_d = nc.dram_tensor("invc2", [1, 1022], F32, kind="ExternalInput")
    tpn_out = nc.dram_tensor("tpn_out", [bpc, R, R], F32, kind="ExternalOutput")
    spn_out = nc.dram_tensor("spn_out", [bpc, R, R], F32, kind="ExternalOutput")
    scr1 = nc.dram_tensor("scr1", [bpc, SHEAR_N], BF16)
    scrq = nc.dram_tensor("scrq", [bpc, QBUF_N], F32)
    npair = bpc // 2

    with tile.TileContext(nc) as tc, ExitStack() as ctx:
        cpool = ctx.enter_context(tc.tile_pool(name="consts", bufs=1))
        inpool = ctx.enter_context(tc.tile_pool(name="inp", bufs=2))
        tpool = ctx.enter_context(tc.tile_pool(name="trans", bufs=2))
        keep = ctx.enter_context(tc.tile_pool(name="keep", bufs=1))
        spool = ctx.enter_context(tc.tile_pool(name="small", bufs=3))
        pmm = ctx.enter_context(tc.tile_pool(name="pmm", bufs=2, space="PSUM"))
        ptr = ctx.enter_context(tc.tile_pool(name="ptr", bufs=2, space="PSUM"))
        psm = ctx.enter_context(tc.tile_pool(name="psm", bufs=2, space="PSUM"))

        ident = cpool.tile([H, H], F32)
        nc.sync.dma_start(out=ident[:, :], in_=ident_d[:, :])
        invc2 = cpool.tile([1, 1022], F32)
        nc.sync.dma_start(out=invc2[:, :], in_=invc2_d[:, :])
        ones = cpool.tile([H, 1], BF16)
        nc.any.memset(ones[:, :], 1.0)
        onesr = cpool.tile([1, H], BF16)
        nc.any.memset(onesr[:, :], 1.0)

        # per-pair zero-padded staging tiles (pads stay zero) + results
        m2zs, tpns, sps = [], [], []
        for q in range(npair):
            m2z = keep.tile([H, 2048], BF16, tag=f"m2z{q}", name=f"m2z{q}")
            nc.any.memset(m2z[:, :], 0.0)
            m2zs.append(m2z)
            tpn = keep.tile([H, 4 * R], F32, tag=f"tpn{q}", name=f"tpn{q}")
            tpns.append(tpn)
        # one DMA zeroes the shear-gap head [0,255) of every matrix slot
        nc.sync.dma_start(out=scr1[:, 0:255], in_=m2zs[0][0:bpc, 256:511])

        # ---- phase A: tpn + shear writes (pipelines across pairs) ----
        for q in range(npair):
            b0 = 2 * q
            sp2 = keep.tile([H, 4 * R], F32, tag=f"sp{q}", name=f"sp{q}")
            nc.sync.dma_start(
                out=sp2[:, :].rearrange("p (b hh c) -> p b hh c",
                                        b=2, hh=2, c=R),
                in_=sp_d[b0:b0 + 2].rearrange("b (hh p) c -> p b hh c", p=H),
            )
            sps.append(sp2)
            vwm2 = inpool.tile([LA, 2 * (2 * R + LA)], F32R, tag="vwm")
            nc.gpsimd.dma_start(
                out=vwm2[:, :].rearrange("p (b c) -> p b c", b=2),
                in_=vwm_d[b0:b0 + 2].rearrange("b p c -> p b c"),
            )
            for m in range(2):
                b = b0 + m
                W = 2 * R + LA
                sp_t = sp2[:, 2 * R * m: 2 * R * m + 2 * R]
                vt_t = vwm2[:, W * m: W * m + R]
                wpt_t = vwm2[:, W * m + R: W * m + 2 * R]
                ms_t = vwm2[:, W * m + 2 * R: W * m + 2 * R + LA]

                # P2 = Ms @ Vt   (Ms symmetric)
                p2_ps = psm.tile([LA, R], F32, tag="sm")
                nc.tensor.matmul(p2_ps[:, :], ms_t, vt_t, start=True, stop=True)
                p2_t = spool.tile([LA, R], F32R, tag="p2")
                nc.vector.tensor_copy(p2_t[:, :], p2_ps[:, :])

                # Tpnew = W' P2  (fp32r, both halves in one psum bank)
                tpn = tpns[q]
                ps2 = pmm.tile([H, 2 * R], F32, tag="wide")
                for hh in range(2):
                    nc.tensor.matmul(
                        ps2[:, R * hh: R * hh + R],
                        wpt_t[:, H * hh: H * hh + H],
                        p2_t[:, :],
                        start=True, stop=True,
                    )
                if m == 0:
                    nc.vector.tensor_copy(tpn[:, 0:2 * R], ps2[:, :])
                else:
                    nc.scalar.mul(tpn[:, 2 * R:4 * R], ps2[:, :], 1.0)
                if stage >= 2:
                    # M2 = 2*Tpnew - Sp (natural) into the bf16 staging
                    m2z = m2zs[q]
                    for hh in range(2):
                        o = 1024 * m + 512 * hh
                        nc.vector.tensor_scalar_mul(
                            m2z[:, o: o + R],
                            tpn[:, 2 * R * m + R * hh: 2 * R * m + R * hh + R],
                            2.0,
                        )
                        nc.vector.tensor_tensor(
                            out=m2z[:, o: o + R],
                            in0=m2z[:, o: o + R],
                            in1=sp_t[:, R * hh: R * hh + R],
                            op=mybir.AluOpType.subtract,
                        )
            # Tpnew out, one DMA per pair (natural layout)
            nc.sync.dma_start(
                out=tpn_out[b0:b0 + 2].rearrange("b (hh p) c -> p b hh c", p=H),
                in_=tpns[q][:, :].rearrange("p (b hh c) -> p b hh c",
                                            b=2, hh=2, c=R),
            )
            if stage < 2:
                continue
            # shear-write the pair, one DMA per row-half (3D APs balance)
            for hh in range(2):
                o = 255 + 511 * H * hh
                nc.gpsimd.dma_start(
                    out=scr1[b0:b0 + 2, o: o + 511 * H].rearrange(
                        "b (p f) -> p b f", p=H),
                    in_=m2zs[q][:, :].rearrange(
                        "p (b hh x) -> p b hh x", b=2, hh=2)[:, :, hh, 0:511],
                )

        # ---- phase B: diagonal sums -> periodic Q buffer (per pair) ----
        if stage >= 2:
            for q in range(npair):
                b0 = 2 * q
                shm = tpool.tile([H, 2044], BF16, tag="shm")
                for hh in range(2):
                    nc.gpsimd.dma_start(
                        out=shm[:, :].rearrange("p (b hh f) -> p b hh f",
                                                b=2, hh=2)[:, :, hh],
                        in_=scr1[b0:b0 + 2, 512 * H * hh: 512 * H * hh
                                 + 512 * H].rearrange(
                            "b (p f) -> p b f", p=H)[:, :, 0:511],
                    )
                avg = spool.tile([1, 1022], BF16, tag="avg")
                for m in range(2):
                    sums_ps = psm.tile([1, 511], F32, tag="sm3")
                    for hh in range(2):
                        nc.tensor.matmul(
                            sums_ps[:, :], ones[:, :],
                            shm[:, 1022 * m + 511 * hh: 1022 * m + 511 * hh + 511],
                            start=(hh == 0), stop=(hh == 1))
                    nc.vector.tensor_mul(avg[:, 511 * m: 511 * m + 511],
                                         sums_ps[:1, :],
                                         invc2[:, 511 * m: 511 * m + 511])
                avgb = spool.tile([H, 1022], F32, tag="avgb")
                for m in range(2):
                    avgb_ps = pmm.tile([H, 2 * R], F32, tag="wide")
                    nc.tensor.matmul(avgb_ps[:, 0:511], onesr[:, :],
                                     avg[:, 511 * m: 511 * m + 511],
                                     start=True, stop=True)
                    if m == 0:
                        nc.vector.tensor_copy(avgb[:, 0:511], avgb_ps[:, 0:511])
                    else:
                        nc.scalar.mul(avgb[:, 511:1022], avgb_ps[:, 0:511], 1.0)
                # periodic Q: 128 rows + 1 wrap row of avg, per pair
                nc.sync.dma_start(
                    out=scrq[b0:b0 + 2, 0: 511 * H].rearrange(
                        "b (p f) -> p b f", p=H),
                    in_=avgb[:, :].rearrange("p (b f) -> p b f", b=2),
                )
                nc.sync.dma_start(
                    out=scrq[b0:b0 + 2, 511 * H: 511 * (H + 1)],
                    in_=avgb[0:1, :],
                )

        # ---- phase C: Toeplitz + Spnew (per pair) ----
        if stage >= 3:
            for q in range(npair):
                b0 = 2 * q
                # toep_nat[p, f] = avg[f - p + 255] via mod-511 reads:
                # rows 0..127 from base 255, rows 128..255 from base 127
                ttN2 = tpool.tile([H, 4 * R], F32, tag="ttN2")
                for hh, base in ((0, 255), (1, 127)):
                    nc.gpsimd.dma_start(
                        out=ttN2[:, :].rearrange("p (b hh f) -> p b hh f",
                                                 b=2, hh=2, f=R)[:, :, hh],
                        in_=scrq[b0:b0 + 2, base: base + 510 * H].rearrange(
                            "b (p f) -> p b f", p=H, f=510)[:, :, 0:R],
                    )
                spn2 = tpool.tile([H, 4 * R], F32, tag="spn2")
                for m in range(2):
                    sp_t = sps[q][:, 2 * R * m: 2 * R * m + 2 * R]
                    # Spnew = Sp - Tpnew + toep_nat
                    o = 2 * R * m
                    nc.vector.tensor_tensor(
                        out=spn2[:, o: o + 2 * R], in0=sp_t,
                        in1=tpns[q][:, o: o + 2 * R],
                        op=mybir.AluOpType.subtract,
                    )
                    nc.vector.tensor_add(
                        spn2[:, o: o + 2 * R], spn2[:, o: o + 2 * R],
                        ttN2[:, o: o + 2 * R])
                nc.sync.dma_start(
                    out=spn_out[b0:b0 + 2].rearrange(
                        "b (hh p) c -> p b hh c", p=H),
                    in_=spn2[:, :].rearrange("p (b hh c) -> p b hh c",
                                             b=2, hh=2, c=R),
                )
    nc.compile()
    return nc


def _transpose_256f(nc, ptr_pool, out_t, in_t, ident):
    """out = in^T for a 256x256 [128,512] fp32 tile (4 PE transposes)."""
    for i in range(2):
        for j in range(2):
            ps = ptr_pool.tile([H, H], F32, tag="tr")
            nc.tensor.transpose(
                ps[:, :], in_t[:, R * j + H * i: R * j + H * i + H], ident[:, :]
            )
            nc.vector.tensor_copy(out_t[:, R * i + H * j: R * i + H * j + H], ps[:, :])


def _host_consts():
    ident = np.eye(H, dtype=np.float32)
    eyema = (MUO[0] * np.eye(H)).astype(np.float32)
    blocktr = np.zeros((H, H), np.float32)
    for g in range(4):
        blocktr[g * 32: g * 32 + LA, g * 32: g * 32 + 32] = 1.0
    i = np.arange(R, dtype=np.float32)[:, None]
    j = np.arange(LA, dtype=np.float32)[None, :]
    v0 = np.cos(0.37 * (i + 1) * (j + 1) + 0.11 * i).astype(np.float32)
    seed = np.concatenate([v0[0:H, :], v0[H:R, :]], axis=1)  # [128, 32]
    counts = (R - np.abs(np.arange(511) - 255)).astype(np.float32)
    invc = (1.0 / counts)[None, :].astype(np.float32)
    return ident, eyema, blocktr, seed, invc


def _host_bridge(gh, bh, Kv):
    """Robust whitened generalized eig; returns Ms = Z10 Z10^T per matrix."""
    n = gh.shape[0]
    ms = np.zeros((n, LA, LA), np.float32)
    for b in range(n):
        Gs = 0.5 * (gh[b] + gh[b].T).astype(np.float64)
        Bs = 0.5 * (bh[b] + bh[b].T).astype(np.float64)
        lb, Ub = np.linalg.eigh(Bs)
        lmax = max(float(lb.max()), 0.0)
        keep = lb > lmax * 1e-7 if lmax > 0 else lb > -1.0
        if not np.any(keep):
            continue
        Wh = Ub[:, keep] / np.sqrt(np.maximum(lb[keep], 1e-300))[None, :]
        Gw = Wh.T @ Gs @ Wh
        d, Qw = np.linalg.eigh(Gw)
        Z = Wh @ Qw[:, ::-1][:, :Kv]
        ms[b] = (Z @ Z.T).astype(np.float32)
    return ms


def _host_fallback(T, Tp, Sp, w1, w2, w3, w4, Kv):
    """Numpy implementation (used only if the device path fails)."""
    f32 = np.float32
    A = (np.einsum('rk,bkc->brc', w1, Sp) + np.einsum('rk,bkc->brc', w2, Tp)
         + w4[None] * Tp + w3[None] * T).astype(f32)
    G = np.einsum('brc,brd->bcd', A, A).astype(f32)
    d, q = np.linalg.eigh(G.astype(np.float64))
    Vs = q[:, :, ::-1][:, :, :Kv]
    AV = np.einsum('brc,bcl->brl', A.astype(np.float64), Vs)
    Tpnew = np.einsum('brl,bcl->brc', AV, Vs).astype(f32)
    m, n = R, R
    D = m + n - 1
    ii = np.arange(m)[:, None]; jj = np.arange(n)[None, :]
    dd = jj - ii + (m - 1)
    M2 = (2.0 * Tpnew - Sp).astype(f32)
    Z = np.zeros((M2.shape[0], m, D), f32)
    Z[:, ii, dd] = M2
    sums = Z.sum(axis=1)
    counts = (m - np.abs(np.arange(D) - (m - 1))).astype(f32)
    avg = sums / counts
    Spnew = (Sp - Tpnew + avg[:, dd]).astype(f32)
    return (T, Tpnew, Spnew)


_K1 = {}
_K2 = None


def _get_kernels(fuse_w34=True):
    global _K2
    if fuse_w34 not in _K1:
        _K1[fuse_w34] = build_k1(fuse_w34=fuse_w34)
    if _K2 is None:
        _K2 = build_k2()
    return _K1[fuse_w34], _K2


def _run_k2(Sp, vt_all, wpt_all, ms_all, nc2=None):
    ident, eyema, blocktr, seed, invc = _host_consts()
    if nc2 is None:
        nc2 = build_k2()
    vwm = np.concatenate([vt_all, wpt_all, ms_all], axis=2)  # [B, 16, 528]
    vwm = np.ascontiguousarray(vwm, dtype=np.float32)
    in_maps = []
    for c in range(N_CORES):
        sl = slice(c * BPC, (c + 1) * BPC)
        in_maps.append({
            "sp": Sp[sl], "vwm": vwm[sl], "ident": ident,
            "invc2": np.concatenate([invc, invc], axis=1),
        })
    r2 = run_bass_kernel_spmd(nc2, in_maps, list(range(N_CORES)))
    LAST_EXEC_NS[1] = r2.exec_time_ns
    res2 = r2.results
    Tpnew = np.concatenate([res2[c]["tpn_out"] for c in range(N_CORES)], axis=0)
    Spnew = np.concatenate([res2[c]["spn_out"] for c in range(N_CORES)], axis=0)
    return Tpnew, Spnew


def _kernel_device(T, Tp, Sp, w1, w2, w3, w4, Kv):
    ident, eyema, blocktr, seed, invc = _host_consts()
    w1t = np.ascontiguousarray(w1.T)
    w2t = np.ascontiguousarray(w2.T)
    fuse = bool(np.array_equal(w4, -w3))
    nc1, nc2 = _get_kernels(fuse_w34=fuse)
    in_maps1 = []
    for c in range(N_CORES):
        sl = slice(c * BPC, (c + 1) * BPC)
        in_maps1.append({
            "sp": Sp[sl], "tp": Tp[sl], "t": T[sl],
            "w1t": w1t, "w2t": w2t, "w3": w3, "w4": w4,
            "ident": ident, "eyema": eyema, "blocktr": blocktr, "seed": seed,
        })
    r1 = run_bass_kernel_spmd(nc1, in_maps1, list(range(N_CORES)))
    LAST_EXEC_NS[0] = r1.exec_time_ns
    res1 = r1.results
    ghbh = np.concatenate([res1[c]["ghbh_out"] for c in range(N_CORES)], axis=0)
    vtwpt = np.concatenate([res1[c]["vtwpt_out"] for c in range(N_CORES)], axis=0)
    gh, bh = ghbh[:, :, 0:LA], ghbh[:, :, LA:2 * LA]
    vt_all, wpt_all = vtwpt[:, :, 0:R], vtwpt[:, :, R:2 * R]
    ms_all = _host_bridge(gh, bh, Kv)
    Tpnew, Spnew = _run_k2(Sp, vt_all, wpt_all, ms_all, nc2=nc2)
    return (T, Tpnew, Spnew)


def _kernel_hybrid(T, Tp, Sp, w1, w2, w3, w4, Kv):
    """Host eigensolve for the subspace + device K2 for apply/averaging."""
    f32 = np.float32
    A = (np.einsum('rk,bkc->brc', w1, Sp) + np.einsum('rk,bkc->brc', w2, Tp)
         + w4[None] * Tp + w3[None] * T).astype(f32)
    G = np.einsum('brc,brd->bcd', A, A)
    d, q = np.linalg.eigh(G.astype(np.float64))
    Vs = q[:, :, ::-1][:, :, :Kv]                       # [B, 256, K]
    vt_all = np.zeros((B_FULL, LA, R), f32)
    vt_all[:, :Kv, :] = Vs.transpose(0, 2, 1).astype(f32)
    AV = np.einsum('brc,bcl->brl', A.astype(np.float64), Vs)
    wpt_all = np.zeros((B_FULL, LA, R), f32)
    wpt_all[:, :Kv, :] = AV.transpose(0, 2, 1).astype(f32)
    ms_all = np.zeros((B_FULL, LA, LA), f32)
    ms_all[:, :Kv, :Kv] = np.eye(Kv, dtype=f32)[None]
    Tpnew, Spnew = _run_k2(Sp, vt_all, wpt_all, ms_all)
    return (T, Tpnew, Spnew)


def kernel(T, Tp, Sp, w1, w2, w3, w4, K):
    T = np.ascontiguousarray(np.asarray(T, dtype=np.float32))
    Tp = np.ascontiguousarray(np.asarray(Tp, dtype=np.float32))
    Sp = np.ascontiguousarray(np.asarray(Sp, dtype=np.float32))
    w1 = np.asarray(w1, dtype=np.float32); w2 = np.asarray(w2, dtype=np.float32)
    w3 = np.asarray(w3, dtype=np.float32); w4 = np.asarray(w4, dtype=np.float32)
    Kv = int(np.asarray(K))
    try:
        return _kernel_device(T, Tp, Sp, w1, w2, w3, w4, Kv)
    except Exception:
        import traceback
        traceback.print_exc()
        print("K1 device path failed; host eigensolve + device K2")
    try:
        return _kernel_hybrid(T, Tp, Sp, w1, w2, w3, w4, Kv)
    except Exception:
        import traceback
        traceback.print_exc()
        print("hybrid path failed; full host fallback")
        return _host_fallback(T, Tp, Sp, w1, w2, w3, w4, Kv)


LAST_EXEC_NS = [None, None]


# revision 51
# speedup vs baseline: 2.3699x; 1.0239x over previous
"""Cadzow update (batched rank-K truncation + Toeplitz averaging) on 8 trn2 cores.

Data-parallel over the batch of 128 matrices (16 per core). Per matrix:
  A = w1@Sp + w2@Tp + w4*Tp + w3*T
  rank-K via oversampled subspace iteration + host Rayleigh-Ritz:
    K1 (device): G = A^T A; chain G2=(G^2*2^-21), G4, G8, G16 (fp32r matmuls);
      3 rungs V <- orth(G16 V) with a quintic Newton-Schulz Gram conditioner
      (4 matrices packed per 128x128 block-diag tile); ships per matrix
      Gh = V^T G16 V, Bh = V^T V (16x16), Vt = V^T and Wpt = (A V)^T.
    host: robust whitened generalized eig of (Gh, Bh); top-K selector
      Ms = Z10 Z10^T (16x16).
    K2 (device): Tpnew = Wpt^T Ms Vt (both orientations from the small
      factors); Spnew = Sp - Tpnew + avgdiag(2 Tpnew - Sp) where the
      diagonal averaging runs via a shear-DMA layout (diag sums by
      ones-matmul) and the Toeplitz broadcast is read back from a
      mod-511 periodic DRAM buffer with all-positive strides.

All big matmuls run as fp32r (~4x PE throughput at >=256-wide outputs);
the 16x16 Grams / Newton-Schulz stay fp32. Outputs are written in natural
layout (no 4-byte-granular transposed DMA anywhere).
"""
import os
import numpy as np
from contextlib import ExitStack

# The axon ntff profile hook (antenv.axon_hooks) is absent in this image;
# a set BASS_TRACE would crash run_bass_kernel_spmd, so clear it.
os.environ.pop("BASS_TRACE", None)

import concourse.bass as bass
import concourse.bacc as bacc
import concourse.mybir as mybir
from concourse import tile
from concourse.bass_utils import run_bass_kernel_spmd

F32 = mybir.dt.float32
F32R = mybir.dt.float32r
BF16 = mybir.dt.bfloat16
N_CORES = 8
B_FULL = 128
BPC = B_FULL // N_CORES          # 16 matrices per core
R = 256
LA = 32                          # subspace dim (oversampled, 4x32 pack)
H = 128                          # partitions
GRP = 4                          # matrices packed per 128x128 Gram tile
N_RUNGS = 1
NS_STEPS = 3
MUO = (3.4445, -4.7750, 2.0315)  # quintic NS coefficients
G2_SCALE = 2.0 ** -21

SHEAR_N = 512 * 257              # shear scratch elems per matrix
QBUF_N = 511 * 129               # periodic Toeplitz buffer elems per matrix


def _halfslc(hh, w=R):
    return slice(w * hh, w * hh + w)


def _load_256(nc, dst, src_b):
    """DRAM (256, X) -> SBUF [128, 2X] (row halves side by side)."""
    X = src_b.shape[-1]
    nc.sync.dma_start(out=dst[:, 0:X], in_=src_b[0:H, :])
    nc.sync.dma_start(out=dst[:, X:2 * X], in_=src_b[H:2 * H, :])


def _mm256_wide(nc, psum_pool, out_t, lhs_t, rhs_t, scale=None, alt=0):
    """out = L^T @ Rhs for 256x256 [128,512]-tiled operands (4 matmuls,
    both output halves in one psum bank, a single copy out)."""
    ps = psum_pool.tile([H, 2 * R], F32, tag="wide")
    for mh in range(2):
        for kh in range(2):
            nc.tensor.matmul(
                ps[:, R * mh: R * mh + R],
                lhs_t[:, R * kh + H * mh: R * kh + H * mh + H],
                rhs_t[:, R * kh: R * kh + R],
                start=(kh == 0), stop=(kh == 1),
            )
    if scale is None:
        if alt == 0:
            nc.vector.tensor_copy(out_t[:, :], ps[:, :])
        else:
            nc.scalar.mul(out_t[:, :], ps[:, :], 1.0)
    else:
        if alt == 0:
            nc.vector.tensor_scalar_mul(out_t[:, :], ps[:, :], scale)
        else:
            nc.scalar.mul(out_t[:, :], ps[:, :], scale)


def _transpose_256(nc, ptr_pool, out_t, in_t, ident, alt=0):
    """out = in^T for a 256x256 [128,512] tile (4 PE transposes, 2 copies)."""
    for i in range(2):
        ps = ptr_pool.tile([H, 2 * H], F32, tag="tr")
        for j in range(2):
            nc.tensor.transpose(
                ps[:, H * j: H * j + H],
                in_t[:, R * j + H * i: R * j + H * i + H].bitcast(F32),
                ident[:, :],
            )
        if (i + alt) % 2 == 0:
            nc.vector.tensor_copy(out_t[:, R * i: R * i + R], ps[:, :])
        else:
            nc.scalar.mul(out_t[:, R * i: R * i + R], ps[:, :], 1.0)


def build_k1(bpc=BPC, n_rungs=N_RUNGS, ns_steps=NS_STEPS, fuse_w34=True):
    nc = bacc.Bacc("TRN2", target_bir_lowering=False)
    sp_d = nc.dram_tensor("sp", [bpc, R, R], F32R, kind="ExternalInput")
    tp_d = nc.dram_tensor("tp", [bpc, R, R], F32R, kind="ExternalInput")
    t_d = nc.dram_tensor("t", [bpc, R, R], F32, kind="ExternalInput")
    w1t_d = nc.dram_tensor("w1t", [R, R], F32R, kind="ExternalInput")
    w2t_d = nc.dram_tensor("w2t", [R, R], F32R, kind="ExternalInput")
    w3_d = nc.dram_tensor("w3", [R, R], F32, kind="ExternalInput")
    w4_d = nc.dram_tensor("w4", [R, R], F32, kind="ExternalInput")
    ident_d = nc.dram_tensor("ident", [H, H], F32, kind="ExternalInput")
    eyema_d = nc.dram_tensor("eyema", [H, H], F32, kind="ExternalInput")  # MUO[0]*I
    blocktr_d = nc.dram_tensor("blocktr", [H, H], F32, kind="ExternalInput")
    seed_d = nc.dram_tensor("seed", [H, 2 * LA], F32R, kind="ExternalInput")
    ghbh_out = nc.dram_tensor("ghbh_out", [bpc, LA, 2 * LA], F32,
                              kind="ExternalOutput")
    vtwpt_out = nc.dram_tensor("vtwpt_out", [bpc, LA, 2 * R], F32R,
                               kind="ExternalOutput")

    n_pack = (bpc + GRP - 1) // GRP
    with tile.TileContext(nc) as tc, ExitStack() as ctx:
        cpool = ctx.enter_context(tc.tile_pool(name="consts", bufs=1))
        inpool = ctx.enter_context(tc.tile_pool(name="inp", bufs=2))
        tpool = ctx.enter_context(tc.tile_pool(name="trans", bufs=2))
        keep = ctx.enter_context(tc.tile_pool(name="keep", bufs=1))
        spool = ctx.enter_context(tc.tile_pool(name="small", bufs=2))
        sone = ctx.enter_context(tc.tile_pool(name="sone", bufs=1))
        pmm = ctx.enter_context(tc.tile_pool(name="pmm", bufs=2, space="PSUM"))
        ptr = ctx.enter_context(tc.tile_pool(name="ptr", bufs=2, space="PSUM"))
        psm = ctx.enter_context(tc.tile_pool(name="psm", bufs=2, space="PSUM"))
        psb = ctx.enter_context(tc.tile_pool(name="psb", bufs=2, space="PSUM"))

        w1t = cpool.tile([H, 2 * R], F32R); _load_256(nc, w1t, w1t_d)
        w2t = cpool.tile([H, 2 * R], F32R); _load_256(nc, w2t, w2t_d)
        w3 = cpool.tile([H, 2 * R], F32); _load_256(nc, w3, w3_d)
        w4 = cpool.tile([H, 2 * R], F32); _load_256(nc, w4, w4_d)
        ident = cpool.tile([H, H], F32)
        nc.sync.dma_start(out=ident[:, :], in_=ident_d[:, :])
        eyema = cpool.tile([H, H], F32)
        nc.sync.dma_start(out=eyema[:, :], in_=eyema_d[:, :])
        blocktr = cpool.tile([H, H], F32)
        nc.sync.dma_start(out=blocktr[:, :], in_=blocktr_d[:, :])
        seed = cpool.tile([H, 2 * LA], F32R)
        nc.sync.dma_start(out=seed[:, :], in_=seed_d[:, :])
        onescol = cpool.tile([H, 1], F32)
        nc.any.memset(onescol[:, :], 1.0)
        eyema_bf = cpool.tile([H, H], BF16)
        nc.vector.tensor_copy(eyema_bf[:, :], eyema[:, :])
        ident_bf = cpool.tile([H, H], BF16)
        nc.vector.tensor_copy(ident_bf[:, :], ident[:, :])

        ats, s0s, s1s, vs = [], [], [], []
        # ---- phase L: loads, A, A^T, G (per matrix; pipelines across b) ----
        sp2 = tp2 = t2 = None
        for b in range(bpc):
            if b % 2 == 0:
                # one DMA per tensor loads a PAIR of matrices [128, 1024]
                sp2 = inpool.tile([H, 4 * R], F32R, tag="sp")
                tp2 = inpool.tile([H, 4 * R], F32R, tag="tp")
                t2 = inpool.tile([H, 4 * R], F32, tag="t")
                for dst, src, eng in ((sp2, sp_d, nc.sync), (tp2, tp_d, nc.gpsimd),
                                      (t2, t_d, nc.gpsimd)):
                    eng.dma_start(
                        out=dst[:, :].rearrange("p (b hh c) -> p b hh c",
                                                b=2, hh=2, c=R),
                        in_=src[b:b + 2].rearrange("b (hh p) c -> p b hh c", p=H),
                    )
            m = b % 2
            sp_t = sp2[:, 2 * R * m: 2 * R * m + 2 * R]
            tp_t = tp2[:, 2 * R * m: 2 * R * m + 2 * R]
            t_t = t2[:, 2 * R * m: 2 * R * m + 2 * R]

            x1 = tpool.tile([H, 2 * R], F32, tag="x1")
            if fuse_w34:
                # w4 == -w3 for this model: w4*Tp + w3*T = w3*(T - Tp)
                nc.vector.tensor_tensor(
                    out=x1[:, :], in0=t_t[:, :], in1=tp_t[:, :].bitcast(F32),
                    op=mybir.AluOpType.subtract,
                )
                nc.vector.tensor_mul(x1[:, :], x1[:, :], w3[:, :])
            else:
                nc.vector.tensor_mul(x1[:, :], w4[:, :], tp_t[:, :].bitcast(F32))
                x2 = tpool.tile([H, 2 * R], F32, tag="x2")
                nc.vector.tensor_mul(x2[:, :], w3[:, :], t_t[:, :])
                nc.vector.tensor_add(x1[:, :], x1[:, :], x2[:, :])
            a_t = tpool.tile([H, 2 * R], F32R, tag="a")
            ps = pmm.tile([H, 2 * R], F32, tag="wide")
            for mh in range(2):
                for kh in range(2):
                    nc.tensor.matmul(
                        ps[:, R * mh: R * mh + R],
                        w1t[:, R * kh + H * mh: R * kh + H * mh + H],
                        sp_t[:, R * kh: R * kh + R],
                        start=(kh == 0), stop=False,
                    )
                for kh in range(2):
                    nc.tensor.matmul(
                        ps[:, R * mh: R * mh + R],
                        w2t[:, R * kh + H * mh: R * kh + H * mh + H],
                        tp_t[:, R * kh: R * kh + R],
                        start=False, stop=(kh == 1),
                    )
            nc.vector.tensor_add(a_t[:, :], ps[:, :], x1[:, :])
            at_t = keep.tile([H, 2 * R], F32R, tag=f"at{b}")
            _transpose_256(nc, ptr, at_t, a_t, ident, alt=b % 2)
            s0_t = keep.tile([H, 2 * R], F32R, tag=f"s0_{b}")
            _mm256_wide(nc, pmm, s0_t, a_t, a_t, alt=b % 2)  # G
            s1_t = keep.tile([H, 2 * R], F32R, tag=f"s1_{b}")
            v_t = keep.tile([H, 2 * LA], F32R, tag=f"v{b}")
            nc.vector.tensor_copy(v_t[:, :], seed[:, :].bitcast(F32))
            ats.append(at_t); s0s.append(s0_t); s1s.append(s1_t); vs.append(v_t)

        # ---- phase C: chain G2..G16, step-major so the PE never stalls ----
        for b in range(bpc):                               # G2 = (G^2)*2^-21
            _mm256_wide(nc, pmm, s1s[b], s0s[b], s0s[b], scale=G2_SCALE,
                        alt=b % 2)
        for b in range(bpc):                               # G4
            _mm256_wide(nc, pmm, s0s[b], s1s[b], s1s[b], alt=b % 2)
        for b in range(bpc):                               # G8
            _mm256_wide(nc, pmm, s1s[b], s0s[b], s0s[b], alt=b % 2)
        for b in range(bpc):                               # G16 -> hs = s0s
            _mm256_wide(nc, pmm, s0s[b], s1s[b], s1s[b], alt=b % 2)
        hs = s0s

        # ---- phase R: rungs, the 4 packs' NS chains interleaved ----
        a_c, b_c, c_c = MUO
        for r in range(n_rungs):
            mbds, cts, yts = [], [], []
            for p in range(n_pack):
                mbd = sone.tile([H, H], F32, tag=f"mbd{p}")
                nc.any.memset(mbd[:, :], 0.0)
                mbds.append(mbd)
            for b in range(bpc):
                p, sl = b // GRP, (b % GRP) * 32
                yt_ps = psm.tile([LA, R], F32, tag="sm")
                for kh in range(2):
                    nc.tensor.matmul(
                        yt_ps[:, :],
                        vs[b][:, LA * kh: LA * kh + LA],
                        hs[b][:, R * kh: R * kh + R],
                        start=(kh == 0), stop=(kh == 1),
                    )
                yt_t = sone.tile([LA, R], F32, tag=f"ytt{b}")
                if b % 2 == 0:
                    nc.vector.tensor_copy(yt_t[:, :], yt_ps[:, :])
                else:
                    nc.scalar.mul(yt_t[:, :], yt_ps[:, :], 1.0)
                y_t = spool.tile([H, 2 * LA], F32, tag="yy")
                tr_ps = ptr.tile([H, 2 * LA], F32, tag="tr")
                for hh in range(2):
                    nc.tensor.transpose(
                        tr_ps[:, LA * hh: LA * hh + LA],
                        yt_t[:, H * hh: H * hh + H],
                        ident[:LA, :LA],
                    )
                nc.vector.tensor_copy(y_t[:, :], tr_ps[:, :])
                # gram into the pack's block-diag tile
                m_ps = psb.tile([H, H], F32, tag="smb")
                for kh in range(2):
                    nc.tensor.matmul(
                        m_ps[sl:sl + LA, sl:sl + LA],
                        y_t[:, LA * kh: LA * kh + LA],
                        y_t[:, LA * kh: LA * kh + LA],
                        start=(kh == 0), stop=(kh == 1),
                        tile_position=(0, sl),
                    )
                nc.vector.tensor_copy(
                    mbds[p][sl:sl + LA, sl:sl + LA], m_ps[sl:sl + LA, sl:sl + LA]
                )
                yts.append(yt_t)

            # trace normalization, p-interleaved
            mns, rrvs = [], []
            for p in range(n_pack):
                masked = spool.tile([H, H], F32, tag="masked")
                nc.vector.tensor_mul(masked[:, :], mbds[p][:, :], ident[:, :])
                dr_ps = psb.tile([1, H], F32, tag="smb")
                nc.tensor.matmul(dr_ps[:, :], onescol[:, :], masked[:, :],
                                 start=True, stop=True)
                drow = spool.tile([1, H], F32, tag="drow")
                nc.vector.tensor_copy(drow[:, :], dr_ps[:, :])
                dg_ps = psb.tile([H, 1], F32, tag="smb")
                nc.tensor.transpose(dg_ps[:, :], drow[:, :], ident[:1, :1])
                diag = spool.tile([H, 1], F32, tag="diag")
                nc.vector.tensor_copy(diag[:, :], dg_ps[:, :])
                tr_ps = psb.tile([H, 1], F32, tag="smb")
                nc.tensor.matmul(tr_ps[:, :], blocktr[:, :], diag[:, :],
                                 start=True, stop=True)
                tre = spool.tile([H, 1], F32, tag="tre")
                nc.vector.tensor_scalar_add(tre[:, :], tr_ps[:, :], 1e-30)
                itv = spool.tile([H, 1], F32, tag="itv")
                nc.vector.reciprocal(itv[:, :], tre[:, :])
                sq = spool.tile([H, 1], F32, tag="sq")
                nc.scalar.activation(
                    sq[:, :], tre[:, :], mybir.ActivationFunctionType.Sqrt,
                )
                rrv = sone.tile([H, 1], F32, tag=f"rrv{p}")
                nc.vector.reciprocal(rrv[:, :], sq[:, :])
                mn = sone.tile([H, H], BF16, tag=f"mn{p}")
                nc.vector.tensor_scalar_mul(mn[:, :], mbds[p][:, :], itv[:, :])
                mns.append(mn); rrvs.append(rrv)

            # quintic NS, steps interleaved across the 4 packs
            mcurs = list(mns)
            cts = [sone.tile([H, H], BF16, tag=f"ct{p}", name=f"ct{p}")
                   for p in range(n_pack)]
            for st in range(ns_steps):
                m2_pss, csts = [], []
                for p in range(n_pack):
                    m2_ps = psb.tile([H, H], F32, tag="smb")
                    nc.tensor.matmul(m2_ps[:, :], mcurs[p][:, :], mcurs[p][:, :],
                                     start=True, stop=True)
                    m2_pss.append(m2_ps)
                for p in range(n_pack):
                    cst = sone.tile([H, H], BF16, tag=f"cst{p}")
                    nc.vector.tensor_scalar_mul(
                        cst[:, :], mcurs[p][:, :].bitcast(BF16), b_c)
                    nc.vector.tensor_add(cst[:, :], cst[:, :], eyema_bf[:, :])
                    m2s = spool.tile([H, H], BF16, tag="m2s")
                    nc.scalar.mul(m2s[:, :], m2_pss[p][:, :], c_c)
                    nc.vector.tensor_add(cst[:, :], cst[:, :], m2s[:, :])
                    csts.append(cst)
                if st < ns_steps - 1:
                    cms = []
                    for p in range(n_pack):
                        cm_ps = psb.tile([H, H], F32, tag="smb")
                        nc.tensor.matmul(cm_ps[:, :], csts[p][:, :], mcurs[p][:, :],
                                         start=True, stop=True)
                        cm = spool.tile([H, H], BF16, tag=f"cm{p}")
                        nc.vector.tensor_copy(cm[:, :], cm_ps[:, :])
                        cms.append(cm)
                    for p in range(n_pack):
                        mn2_ps = psb.tile([H, H], F32, tag="smb")
                        nc.tensor.matmul(mn2_ps[:, :], cms[p][:, :], csts[p][:, :],
                                         start=True, stop=True)
                        mnew = sone.tile([H, H], BF16, tag=f"mnew{p}_{st}")
                        nc.vector.tensor_copy(mnew[:, :], mn2_ps[:, :])
                        mcurs[p] = mnew
                for p in range(n_pack):
                    if st == 0:
                        nc.vector.tensor_copy(cts[p][:, :], csts[p][:, :])
                    else:
                        ct_ps = psb.tile([H, H], F32, tag="smb")
                        nc.tensor.matmul(ct_ps[:, :], cts[p][:, :], csts[p][:, :],
                                         start=True, stop=True)
                        nc.vector.tensor_copy(cts[p][:, :], ct_ps[:, :])
            for p in range(n_pack):
                nc.vector.tensor_scalar_mul(cts[p][:, :], cts[p][:, :], rrvs[p][:, :])

            # extract each pack's diag blocks to partition base 0 via an
            # identity matmul (operands share base sl; out lands at base 0)
            ct0s = []
            for p in range(n_pack):
                for kk in range(GRP):
                    sl = kk * 32
                    c0_ps = psb.tile([LA, LA], F32, tag="smb")
                    nc.tensor.matmul(
                        c0_ps[:, :],
                        ident_bf[sl:sl + LA, sl:sl + LA],
                        cts[p][sl:sl + LA, sl:sl + LA],
                        start=True, stop=True,
                        tile_position=(sl, 0),
                    )
                    ct0 = sone.tile([LA, LA], F32, tag=f"ct0_{p}_{kk}",
                                    name=f"ct0_{p}_{kk}")
                    nc.vector.tensor_copy(ct0[:, :], c0_ps[:, :])
                    ct0s.append(ct0)
            # apply: V_b = Y_b @ Ct0_b (all operands at base 0)
            for b in range(bpc):
                vp = ptr.tile([H, 2 * LA], F32, tag="tr")
                for hh in range(2):
                    nc.tensor.matmul(
                        vp[:, LA * hh: LA * hh + LA],
                        yts[b][:, H * hh: H * hh + H],
                        ct0s[b][:, :],
                        start=True, stop=True,
                    )
                nc.vector.tensor_copy(vs[b][:, :], vp[:, :])

        # ---- phase O: outputs Gh, Bh, Vt, Wpt (pipelines across b) ----
        for b in range(bpc):
            zt_ps = psm.tile([LA, R], F32, tag="sm")
            for kh in range(2):
                nc.tensor.matmul(
                    zt_ps[:, :],
                    vs[b][:, LA * kh: LA * kh + LA],
                    hs[b][:, R * kh: R * kh + R],
                    start=(kh == 0), stop=(kh == 1),
                )
            zt_t = spool.tile([LA, R], F32, tag="ztt")
            if b % 2 == 0:
                nc.vector.tensor_copy(zt_t[:, :], zt_ps[:, :])
            else:
                nc.scalar.mul(zt_t[:, :], zt_ps[:, :], 1.0)
            z_t = spool.tile([H, 2 * LA], F32, tag="zz")
            ztr_ps = ptr.tile([H, 2 * LA], F32, tag="tr")
            for hh in range(2):
                nc.tensor.transpose(
                    ztr_ps[:, LA * hh: LA * hh + LA],
                    zt_t[:, H * hh: H * hh + H],
                    ident[:LA, :LA],
                )
            nc.vector.tensor_copy(z_t[:, :], ztr_ps[:, :])
            if b % 2 == 0:
                ghbh2 = spool.tile([LA, 4 * LA], F32, tag="ghbh2")
                vw2 = spool.tile([LA, 4 * R], F32R, tag="vw2")
            gb_ps = psb.tile([LA, 2 * LA], F32, tag="smb")
            for kh in range(2):
                nc.tensor.matmul(
                    gb_ps[:, 0:LA],
                    z_t[:, LA * kh: LA * kh + LA],
                    vs[b][:, LA * kh: LA * kh + LA].bitcast(F32),
                    start=(kh == 0), stop=(kh == 1),
                )
            for kh in range(2):
                nc.tensor.matmul(
                    gb_ps[:, LA:2 * LA],
                    vs[b][:, LA * kh: LA * kh + LA].bitcast(F32),
                    vs[b][:, LA * kh: LA * kh + LA].bitcast(F32),
                    start=(kh == 0), stop=(kh == 1),
                )
            nc.vector.tensor_copy(
                ghbh2[:, 2 * LA * (b % 2): 2 * LA * (b % 2) + 2 * LA],
                gb_ps[:, :])
            if b % 2 == 1:
                nc.sync.dma_start(
                    out=ghbh_out[b - 1:b + 1].rearrange("b p c -> p b c"),
                    in_=ghbh2[:, :].rearrange("p (b c) -> p b c", b=2),
                )

            vo = 2 * R * (b % 2)
            vtr_ps = psm.tile([LA, R], F32, tag="sm")
            for hh in range(2):
                nc.tensor.transpose(
                    vtr_ps[:, H * hh: H * hh + H],
                    vs[b][:, LA * hh: LA * hh + LA].bitcast(F32),
                    ident[:, :],
                )
            nc.vector.tensor_copy(vw2[:, vo: vo + R], vtr_ps[:, :])
            wpt_ps = psm.tile([LA, R], F32, tag="sm")
            for kh in range(2):
                nc.tensor.matmul(
                    wpt_ps[:, :],
                    vs[b][:, LA * kh: LA * kh + LA],
                    ats[b][:, R * kh: R * kh + R],
                    start=(kh == 0), stop=(kh == 1),
                )
            nc.vector.tensor_copy(vw2[:, vo + R: vo + 2 * R], wpt_ps[:, :])
            if b % 2 == 1:
                nc.sync.dma_start(
                    out=vtwpt_out[b - 1:b + 1].rearrange("b p c -> p b c"),
                    in_=vw2[:, :].rearrange("p (b c) -> p b c", b=2),
                )
    nc.compile()
    return nc


def build_k2(bpc=BPC, stage=3):
    nc = bacc.Bacc("TRN2", target_bir_lowering=False)
    sp_d = nc.dram_tensor("sp", [bpc, R, R], F32, kind="ExternalInput")
    # packed per-matrix smalls: [vt | wpt | ms] = [32, 256+256+32]
    vwm_d = nc.dram_tensor("vwm", [bpc, LA, 2 * R + LA], F32R,
                           kind="ExternalInput")
    ident_d = nc.dram_tensor("ident", [H, H], F32, kind="ExternalInput")
    invc2_d = nc.dram_tensor("invc2", [1, 1022], F32, kind="ExternalInput")
    tpn_out = nc.dram_tensor("tpn_out", [bpc, R, R], F32, kind="ExternalOutput")
    spn_out = nc.dram_tensor("spn_out", [bpc, R, R], F32, kind="ExternalOutput")
    scr1 = nc.dram_tensor("scr1", [bpc, SHEAR_N], BF16)
    scrq = nc.dram_tensor("scrq", [bpc, QBUF_N], F32)
    npair = bpc // 2

    with tile.TileContext(nc) as tc, ExitStack() as ctx:
        cpool = ctx.enter_context(tc.tile_pool(name="consts", bufs=1))
        inpool = ctx.enter_context(tc.tile_pool(name="inp", bufs=2))
        tpool = ctx.enter_context(tc.tile_pool(name="trans", bufs=2))
        keep = ctx.enter_context(tc.tile_pool(name="keep", bufs=1))
        spool = ctx.enter_context(tc.tile_pool(name="small", bufs=3))
        pmm = ctx.enter_context(tc.tile_pool(name="pmm", bufs=2, space="PSUM"))
        ptr = ctx.enter_context(tc.tile_pool(name="ptr", bufs=2, space="PSUM"))
        psm = ctx.enter_context(tc.tile_pool(name="psm", bufs=2, space="PSUM"))

        ident = cpool.tile([H, H], F32)
        nc.sync.dma_start(out=ident[:, :], in_=ident_d[:, :])
        invc2 = cpool.tile([1, 1022], F32)
        nc.sync.dma_start(out=invc2[:, :], in_=invc2_d[:, :])
        ones = cpool.tile([H, 1], BF16)
        nc.any.memset(ones[:, :], 1.0)
        onesr = cpool.tile([1, H], BF16)
        nc.any.memset(onesr[:, :], 1.0)

        # per-pair zero-padded staging tiles (pads stay zero) + results
        m2zs, tpns, sps = [], [], []
        for q in range(npair):
            m2z = keep.tile([H, 2048], BF16, tag=f"m2z{q}", name=f"m2z{q}")
            nc.any.memset(m2z[:, :], 0.0)
            m2zs.append(m2z)
            tpn = keep.tile([H, 4 * R], F32, tag=f"tpn{q}", name=f"tpn{q}")
            tpns.append(tpn)
        # one DMA zeroes the shear-gap head [0,255) of every matrix slot
        nc.sync.dma_start(out=scr1[:, 0:255], in_=m2zs[0][0:bpc, 256:511])

        # ---- phase A: tpn + shear writes (pipelines across pairs) ----
        for q in range(npair):
            b0 = 2 * q
            sp2 = keep.tile([H, 4 * R], F32, tag=f"sp{q}", name=f"sp{q}")
            nc.sync.dma_start(
                out=sp2[:, :].rearrange("p (b hh c) -> p b hh c",
                                        b=2, hh=2, c=R),
                in_=sp_d[b0:b0 + 2].rearrange("b (hh p) c -> p b hh c", p=H),
            )
            sps.append(sp2)
            vwm2 = inpool.tile([LA, 2 * (2 * R + LA)], F32R, tag="vwm")
            nc.gpsimd.dma_start(
                out=vwm2[:, :].rearrange("p (b c) -> p b c", b=2),
                in_=vwm_d[b0:b0 + 2].rearrange("b p c -> p b c"),
            )
            for m in range(2):
                b = b0 + m
                W = 2 * R + LA
                sp_t = sp2[:, 2 * R * m: 2 * R * m + 2 * R]
                vt_t = vwm2[:, W * m: W * m + R]
                wpt_t = vwm2[:, W * m + R: W * m + 2 * R]
                ms_t = vwm2[:, W * m + 2 * R: W * m + 2 * R + LA]

                # P2 = Ms @ Vt   (Ms symmetric)
                p2_ps = psm.tile([LA, R], F32, tag="sm")
                nc.tensor.matmul(p2_ps[:, :], ms_t, vt_t, start=True, stop=True)
                p2_t = spool.tile([LA, R], F32R, tag="p2")
                nc.vector.tensor_copy(p2_t[:, :], p2_ps[:, :])

                # Tpnew = W' P2  (fp32r, both halves in one psum bank)
                tpn = tpns[q]
                ps2 = pmm.tile([H, 2 * R], F32, tag="wide")
                for hh in range(2):
                    nc.tensor.matmul(
                        ps2[:, R * hh: R * hh + R],
                        wpt_t[:, H * hh: H * hh + H],
                        p2_t[:, :],
                        start=True, stop=True,
                    )
                if m == 0:
                    nc.vector.tensor_copy(tpn[:, 0:2 * R], ps2[:, :])
                else:
                    nc.scalar.mul(tpn[:, 2 * R:4 * R], ps2[:, :], 1.0)
                if stage >= 2:
                    # M2 = 2*Tpnew - Sp (natural) into the bf16 staging
                    m2z = m2zs[q]
                    for hh in range(2):
                        o = 1024 * m + 512 * hh
                        nc.vector.tensor_scalar_mul(
                            m2z[:, o: o + R],
                            tpn[:, 2 * R * m + R * hh: 2 * R * m + R * hh + R],
                            2.0,
                        )
                        nc.vector.tensor_tensor(
                            out=m2z[:, o: o + R],
                            in0=m2z[:, o: o + R],
                            in1=sp_t[:, R * hh: R * hh + R],
                            op=mybir.AluOpType.subtract,
                        )
            # Tpnew out, one DMA per pair (natural layout)
            nc.sync.dma_start(
                out=tpn_out[b0:b0 + 2].rearrange("b (hh p) c -> p b hh c", p=H),
                in_=tpns[q][:, :].rearrange("p (b hh c) -> p b hh c",
                                            b=2, hh=2, c=R),
            )
            if stage < 2:
                continue
            # shear-write the pair, one DMA per row-half (3D APs balance)
            for hh in range(2):
                o = 255 + 511 * H * hh
                nc.gpsimd.dma_start(
                    out=scr1[b0:b0 + 2, o: o + 511 * H].rearrange(
                        "b (p f) -> p b f", p=H),
                    in_=m2zs[q][:, :].rearrange(
                        "p (b hh x) -> p b hh x", b=2, hh=2)[:, :, hh, 0:511],
                )

        # ---- phase B: diagonal sums -> periodic Q buffer (per pair) ----
        if stage >= 2:
            for q in range(npair):
                b0 = 2 * q
                shm = tpool.tile([H, 2044], BF16, tag="shm")
                for hh in range(2):
                    nc.gpsimd.dma_start(
                        out=shm[:, :].rearrange("p (b hh f) -> p b hh f",
                                                b=2, hh=2)[:, :, hh],
                        in_=scr1[b0:b0 + 2, 512 * H * hh: 512 * H * hh
                                 + 512 * H].rearrange(
                            "b (p f) -> p b f", p=H)[:, :, 0:511],
                    )
                avg = spool.tile([1, 1022], BF16, tag="avg")
                for m in range(2):
                    sums_ps = psm.tile([1, 511], F32, tag="sm3")
                    for hh in range(2):
                        nc.tensor.matmul(
                            sums_ps[:, :], ones[:, :],
                            shm[:, 1022 * m + 511 * hh: 1022 * m + 511 * hh + 511],
                            start=(hh == 0), stop=(hh == 1))
                    nc.vector.tensor_mul(avg[:, 511 * m: 511 * m + 511],
                                         sums_ps[:1, :],
                                         invc2[:, 511 * m: 511 * m + 511])
                avgb = spool.tile([H, 1022], F32, tag="avgb")
                for m in range(2):
                    avgb_ps = pmm.tile([H, 2 * R], F32, tag="wide")
                    nc.tensor.matmul(avgb_ps[:, 0:511], onesr[:, :],
                                     avg[:, 511 * m: 511 * m + 511],
                                     start=True, stop=True)
                    if m == 0:
                        nc.vector.tensor_copy(avgb[:, 0:511], avgb_ps[:, 0:511])
                    else:
                        nc.scalar.mul(avgb[:, 511:1022], avgb_ps[:, 0:511], 1.0)
                # periodic Q: 128 rows + 1 wrap row of avg, per pair
                nc.sync.dma_start(
                    out=scrq[b0:b0 + 2, 0: 511 * H].rearrange(
                        "b (p f) -> p b f", p=H),
                    in_=avgb[:, :].rearrange("p (b f) -> p b f", b=2),
                )
                nc.sync.dma_start(
                    out=scrq[b0:b0 + 2, 511 * H: 511 * (H + 1)],
                    in_=avgb[0:1, :],
                )

        # ---- phase C: Toeplitz + Spnew (per pair) ----
        if stage >= 3:
            for q in range(npair):
                b0 = 2 * q
                # toep_nat[p, f] = avg[f - p + 255] via mod-511 reads:
                # rows 0..127 from base 255, rows 128..255 from base 127
                ttN2 = tpool.tile([H, 4 * R], F32, tag="ttN2")
                for hh, base in ((0, 255), (1, 127)):
                    nc.gpsimd.dma_start(
                        out=ttN2[:, :].rearrange("p (b hh f) -> p b hh f",
                                                 b=2, hh=2, f=R)[:, :, hh],
                        in_=scrq[b0:b0 + 2, base: base + 510 * H].rearrange(
                            "b (p f) -> p b f", p=H, f=510)[:, :, 0:R],
                    )
                spn2 = tpool.tile([H, 4 * R], F32, tag="spn2")
                for m in range(2):
                    sp_t = sps[q][:, 2 * R * m: 2 * R * m + 2 * R]
                    # Spnew = Sp - Tpnew + toep_nat
                    o = 2 * R * m
                    nc.vector.tensor_tensor(
                        out=spn2[:, o: o + 2 * R], in0=sp_t,
                        in1=tpns[q][:, o: o + 2 * R],
                        op=mybir.AluOpType.subtract,
                    )
                    nc.vector.tensor_add(
                        spn2[:, o: o + 2 * R], spn2[:, o: o + 2 * R],
                        ttN2[:, o: o + 2 * R])
                nc.sync.dma_start(
                    out=spn_out[b0:b0 + 2].rearrange(
                        "b (hh p) c -> p b hh c", p=H),
                    in_=spn2[:, :].rearrange("p (b hh c) -> p b hh c",
                                             b=2, hh=2, c=R),
                )
    nc.compile()
    return nc


def _transpose_256f(nc, ptr_pool, out_t, in_t, ident):
    """out = in^T for a 256x256 [128,512] fp32 tile (4 PE transposes)."""
    for i in range(2):
        for j in range(2):
            ps = ptr_pool.tile([H, H], F32, tag="tr")
            nc.tensor.transpose(
                ps[:, :], in_t[:, R * j + H * i: R * j + H * i + H], ident[:, :]
            )
            nc.vector.tensor_copy(out_t[:, R * i + H * j: R * i + H * j + H], ps[:, :])


def _host_consts():
    ident = np.eye(H, dtype=np.float32)
    eyema = (MUO[0] * np.eye(H)).astype(np.float32)
    blocktr = np.zeros((H, H), np.float32)
    for g in range(4):
        blocktr[g * 32: g * 32 + LA, g * 32: g * 32 + 32] = 1.0
    i = np.arange(R, dtype=np.float32)[:, None]
    j = np.arange(LA, dtype=np.float32)[None, :]
    v0 = np.cos(0.37 * (i + 1) * (j + 1) + 0.11 * i).astype(np.float32)
    seed = np.concatenate([v0[0:H, :], v0[H:R, :]], axis=1)  # [128, 32]
    counts = (R - np.abs(np.arange(511) - 255)).astype(np.float32)
    invc = (1.0 / counts)[None, :].astype(np.float32)
    return ident, eyema, blocktr, seed, invc


def _host_bridge(gh, bh, Kv):
    """Robust whitened generalized eig; returns Ms = Z10 Z10^T per matrix."""
    n = gh.shape[0]
    ms = np.zeros((n, LA, LA), np.float32)
    for b in range(n):
        Gs = 0.5 * (gh[b] + gh[b].T).astype(np.float64)
        Bs = 0.5 * (bh[b] + bh[b].T).astype(np.float64)
        lb, Ub = np.linalg.eigh(Bs)
        lmax = max(float(lb.max()), 0.0)
        keep = lb > lmax * 1e-7 if lmax > 0 else lb > -1.0
        if not np.any(keep):
            continue
        Wh = Ub[:, keep] / np.sqrt(np.maximum(lb[keep], 1e-300))[None, :]
        Gw = Wh.T @ Gs @ Wh
        d, Qw = np.linalg.eigh(Gw)
        Z = Wh @ Qw[:, ::-1][:, :Kv]
        ms[b] = (Z @ Z.T).astype(np.float32)
    return ms


def _host_fallback(T, Tp, Sp, w1, w2, w3, w4, Kv):
    """Numpy implementation (used only if the device path fails)."""
    f32 = np.float32
    A = (np.einsum('rk,bkc->brc', w1, Sp) + np.einsum('rk,bkc->brc', w2, Tp)
         + w4[None] * Tp + w3[None] * T).astype(f32)
    G = np.einsum('brc,brd->bcd', A, A).astype(f32)
    d, q = np.linalg.eigh(G.astype(np.float64))
    Vs = q[:, :, ::-1][:, :, :Kv]
    AV = np.einsum('brc,bcl->brl', A.astype(np.float64), Vs)
    Tpnew = np.einsum('brl,bcl->brc', AV, Vs).astype(f32)
    m, n = R, R
    D = m + n - 1
    ii = np.arange(m)[:, None]; jj = np.arange(n)[None, :]
    dd = jj - ii + (m - 1)
    M2 = (2.0 * Tpnew - Sp).astype(f32)
    Z = np.zeros((M2.shape[0], m, D), f32)
    Z[:, ii, dd] = M2
    sums = Z.sum(axis=1)
    counts = (m - np.abs(np.arange(D) - (m - 1))).astype(f32)
    avg = sums / counts
    Spnew = (Sp - Tpnew + avg[:, dd]).astype(f32)
    return (T, Tpnew, Spnew)


_K1 = {}
_K2 = None


def _get_kernels(fuse_w34=True):
    global _K2
    if fuse_w34 not in _K1:
        _K1[fuse_w34] = build_k1(fuse_w34=fuse_w34)
    if _K2 is None:
        _K2 = build_k2()
    return _K1[fuse_w34], _K2


def _run_k2(Sp, vt_all, wpt_all, ms_all, nc2=None):
    ident, eyema, blocktr, seed, invc = _host_consts()
    if nc2 is None:
        nc2 = build_k2()
    vwm = np.concatenate([vt_all, wpt_all, ms_all], axis=2)  # [B, 16, 528]
    vwm = np.ascontiguousarray(vwm, dtype=np.float32)
    in_maps = []
    for c in range(N_CORES):
        sl = slice(c * BPC, (c + 1) * BPC)
        in_maps.append({
            "sp": Sp[sl], "vwm": vwm[sl], "ident": ident,
            "invc2": np.concatenate([invc, invc], axis=1),
        })
    r2 = run_bass_kernel_spmd(nc2, in_maps, list(range(N_CORES)))
    LAST_EXEC_NS[1] = r2.exec_time_ns
    res2 = r2.results
    Tpnew = np.concatenate([res2[c]["tpn_out"] for c in range(N_CORES)], axis=0)
    Spnew = np.concatenate([res2[c]["spn_out"] for c in range(N_CORES)], axis=0)
    return Tpnew, Spnew


def _kernel_device(T, Tp, Sp, w1, w2, w3, w4, Kv):
    ident, eyema, blocktr, seed, invc = _host_consts()
    w1t = np.ascontiguousarray(w1.T)
    w2t = np.ascontiguousarray(w2.T)
    fuse = bool(np.array_equal(w4, -w3))
    nc1, nc2 = _get_kernels(fuse_w34=fuse)
    in_maps1 = []
    for c in range(N_CORES):
        sl = slice(c * BPC, (c + 1) * BPC)
        in_maps1.append({
            "sp": Sp[sl], "tp": Tp[sl], "t": T[sl],
            "w1t": w1t, "w2t": w2t, "w3": w3, "w4": w4,
            "ident": ident, "eyema": eyema, "blocktr": blocktr, "seed": seed,
        })
    r1 = run_bass_kernel_spmd(nc1, in_maps1, list(range(N_CORES)))
    LAST_EXEC_NS[0] = r1.exec_time_ns
    res1 = r1.results
    ghbh = np.concatenate([res1[c]["ghbh_out"] for c in range(N_CORES)], axis=0)
    vtwpt = np.concatenate([res1[c]["vtwpt_out"] for c in range(N_CORES)], axis=0)
    gh, bh = ghbh[:, :, 0:LA], ghbh[:, :, LA:2 * LA]
    vt_all, wpt_all = vtwpt[:, :, 0:R], vtwpt[:, :, R:2 * R]
    ms_all = _host_bridge(gh, bh, Kv)
    Tpnew, Spnew = _run_k2(Sp, vt_all, wpt_all, ms_all, nc2=nc2)
    return (T, Tpnew, Spnew)


def _kernel_hybrid(T, Tp, Sp, w1, w2, w3, w4, Kv):
    """Host eigensolve for the subspace + device K2 for apply/averaging."""
    f32 = np.float32
    A = (np.einsum('rk,bkc->brc', w1, Sp) + np.einsum('rk,bkc->brc', w2, Tp)
         + w4[None] * Tp + w3[None] * T).astype(f32)
    G = np.einsum('brc,brd->bcd', A, A)
    d, q = np.linalg.eigh(G.astype(np.float64))
    Vs = q[:, :, ::-1][:, :, :Kv]                       # [B, 256, K]
    vt_all = np.zeros((B_FULL, LA, R), f32)
    vt_all[:, :Kv, :] = Vs.transpose(0, 2, 1).astype(f32)
    AV = np.einsum('brc,bcl->brl', A.astype(np.float64), Vs)
    wpt_all = np.zeros((B_FULL, LA, R), f32)
    wpt_all[:, :Kv, :] = AV.transpose(0, 2, 1).astype(f32)
    ms_all = np.zeros((B_FULL, LA, LA), f32)
    ms_all[:, :Kv, :Kv] = np.eye(Kv, dtype=f32)[None]
    Tpnew, Spnew = _run_k2(Sp, vt_all, wpt_all, ms_all)
    return (T, Tpnew, Spnew)


def kernel(T, Tp, Sp, w1, w2, w3, w4, K):
    T = np.ascontiguousarray(np.asarray(T, dtype=np.float32))
    Tp = np.ascontiguousarray(np.asarray(Tp, dtype=np.float32))
    Sp = np.ascontiguousarray(np.asarray(Sp, dtype=np.float32))
    w1 = np.asarray(w1, dtype=np.float32); w2 = np.asarray(w2, dtype=np.float32)
    w3 = np.asarray(w3, dtype=np.float32); w4 = np.asarray(w4, dtype=np.float32)
    Kv = int(np.asarray(K))
    try:
        return _kernel_device(T, Tp, Sp, w1, w2, w3, w4, Kv)
    except Exception:
        import traceback
        traceback.print_exc()
        print("K1 device path failed; host eigensolve + device K2")
    try:
        return _kernel_hybrid(T, Tp, Sp, w1, w2, w3, w4, Kv)
    except Exception:
        import traceback
        traceback.print_exc()
        print("hybrid path failed; full host fallback")
        return _host_fallback(T, Tp, Sp, w1, w2, w3, w4, Kv)


LAST_EXEC_NS = [None, None]
